# revision 28
# baseline (speedup 1.0000x reference)
"""Trainium2 Bass kernel: 4x EmbeddingBag(sum over 32 codes) + 3-layer MLP.

Data-parallel over 8 NeuronCores (batch 16384 -> 8 x 2048).  Embedding tables
are concatenated (proc offset by +100000), cast to bf16 and split into 5
chunks of <=32000 rows (so per-chunk row indices fit int16 for dma_gather),
each chunk followed by one zero row used as gather padding.  Table rows are
assigned to chunks by a quota-balancing greedy (plus a repair pass) so that
every (core, window, bag) cell's per-chunk lookup counts stay under rotated
multiples-of-128 quotas - minimizing the ceil-128 padding below.

Per core the 262144 lookups (4 bags x 2048 examples x 32 codes) are sorted by
(window of 128 examples, chunk, bag).  Each (win,ck,bag) segment is padded to
a multiple of 128 rows ("blocks") with zero-row fetches.  One dma_gather per
(win, ck) pulls all its blocks' rows (bf16, 256B each) into SBUF in
partition-fastest order.  Per block, the selection matrix E [128 rows x 128
examples] is generated ON-CHIP by the vector engine (tensor_scalar is_equal
of a constant iota row-tile against the block's per-row example ids "mids",
a per-window fp32 DMA), then one PE matmul per block accumulates the rows
into a per-(bag, win) PSUM tile [D=128, 128 examples] in fp32 - start=True on
the first block of each (win,bag), stop on the last.  Pad rows fetch the
chunk's zero row and carry a sentinel mid (no E column), so they add zero.

The MLP then runs per window in fp32: bag sums are already feature-major in
PSUM, copy to SBUF (ACT/DVE), layer1+layer2 feature-major (ACT applies
bias/relu on the PSUM->SBUF copy), layer3 uses the activations as lhsT to
emit example-major [128, 1000] directly (bias via a K=1 ones-row matmul PSUM
init), ACT sigmoid, output DMA'd bf16 on the ACT HWDGE queue (host converts
to fp32).  Weight preloads are emitted after window 0's gathers; the last two
windows' gather ops are split so the drain tail overlaps remaining gathers.

The Bass program structure is shared by all 8 cores (SPMD); per-op sizes are
the max over cores, deficit cores pad with zero-row gathers and sentinel mids.
"""

import numpy as np

B, L, D = 16384, 32, 128
DIAG_LEN, PROC_LEN, MED_LEN = 100000, 50000, 1000
N_CORES = 8
P = 128
CS = 32000          # chunk size (int16-addressable)
NCK = 5             # chunks
WIN = 128           # examples per window
SENT = 200.0        # mids sentinel (never equals iota 0..127; exact in bf16)
QUOTA = (7, 7, 7, 6, 6)  # per-(win,bag) chunk quotas in 128-blocks, rotated
PLACE = False       # example->window placement (didn't help; identity)


def _balance_chunks(rows_all, cell_all, v_cat, nwin):
    """Assign table rows to chunks so that per-(core,win,bag) chunk counts
    stay under rotated 128-multiple quotas (minimizing ceil-128 padding).

    rows_all/cell_all: per-lookup row id and cell id (c*nwin*4 + w*4 + b).
    Returns (asg [v_cat] chunk id, loc [v_cat] position within chunk).
    """
    n_cells = cell_all.max() + 1
    o = np.argsort(rows_all, kind="stable")
    rs, cells_s = rows_all[o], cell_all[o]
    row_start = np.searchsorted(rs, np.arange(v_cat + 1))
    cnts = np.diff(row_start)

    # per-(row, cell) occurrence counts, row-major
    key = rs * n_cells + cells_s
    ukey, uocc = np.unique(key, return_counts=True)
    urow = ukey // n_cells
    ucell = ukey % n_cells
    ustart = np.searchsorted(urow, np.arange(v_cat + 1))

    # quotas per (ck, cell): rotate QUOTA by (w*4+b) % NCK
    j = np.arange(n_cells) % (nwin * 4)
    q = np.array(QUOTA, np.int64) * P
    Q = np.empty((NCK, n_cells), np.int64)
    for ck in range(NCK):
        Q[ck] = q[(ck + j) % NCK]

    L = np.zeros((NCK, n_cells), np.int64)
    cap = np.full(NCK, CS, np.int64)
    asg = np.full(v_cat, -1, np.int64)
    # effective ceiling per (ck, cell): quota, ratcheted up by 128 whenever a
    # cell in the same (w,b,ck) group has already overflowed past it (the
    # extra block is paid once per group; later rows fill it for free).
    wb = j  # cell -> (w*4+b) group id
    n_grp = nwin * 4
    C = Q.copy()

    row_order = np.argsort(-cnts, kind="stable")
    BS = 512
    for i0 in range(0, v_cat, BS):
        br = row_order[i0 : i0 + BS]
        # flatten this batch's (row, cell, occ) entries
        ent_s = ustart[br]
        ent_e = ustart[br + 1]
        ent_n = ent_e - ent_s
        flat = np.concatenate([np.arange(s, e) for s, e in zip(ent_s, ent_e)]) \
            if ent_n.sum() else np.empty(0, np.int64)
        bounds = np.concatenate([[0], np.cumsum(ent_n)])
        bcell = ucell[flat]
        bocc = uocc[flat]
        # slack per (row-in-batch, ck) = min over row's cells of C-L-occ
        nb = br.size
        slack = np.full((NCK, nb), 1 << 30, np.int64)
        has = ent_n > 0
        red_idx = bounds[:-1][has]
        for ck in range(NCK):
            cs_ = C[ck, bcell] - L[ck, bcell] - bocc
            if red_idx.size:
                slack[ck, has] = np.minimum.reduceat(cs_, red_idx)
            slack[ck, ~has] = 1 << 30
            if cap[ck] <= 0:
                slack[ck, :] = -(1 << 30)
        choice = np.argmax(slack, axis=0)
        asg[br] = choice
        for ck in range(NCK):
            sel = choice == ck
            cap[ck] -= int(sel.sum())
            csel = np.repeat(sel, ent_n)
            np.add.at(L[ck], bcell[csel], bocc[csel])
            # ratchet ceilings: group ceiling = max over its cells of
            # ceil128(load), at least the quota
            gmax = np.zeros(n_grp, np.int64)
            np.maximum.at(gmax, wb, L[ck])
            gceil = -(-gmax // P) * P
            C[ck] = np.maximum(Q[ck], gceil[wb])

    # repair pass: groups (w,b,ck) barely over a 128 boundary -> move rows
    # contributing to the over-boundary cores into chunks with slack
    cell_rows_order = np.argsort(ucell, kind="stable")
    cell_start = np.searchsorted(ucell[cell_rows_order], np.arange(n_cells + 1))
    for _ in range(2):
        gmaxs = np.zeros((NCK, n_grp), np.int64)
        for ck in range(NCK):
            np.maximum.at(gmaxs[ck], wb, L[ck])
        over = gmaxs % P
        order = np.argsort(np.where(over > 0, over, 1 << 30).reshape(-1))
        moved = 0
        for flatg in order:
            ck, g = divmod(int(flatg), n_grp)
            exc = int(over[ck, g])
            if exc == 0 or exc > 48:
                break
            floor_l = gmaxs[ck, g] - exc
            # offending cells of this group
            gcells = np.nonzero(wb == g)[0]
            bad = gcells[L[ck, gcells] > floor_l]
            for cell in bad:
                need = int(L[ck, cell] - floor_l)
                ent = cell_rows_order[cell_start[cell] : cell_start[cell + 1]]
                cand = ent[asg[urow[ent]] == ck]
                # smallest contributors first
                cand = cand[np.argsort(uocc[cand], kind="stable")]
                for e in cand:
                    if need <= 0:
                        break
                    r = int(urow[e])
                    es, ee = int(ustart[r]), int(ustart[r + 1])
                    rc, ro = ucell[es:ee], uocc[es:ee]
                    for ck2 in range(NCK):
                        if ck2 == ck or cap[ck2] <= 0:
                            continue
                        if np.all(C[ck2, rc] - L[ck2, rc] >= ro):
                            asg[r] = ck2
                            L[ck, rc] -= ro
                            L[ck2, rc] += ro
                            cap[ck] += 1
                            cap[ck2] -= 1
                            need -= int(ro[np.nonzero(rc == cell)[0][0]])
                            moved += 1
                            break
        if moved == 0:
            break

    # positions within chunks (original row order)
    loc = np.zeros(v_cat, np.int64)
    for ck in range(NCK):
        sel = np.nonzero(asg == ck)[0]
        loc[sel] = np.arange(sel.size)
    return asg, loc


def _structure(counts):
    """Static program structure from per-core segment counts.

    counts: [n_cores, NWIN, NCK, 4] lookup counts per (win, ck, bag) segment.
    """
    n_cores, NWIN, NCK, NB_ = counts.shape
    cmax = counts.max(axis=0)  # [NWIN, NCK, 4]
    nb = -(-cmax // P)  # ceil -> blocks per segment
    nb[:, 0, :][nb[:, 0, :] == 0] = 1  # ck0 segments host the start=True matmul
    ops = []
    idx_off = 0
    blk_off = 0
    for w in range(NWIN):
        win_blocks = {bg: [] for bg in range(4)}
        win_ops = []
        for ck in range(NCK):
            op_blocks = []
            for bg in range(4):
                for b in range(int(nb[w, ck, bg])):
                    blk = [bg, False, False]
                    op_blocks.append(blk)
                    win_blocks[bg].append(blk)
            # split the last window's ops so its tail compute overlaps the
            # remaining gather halves (shrinks the end-of-kernel drain)
            parts = 3 if w == NWIN - 1 else (2 if w == NWIN - 2 else 1)
            per = -(-len(op_blocks) // parts)
            for p0 in range(0, len(op_blocks), per):
                pb = op_blocks[p0 : p0 + per]
                n_op = len(pb) * P
                win_ops.append(
                    dict(win=w, ck=ck, idx_off=idx_off, blk_off=blk_off,
                         nb=len(pb), n=n_op, blocks=pb)
                )
                idx_off += n_op
                blk_off += len(pb)
        for bg in range(4):
            assert win_blocks[bg], "every bag needs blocks in every window"
            win_blocks[bg][0][1] = True   # start
            win_blocks[bg][-1][2] = True  # stop
        ops.extend(win_ops)
    return dict(ops=ops, tot_idx=idx_off, tot_blk=blk_off, nb_arr=nb,
                NWIN=NWIN, NCK=NCK)


def host_prep(inputs, n_cores=N_CORES):
    import ml_dtypes

    bf16 = ml_dtypes.bfloat16

    diag = np.asarray(inputs["diag_emb"], np.float32)
    proc = np.asarray(inputs["proc_emb"], np.float32)
    v_diag, d = diag.shape
    tcat = np.concatenate([diag, proc], axis=0)
    v_cat = tcat.shape[0]
    assert NCK * CS >= v_cat

    gl = {
        "cd": np.asarray(inputs["diag_codes"], np.int64),
        "cp": np.asarray(inputs["proc_codes"], np.int64) + v_diag,
        "pd": np.asarray(inputs["prev_diag_codes"], np.int64),
        "pp": np.asarray(inputs["prev_proc_codes"], np.int64) + v_diag,
    }
    b_total, l_codes = gl["cd"].shape
    assert b_total % n_cores == 0
    bc = b_total // n_cores
    assert bc % WIN == 0
    NWIN = bc // WIN

    # per-core flat (row, example, bag) streams
    core_g, core_e, core_bag = [], [], []
    for c in range(n_cores):
        gs, bags = [], []
        for bi, name in enumerate(("cd", "cp", "pd", "pp")):
            g = gl[name][c * bc : (c + 1) * bc].reshape(-1)
            gs.append(g)
            bags.append(np.full(g.size, bi, np.int64))
        core_g.append(np.concatenate(gs))
        core_bag.append(np.concatenate(bags))
        core_e.append(np.tile(np.repeat(np.arange(bc, dtype=np.int64), l_codes), 4))

    # balance rows across chunks to minimize ceil-128 padding
    rows_all = np.concatenate(core_g)
    cell_all = np.concatenate(
        [
            c * (NWIN * 4) + (core_e[c] // WIN) * 4 + core_bag[c]
            for c in range(n_cores)
        ]
    )
    asg, lmap = _balance_chunks(rows_all, cell_all, v_cat, NWIN)

    # place examples into windows (core-local permutation) to flatten each
    # window's per-(bag, chunk) lookup counts, then re-balance rows with the
    # easier cell structure.  wmaps[c][e] = permuted example slot.
    wmaps = [np.arange(bc, dtype=np.int64) for _ in range(n_cores)]
    if PLACE:
        for c in range(n_cores):
            prof = np.zeros((bc, 4, NCK), np.int64)
            np.add.at(
                prof.reshape(-1),
                (core_e[c] * 4 + core_bag[c]) * NCK + asg[core_g[c]],
                1,
            )
            prof = prof.reshape(bc, 4 * NCK).astype(np.float64)
            target = prof.sum(axis=0) / bc  # per-example mean profile
            Wsum = np.zeros((NWIN, 4 * NCK), np.float64)
            n_w = np.zeros(NWIN, np.int64)
            # most extreme examples first
            eorder = np.argsort(-np.abs(prof - target).sum(axis=1), kind="stable")
            slot = np.empty(bc, np.int64)
            for e in eorder:
                dev = Wsum + prof[e] - target * (n_w + 1)[:, None]
                score = np.square(dev).sum(axis=1)
                score[n_w >= WIN] = np.inf
                w = int(np.argmin(score))
                slot[e] = w * WIN + n_w[w]
                Wsum[w] += prof[e]
                n_w[w] += 1
            wmaps[c] = slot
        cell_all = np.concatenate(
            [
                c * (NWIN * 4) + (wmaps[c][core_e[c]] // WIN) * 4 + core_bag[c]
                for c in range(n_cores)
            ]
        )
        asg, lmap = _balance_chunks(rows_all, cell_all, v_cat, NWIN)

    tbl_dev = np.zeros(((CS + 1) * NCK, d), bf16)
    tbl_dev[asg * (CS + 1) + lmap] = tcat.astype(bf16)

    # flat per-core lookup streams, sorted by (win, ck, bag)
    per_core = []
    counts = np.zeros((n_cores, NWIN, NCK, 4), np.int64)
    for c in range(n_cores):
        g, bag = core_g[c], core_bag[c]
        e = wmaps[c][core_e[c]]
        ck = asg[g]
        loc = lmap[g]
        win = e // WIN
        m = e % WIN
        seg = (win * NCK + ck) * 4 + bag
        order = np.argsort(seg, kind="stable")
        per_core.append((seg[order], loc[order], m[order]))
        np.add.at(counts[c].reshape(-1), seg, 1)

    st = _structure(counts)
    TOT_IDX, TOT_B = st["tot_idx"], st["tot_blk"]

    # static per-segment offsets
    seg_sizes = st["nb_arr"].reshape(-1) * P
    seg_off = np.concatenate([[0], np.cumsum(seg_sizes)])[:-1]

    in_maps = []
    iota_np = np.broadcast_to(
        np.arange(P, dtype=np.float32), (P, P)
    ).astype(bf16).copy()
    for c in range(n_cores):
        seg_s, loc_s, m_s = per_core[c]
        pos_in_seg = np.arange(seg_s.size) - np.concatenate(
            [[0], np.cumsum(np.bincount(seg_s, minlength=seg_sizes.size))]
        )[:-1][seg_s]
        pos = seg_off[seg_s] + pos_in_seg
        idx_flat = np.full(TOT_IDX, CS, np.int16)  # pad -> zero row
        idx_flat[pos] = loc_s.astype(np.int16)
        m_flat = np.full(TOT_IDX, SENT, np.float32)
        m_flat[pos] = m_s
        # pack gidx: position i -> [16k + i%16, i//16]
        blk = idx_flat.reshape(TOT_IDX // 16, 16).T
        gidx = np.tile(blk, (8, 1)).copy()
        # mids: position i -> [i%128, i//128], bf16
        mids = np.ascontiguousarray(m_flat.reshape(TOT_B, P).T)
        in_maps.append(dict(tbl=tbl_dev, gidx=gidx, mids=mids, iota=iota_np))

    w1t = np.ascontiguousarray(np.asarray(inputs["W1"], np.float32).T)
    w2t = np.ascontiguousarray(np.asarray(inputs["W2"], np.float32).T)
    w3t = np.ascontiguousarray(np.asarray(inputs["W3"], np.float32).T)
    b1 = np.ascontiguousarray(np.asarray(inputs["b1"], np.float32).reshape(-1, 1))
    b2 = np.ascontiguousarray(np.asarray(inputs["b2"], np.float32).reshape(-1, 1))
    b3 = np.ascontiguousarray(np.asarray(inputs["b3"], np.float32).reshape(1, -1))
    for im in in_maps:
        im.update(w1t=w1t, w2t=w2t, w3t=w3t, b1=b1, b2=b2, b3=b3)

    med = w3t.shape[1]
    cfg = dict(b_core=bc, med=med, v_dev=tbl_dev.shape[0], st=st, wmaps=wmaps)
    return in_maps, cfg


def assemble(results, cfg):
    """Concatenate per-core outputs, undoing the example->window placement
    permutation, and convert to fp32."""
    outs = []
    for c, r in enumerate(results):
        o = r["out"].astype(np.float32)
        outs.append(o[cfg["wmaps"][c]])
    return np.concatenate(outs, axis=0)


def build_nc(cfg):
    import concourse.bass as bass
    import concourse.mybir as mybir
    import concourse.tile as tile
    from concourse import bacc

    f32 = mybir.dt.float32
    bf = mybir.dt.bfloat16
    i16 = mybir.dt.int16
    AF = mybir.ActivationFunctionType
    EQ = mybir.AluOpType.is_equal

    bc, med, v_dev = cfg["b_core"], cfg["med"], cfg["v_dev"]
    st = cfg["st"]
    NWIN, NCK = st["NWIN"], st["NCK"]
    TOT_IDX, TOT_B = st["tot_idx"], st["tot_blk"]
    n_half = med // 2
    assert n_half <= 512

    nc = bacc.Bacc("TRN2", target_bir_lowering=False, debug=False,
                   enable_asserts=False, num_devices=N_CORES)

    tbl = nc.dram_tensor("tbl", [v_dev, D], bf, kind="ExternalInput").ap()
    gidx = nc.dram_tensor("gidx", [P, TOT_IDX // 16], i16, kind="ExternalInput").ap()
    mids = nc.dram_tensor("mids", [P, TOT_B], f32, kind="ExternalInput").ap()
    iota = nc.dram_tensor("iota", [P, P], bf, kind="ExternalInput").ap()
    w1t = nc.dram_tensor("w1t", [2 * D, D], f32, kind="ExternalInput").ap()
    w2t = nc.dram_tensor("w2t", [2 * D, 2 * D], f32, kind="ExternalInput").ap()
    w3t = nc.dram_tensor("w3t", [2 * D, med], f32, kind="ExternalInput").ap()
    b1 = nc.dram_tensor("b1", [D, 1], f32, kind="ExternalInput").ap()
    b2 = nc.dram_tensor("b2", [2 * D, 1], f32, kind="ExternalInput").ap()
    b3 = nc.dram_tensor("b3", [1, med], f32, kind="ExternalInput").ap()
    out = nc.dram_tensor("out", [bc, med], bf, kind="ExternalOutput").ap()

    ops_by_win = {}
    for op in st["ops"]:
        ops_by_win.setdefault(op["win"], []).append(op)

    with tile.TileContext(nc) as tc:
        with (
            tc.tile_pool(name="const", bufs=1) as cpool,
            tc.tile_pool(name="gi", bufs=3) as gi_pool,
            tc.tile_pool(name="mi", bufs=3) as mi_pool,
            tc.tile_pool(name="em", bufs=8) as em_pool,
            tc.tile_pool(name="gath", bufs=8) as gath_pool,
            tc.tile_pool(name="sT", bufs=8) as sT_pool,
            tc.tile_pool(name="acts", bufs=8) as act_pool,
            tc.tile_pool(name="osb", bufs=2) as out_pool,
            tc.tile_pool(name="spsum", bufs=4, space="PSUM") as s_psum,
            tc.tile_pool(name="mpsum", bufs=2, space="PSUM") as m_psum,
            tc.tile_pool(name="opsum", bufs=2, space="PSUM") as o_psum,
        ):
            iota_t = cpool.tile([P, P], bf, tag="iota")
            nc.sync.dma_start(iota_t[:], iota[:, :])

            consts = {}

            def load_consts():
                # Emitted after window 0's gather ops so the first gathers
                # aren't queued behind ~1.3MB of weight preloads.
                ones = cpool.tile([1, P], f32, tag="ones")
                nc.gpsimd.memset(ones[:], 1.0)
                w1t_k = []
                for k in range(2):
                    t = cpool.tile([D, D], f32, tag=f"w1t{k}")
                    nc.sync.dma_start(t[:], w1t[k * D : (k + 1) * D, :])
                    w1t_k.append(t)
                w2t_km = {}
                for k in range(2):
                    for mm in range(2):
                        t = cpool.tile([D, D], f32, tag=f"w2t{k}{mm}")
                        nc.sync.dma_start(
                            t[:], w2t[k * D : (k + 1) * D, mm * D : (mm + 1) * D]
                        )
                        w2t_km[(k, mm)] = t
                w3t_k = []
                for k in range(2):
                    t = cpool.tile([D, med], f32, tag=f"w3t{k}")
                    nc.sync.dma_start(t[:], w3t[k * D : (k + 1) * D, :])
                    w3t_k.append(t)
                b1_t = cpool.tile([D, 1], f32, tag="b1")
                nc.sync.dma_start(b1_t[:], b1[:, :])
                b2_t = []
                for mm in range(2):
                    t = cpool.tile([D, 1], f32, tag=f"b2{mm}")
                    nc.sync.dma_start(t[:], b2[mm * D : (mm + 1) * D, :])
                    b2_t.append(t)
                b3_t = cpool.tile([1, med], f32, tag="b3")
                nc.sync.dma_start(b3_t[:], b3[:, :])
                consts.update(ones=ones, w1t_k=w1t_k, w2t_km=w2t_km,
                              w3t_k=w3t_k, b1_t=b1_t, b2_t=b2_t, b3_t=b3_t)

            for rep in range(cfg.get("reps", 1)):
              for w in range(NWIN):
                s_ps = [s_psum.tile([D, WIN], f32, tag="s", name=f"s{rep}_{w}_{i}") for i in range(4)]
                wops = ops_by_win[w]
                w_idx_off = wops[0]["idx_off"]
                w_blk_off = wops[0]["blk_off"]
                w_n = sum(op["n"] for op in wops)
                w_nb = sum(op["nb"] for op in wops)
                gi = gi_pool.tile([P, w_n // 16], i16, tag="gi")
                if not cfg.get("skip_gi"):
                    nc.sync.dma_start(
                        gi[:],
                        gidx[:, w_idx_off // 16 : (w_idx_off + w_n) // 16],
                    )
                mi = mi_pool.tile([P, w_nb], f32, tag="mi")
                nc.sync.dma_start(
                    mi[:], mids[:, w_blk_off : w_blk_off + w_nb]
                )
                for op in wops:
                    n, nb = op["n"], op["nb"]
                    o16 = (op["idx_off"] - w_idx_off) // 16
                    ob0 = op["blk_off"] - w_blk_off
                    gt = gath_pool.tile([P, nb * D], bf, tag="gath")
                    if not cfg.get("skip_gather"):
                        nc.gpsimd.dma_gather(
                            out_ap=gt[:].rearrange("p (c d) -> p c d", d=D),
                            in_ap=tbl[
                                op["ck"] * (CS + 1) : (op["ck"] + 1) * (CS + 1), :
                            ],
                            idxs_ap=gi[:, o16 : o16 + n // 16],
                            num_idxs=n,
                            num_idxs_reg=n,
                            elem_size=D,
                            single_packet=False,
                        )
                    gt3 = gt[:].rearrange("p (c d) -> p c d", d=D)
                    em = em_pool.tile([P, P * nb], bf, tag="em")
                    for b in range(nb):
                        nc.vector.tensor_scalar(
                            em[:, b * P : (b + 1) * P],
                            iota_t[:],
                            mi[:, ob0 + b : ob0 + b + 1],
                            None,
                            EQ,
                        )
                    if cfg.get("skip_smm"):
                        continue
                    for b, (bg, start, stop) in enumerate(op["blocks"]):
                        nc.tensor.matmul(
                            s_ps[bg][:],
                            lhsT=gt3[:, b, :],
                            rhs=em[:, b * P : (b + 1) * P],
                            start=start,
                            stop=stop,
                            skip_group_check=True,
                        )
                if not consts:
                    load_consts()
                ones = consts["ones"]
                w1t_k, w2t_km = consts["w1t_k"], consts["w2t_km"]
                w3t_k = consts["w3t_k"]
                b1_t, b2_t, b3_t = consts["b1_t"], consts["b2_t"], consts["b3_t"]
                if cfg.get("skip_mlp"):
                    continue
                # bag sums (feature-major) PSUM -> SBUF on ACT
                sT = []
                for bg in range(4):
                    t = sT_pool.tile([D, P], f32, tag="sT", name=f"sT{w}_{bg}")
                    if bg % 2 == 0:
                        nc.scalar.activation(t[:], s_ps[bg][:], AF.Copy)
                    else:
                        nc.vector.tensor_copy(t[:], s_ps[bg][:])
                    sT.append(t)

                l1 = []
                for ka, kb in ((0, 1), (2, 3)):
                    pc = m_psum.tile([P, P], f32, tag="mp")
                    nc.tensor.matmul(
                        pc[:], lhsT=w1t_k[0][:], rhs=sT[ka][:], start=True, stop=False
                    )
                    nc.tensor.matmul(
                        pc[:], lhsT=w1t_k[1][:], rhs=sT[kb][:], start=False, stop=True
                    )
                    xt = act_pool.tile([D, P], f32, tag="l1")
                    nc.scalar.activation(xt[:], pc[:], AF.Identity, bias=b1_t[:])
                    l1.append(xt)

                hT = []
                for mm in range(2):
                    ph = m_psum.tile([P, P], f32, tag="mp")
                    nc.tensor.matmul(
                        ph[:], lhsT=w2t_km[(0, mm)][:], rhs=l1[0][:],
                        start=True, stop=False,
                    )
                    nc.tensor.matmul(
                        ph[:], lhsT=w2t_km[(1, mm)][:], rhs=l1[1][:],
                        start=False, stop=True,
                    )
                    ht = act_pool.tile([D, P], f32, tag="l2")
                    nc.scalar.activation(ht[:], ph[:], AF.Relu, bias=b2_t[mm][:])
                    hT.append(ht)

                ob = out_pool.tile([P, med], bf, tag="osb")
                for h_i in range(2):
                    n0, n1 = h_i * n_half, (h_i + 1) * n_half
                    po = o_psum.tile([P, n_half], f32, tag="op")
                    nc.tensor.matmul(
                        po[:], lhsT=ones[:1, :], rhs=b3_t[:1, n0:n1],
                        start=True, stop=False,
                    )
                    nc.tensor.matmul(
                        po[:], lhsT=hT[0][:], rhs=w3t_k[0][:, n0:n1],
                        start=False, stop=False,
                    )
                    nc.tensor.matmul(
                        po[:], lhsT=hT[1][:], rhs=w3t_k[1][:, n0:n1],
                        start=False, stop=True,
                    )
                    nc.scalar.activation(ob[:, n0:n1], po[:], AF.Sigmoid)
                nc.scalar.dma_start(out[w * P : (w + 1) * P, :], ob[:])

    nc.compile()
    return nc


def kernel(**inputs) -> np.ndarray:
    from concourse.bass_utils import run_bass_kernel_spmd

    in_maps, cfg = host_prep(inputs)
    nc = build_nc(cfg)
    res = run_bass_kernel_spmd(nc, in_maps, core_ids=list(range(N_CORES)))
    return assemble(res.results, cfg)


# revision 33
# speedup vs baseline: 1.0004x; 1.0004x over previous
"""Trainium2 Bass kernel: 4x EmbeddingBag(sum over 32 codes) + 3-layer MLP.

Data-parallel over 8 NeuronCores (batch 16384 -> 8 x 2048).  Embedding tables
are concatenated (proc offset by +100000), cast to bf16 and split into 5
chunks of <=32000 rows (so per-chunk row indices fit int16 for dma_gather),
each chunk followed by one zero row used as gather padding.  Table rows are
assigned to chunks by a quota-balancing greedy (plus a repair pass) so that
every (core, window, bag) cell's per-chunk lookup counts stay under rotated
multiples-of-128 quotas - minimizing the ceil-128 padding below.

Per core the 262144 lookups (4 bags x 2048 examples x 32 codes) are sorted by
(window of 128 examples, chunk, bag).  Each (win,ck,bag) segment is padded to
a multiple of 128 rows ("blocks") with zero-row fetches.  One dma_gather per
(win, ck) pulls all its blocks' rows (bf16, 256B each) into SBUF in
partition-fastest order.  Per block, the selection matrix E [128 rows x 128
examples] is generated ON-CHIP by the vector engine (tensor_scalar is_equal
of a constant iota row-tile against the block's per-row example ids "mids",
a per-window fp32 DMA), then one PE matmul per block accumulates the rows
into a per-(bag, win) PSUM tile [D=128, 128 examples] in fp32 - start=True on
the first block of each (win,bag), stop on the last.  Pad rows fetch the
chunk's zero row and carry a sentinel mid (no E column), so they add zero.

The MLP then runs per window in fp32: bag sums are already feature-major in
PSUM, copy to SBUF (ACT/DVE), layer1+layer2 feature-major (ACT applies
bias/relu on the PSUM->SBUF copy), layer3 uses the activations as lhsT to
emit example-major [128, 1000] directly (bias via a K=1 ones-row matmul PSUM
init), ACT sigmoid, output DMA'd bf16 on the ACT HWDGE queue (host converts
to fp32).  Weight preloads are emitted after window 0's gathers; the last two
windows' gather ops are split so the drain tail overlaps remaining gathers.

The Bass program structure is shared by all 8 cores (SPMD); per-op sizes are
the max over cores, deficit cores pad with zero-row gathers and sentinel mids.
"""

import numpy as np

B, L, D = 16384, 32, 128
DIAG_LEN, PROC_LEN, MED_LEN = 100000, 50000, 1000
N_CORES = 8
P = 128
CS = 32000          # chunk size (int16-addressable)
NCK = 5             # chunks
WIN = 128           # examples per window
SENT = 200.0        # mids sentinel (never equals iota 0..127; exact in bf16)
QUOTA = (7, 7, 7, 6, 6)  # per-(win,bag) chunk quotas in 128-blocks, rotated
PLACE = False       # example->window placement (didn't help; identity)


def _balance_chunks(rows_all, cell_all, v_cat, nwin):
    """Assign table rows to chunks so that per-(core,win,bag) chunk counts
    stay under rotated 128-multiple quotas (minimizing ceil-128 padding).

    rows_all/cell_all: per-lookup row id and cell id (c*nwin*4 + w*4 + b).
    Returns (asg [v_cat] chunk id, loc [v_cat] position within chunk).
    """
    n_cells = cell_all.max() + 1
    o = np.argsort(rows_all, kind="stable")
    rs, cells_s = rows_all[o], cell_all[o]
    row_start = np.searchsorted(rs, np.arange(v_cat + 1))
    cnts = np.diff(row_start)

    # per-(row, cell) occurrence counts, row-major
    key = rs * n_cells + cells_s
    ukey, uocc = np.unique(key, return_counts=True)
    urow = ukey // n_cells
    ucell = ukey % n_cells
    ustart = np.searchsorted(urow, np.arange(v_cat + 1))

    # quotas per (ck, cell): rotate QUOTA by (w*4+b) % NCK
    j = np.arange(n_cells) % (nwin * 4)
    q = np.array(QUOTA, np.int64) * P
    Q = np.empty((NCK, n_cells), np.int64)
    for ck in range(NCK):
        Q[ck] = q[(ck + j) % NCK]

    L = np.zeros((NCK, n_cells), np.int64)
    cap = np.full(NCK, CS, np.int64)
    asg = np.full(v_cat, -1, np.int64)
    # effective ceiling per (ck, cell): quota, ratcheted up by 128 whenever a
    # cell in the same (w,b,ck) group has already overflowed past it (the
    # extra block is paid once per group; later rows fill it for free).
    wb = j  # cell -> (w*4+b) group id
    n_grp = nwin * 4
    C = Q.copy()

    row_order = np.argsort(-cnts, kind="stable")
    BS = 512
    for i0 in range(0, v_cat, BS):
        br = row_order[i0 : i0 + BS]
        # flatten this batch's (row, cell, occ) entries
        ent_s = ustart[br]
        ent_e = ustart[br + 1]
        ent_n = ent_e - ent_s
        flat = np.concatenate([np.arange(s, e) for s, e in zip(ent_s, ent_e)]) \
            if ent_n.sum() else np.empty(0, np.int64)
        bounds = np.concatenate([[0], np.cumsum(ent_n)])
        bcell = ucell[flat]
        bocc = uocc[flat]
        # slack per (row-in-batch, ck) = min over row's cells of C-L-occ
        nb = br.size
        slack = np.full((NCK, nb), 1 << 30, np.int64)
        has = ent_n > 0
        red_idx = bounds[:-1][has]
        for ck in range(NCK):
            cs_ = C[ck, bcell] - L[ck, bcell] - bocc
            if red_idx.size:
                slack[ck, has] = np.minimum.reduceat(cs_, red_idx)
            slack[ck, ~has] = 1 << 30
            if cap[ck] <= 0:
                slack[ck, :] = -(1 << 30)
        choice = np.argmax(slack, axis=0)
        asg[br] = choice
        for ck in range(NCK):
            sel = choice == ck
            cap[ck] -= int(sel.sum())
            csel = np.repeat(sel, ent_n)
            np.add.at(L[ck], bcell[csel], bocc[csel])
            # ratchet ceilings: group ceiling = max over its cells of
            # ceil128(load), at least the quota
            gmax = np.zeros(n_grp, np.int64)
            np.maximum.at(gmax, wb, L[ck])
            gceil = -(-gmax // P) * P
            C[ck] = np.maximum(Q[ck], gceil[wb])

    # repair pass: groups (w,b,ck) barely over a 128 boundary -> move rows
    # contributing to the over-boundary cores into chunks with slack
    cell_rows_order = np.argsort(ucell, kind="stable")
    cell_start = np.searchsorted(ucell[cell_rows_order], np.arange(n_cells + 1))
    for _ in range(2):
        gmaxs = np.zeros((NCK, n_grp), np.int64)
        for ck in range(NCK):
            np.maximum.at(gmaxs[ck], wb, L[ck])
        over = gmaxs % P
        order = np.argsort(np.where(over > 0, over, 1 << 30).reshape(-1))
        moved = 0
        for flatg in order:
            ck, g = divmod(int(flatg), n_grp)
            exc = int(over[ck, g])
            if exc == 0 or exc > 48:
                break
            floor_l = gmaxs[ck, g] - exc
            # offending cells of this group
            gcells = np.nonzero(wb == g)[0]
            bad = gcells[L[ck, gcells] > floor_l]
            for cell in bad:
                need = int(L[ck, cell] - floor_l)
                ent = cell_rows_order[cell_start[cell] : cell_start[cell + 1]]
                cand = ent[asg[urow[ent]] == ck]
                # smallest contributors first
                cand = cand[np.argsort(uocc[cand], kind="stable")]
                for e in cand:
                    if need <= 0:
                        break
                    r = int(urow[e])
                    es, ee = int(ustart[r]), int(ustart[r + 1])
                    rc, ro = ucell[es:ee], uocc[es:ee]
                    for ck2 in range(NCK):
                        if ck2 == ck or cap[ck2] <= 0:
                            continue
                        if np.all(C[ck2, rc] - L[ck2, rc] >= ro):
                            asg[r] = ck2
                            L[ck, rc] -= ro
                            L[ck2, rc] += ro
                            cap[ck] += 1
                            cap[ck2] -= 1
                            need -= int(ro[np.nonzero(rc == cell)[0][0]])
                            moved += 1
                            break
        if moved == 0:
            break

    # positions within chunks (original row order)
    loc = np.zeros(v_cat, np.int64)
    for ck in range(NCK):
        sel = np.nonzero(asg == ck)[0]
        loc[sel] = np.arange(sel.size)
    return asg, loc


def _structure(counts):
    """Static program structure from per-core segment counts.

    counts: [n_cores, NWIN, NCK, 4] lookup counts per (win, ck, bag) segment.
    """
    n_cores, NWIN, NCK, NB_ = counts.shape
    cmax = counts.max(axis=0)  # [NWIN, NCK, 4]
    nb = -(-cmax // P)  # ceil -> blocks per segment
    nb[:, 0, :][nb[:, 0, :] == 0] = 1  # ck0 segments host the start=True matmul
    ops = []
    idx_off = 0
    blk_off = 0
    for w in range(NWIN):
        win_blocks = {bg: [] for bg in range(4)}
        win_ops = []
        for ck in range(NCK):
            op_blocks = []
            for bg in range(4):
                for b in range(int(nb[w, ck, bg])):
                    blk = [bg, False, False]
                    op_blocks.append(blk)
                    win_blocks[bg].append(blk)
            # split the last windows' ops so tail compute overlaps remaining
            # gathers; the final window splits at the bag0+1/bag2+3 boundary
            # so half the MLP inputs complete one sub-op early
            if w == NWIN - 1:
                cut = int(nb[w, ck, 0] + nb[w, ck, 1])
                pieces = [op_blocks[:cut], op_blocks[cut:]]
            elif w == NWIN - 2:
                per = -(-len(op_blocks) // 2)
                pieces = [op_blocks[:per], op_blocks[per:]]
            else:
                pieces = [op_blocks]
            for pb in pieces:
                if not pb:
                    continue
                n_op = len(pb) * P
                win_ops.append(
                    dict(win=w, ck=ck, idx_off=idx_off, blk_off=blk_off,
                         nb=len(pb), n=n_op, blocks=pb)
                )
                idx_off += n_op
                blk_off += len(pb)
        for bg in range(4):
            assert win_blocks[bg], "every bag needs blocks in every window"
            win_blocks[bg][0][1] = True   # start
            win_blocks[bg][-1][2] = True  # stop
        ops.extend(win_ops)
    return dict(ops=ops, tot_idx=idx_off, tot_blk=blk_off, nb_arr=nb,
                NWIN=NWIN, NCK=NCK)


def host_prep(inputs, n_cores=N_CORES):
    import ml_dtypes

    bf16 = ml_dtypes.bfloat16

    diag = np.asarray(inputs["diag_emb"], np.float32)
    proc = np.asarray(inputs["proc_emb"], np.float32)
    v_diag, d = diag.shape
    tcat = np.concatenate([diag, proc], axis=0)
    v_cat = tcat.shape[0]
    assert NCK * CS >= v_cat

    gl = {
        "cd": np.asarray(inputs["diag_codes"], np.int64),
        "cp": np.asarray(inputs["proc_codes"], np.int64) + v_diag,
        "pd": np.asarray(inputs["prev_diag_codes"], np.int64),
        "pp": np.asarray(inputs["prev_proc_codes"], np.int64) + v_diag,
    }
    b_total, l_codes = gl["cd"].shape
    assert b_total % n_cores == 0
    bc = b_total // n_cores
    assert bc % WIN == 0
    NWIN = bc // WIN

    # per-core flat (row, example, bag) streams
    core_g, core_e, core_bag = [], [], []
    for c in range(n_cores):
        gs, bags = [], []
        for bi, name in enumerate(("cd", "cp", "pd", "pp")):
            g = gl[name][c * bc : (c + 1) * bc].reshape(-1)
            gs.append(g)
            bags.append(np.full(g.size, bi, np.int64))
        core_g.append(np.concatenate(gs))
        core_bag.append(np.concatenate(bags))
        core_e.append(np.tile(np.repeat(np.arange(bc, dtype=np.int64), l_codes), 4))

    # balance rows across chunks to minimize ceil-128 padding
    rows_all = np.concatenate(core_g)
    cell_all = np.concatenate(
        [
            c * (NWIN * 4) + (core_e[c] // WIN) * 4 + core_bag[c]
            for c in range(n_cores)
        ]
    )
    asg, lmap = _balance_chunks(rows_all, cell_all, v_cat, NWIN)

    # place examples into windows (core-local permutation) to flatten each
    # window's per-(bag, chunk) lookup counts, then re-balance rows with the
    # easier cell structure.  wmaps[c][e] = permuted example slot.
    wmaps = [np.arange(bc, dtype=np.int64) for _ in range(n_cores)]
    if PLACE:
        for c in range(n_cores):
            prof = np.zeros((bc, 4, NCK), np.int64)
            np.add.at(
                prof.reshape(-1),
                (core_e[c] * 4 + core_bag[c]) * NCK + asg[core_g[c]],
                1,
            )
            prof = prof.reshape(bc, 4 * NCK).astype(np.float64)
            target = prof.sum(axis=0) / bc  # per-example mean profile
            Wsum = np.zeros((NWIN, 4 * NCK), np.float64)
            n_w = np.zeros(NWIN, np.int64)
            # most extreme examples first
            eorder = np.argsort(-np.abs(prof - target).sum(axis=1), kind="stable")
            slot = np.empty(bc, np.int64)
            for e in eorder:
                dev = Wsum + prof[e] - target * (n_w + 1)[:, None]
                score = np.square(dev).sum(axis=1)
                score[n_w >= WIN] = np.inf
                w = int(np.argmin(score))
                slot[e] = w * WIN + n_w[w]
                Wsum[w] += prof[e]
                n_w[w] += 1
            wmaps[c] = slot
        cell_all = np.concatenate(
            [
                c * (NWIN * 4) + (wmaps[c][core_e[c]] // WIN) * 4 + core_bag[c]
                for c in range(n_cores)
            ]
        )
        asg, lmap = _balance_chunks(rows_all, cell_all, v_cat, NWIN)

    tbl_dev = np.zeros(((CS + 1) * NCK, d), bf16)
    tbl_dev[asg * (CS + 1) + lmap] = tcat.astype(bf16)

    # flat per-core lookup streams, sorted by (win, ck, bag)
    per_core = []
    counts = np.zeros((n_cores, NWIN, NCK, 4), np.int64)
    for c in range(n_cores):
        g, bag = core_g[c], core_bag[c]
        e = wmaps[c][core_e[c]]
        ck = asg[g]
        loc = lmap[g]
        win = e // WIN
        m = e % WIN
        seg = (win * NCK + ck) * 4 + bag
        order = np.argsort(seg, kind="stable")
        per_core.append((seg[order], loc[order], m[order]))
        np.add.at(counts[c].reshape(-1), seg, 1)

    st = _structure(counts)
    TOT_IDX, TOT_B = st["tot_idx"], st["tot_blk"]

    # static per-segment offsets
    seg_sizes = st["nb_arr"].reshape(-1) * P
    seg_off = np.concatenate([[0], np.cumsum(seg_sizes)])[:-1]

    in_maps = []
    iota_np = np.broadcast_to(
        np.arange(P, dtype=np.float32), (P, P)
    ).astype(bf16).copy()
    for c in range(n_cores):
        seg_s, loc_s, m_s = per_core[c]
        pos_in_seg = np.arange(seg_s.size) - np.concatenate(
            [[0], np.cumsum(np.bincount(seg_s, minlength=seg_sizes.size))]
        )[:-1][seg_s]
        pos = seg_off[seg_s] + pos_in_seg
        idx_flat = np.full(TOT_IDX, CS, np.int16)  # pad -> zero row
        idx_flat[pos] = loc_s.astype(np.int16)
        m_flat = np.full(TOT_IDX, SENT, np.float32)
        m_flat[pos] = m_s
        # pack gidx: position i -> [16k + i%16, i//16]
        blk = idx_flat.reshape(TOT_IDX // 16, 16).T
        gidx = np.tile(blk, (8, 1)).copy()
        # mids: position i -> [i%128, i//128], bf16
        mids = np.ascontiguousarray(m_flat.reshape(TOT_B, P).T)
        in_maps.append(dict(tbl=tbl_dev, gidx=gidx, mids=mids, iota=iota_np))

    w1t = np.ascontiguousarray(np.asarray(inputs["W1"], np.float32).T)
    w2t = np.ascontiguousarray(np.asarray(inputs["W2"], np.float32).T)
    w3t = np.ascontiguousarray(np.asarray(inputs["W3"], np.float32).T)
    b1 = np.ascontiguousarray(np.asarray(inputs["b1"], np.float32).reshape(-1, 1))
    b2 = np.ascontiguousarray(np.asarray(inputs["b2"], np.float32).reshape(-1, 1))
    b3 = np.ascontiguousarray(np.asarray(inputs["b3"], np.float32).reshape(1, -1))
    for im in in_maps:
        im.update(w1t=w1t, w2t=w2t, w3t=w3t, b1=b1, b2=b2, b3=b3)

    med = w3t.shape[1]
    cfg = dict(b_core=bc, med=med, v_dev=tbl_dev.shape[0], st=st, wmaps=wmaps)
    return in_maps, cfg


def assemble(results, cfg):
    """Concatenate per-core outputs, undoing the example->window placement
    permutation, and convert to fp32."""
    outs = []
    for c, r in enumerate(results):
        o = r["out"].astype(np.float32)
        outs.append(o[cfg["wmaps"][c]])
    return np.concatenate(outs, axis=0)


def build_nc(cfg):
    import concourse.bass as bass
    import concourse.mybir as mybir
    import concourse.tile as tile
    from concourse import bacc

    f32 = mybir.dt.float32
    bf = mybir.dt.bfloat16
    i16 = mybir.dt.int16
    AF = mybir.ActivationFunctionType
    EQ = mybir.AluOpType.is_equal

    bc, med, v_dev = cfg["b_core"], cfg["med"], cfg["v_dev"]
    st = cfg["st"]
    NWIN, NCK = st["NWIN"], st["NCK"]
    TOT_IDX, TOT_B = st["tot_idx"], st["tot_blk"]
    n_half = med // 2
    assert n_half <= 512

    nc = bacc.Bacc("TRN2", target_bir_lowering=False, debug=False,
                   enable_asserts=False, num_devices=N_CORES)

    tbl = nc.dram_tensor("tbl", [v_dev, D], bf, kind="ExternalInput").ap()
    gidx = nc.dram_tensor("gidx", [P, TOT_IDX // 16], i16, kind="ExternalInput").ap()
    mids = nc.dram_tensor("mids", [P, TOT_B], f32, kind="ExternalInput").ap()
    iota = nc.dram_tensor("iota", [P, P], bf, kind="ExternalInput").ap()
    w1t = nc.dram_tensor("w1t", [2 * D, D], f32, kind="ExternalInput").ap()
    w2t = nc.dram_tensor("w2t", [2 * D, 2 * D], f32, kind="ExternalInput").ap()
    w3t = nc.dram_tensor("w3t", [2 * D, med], f32, kind="ExternalInput").ap()
    b1 = nc.dram_tensor("b1", [D, 1], f32, kind="ExternalInput").ap()
    b2 = nc.dram_tensor("b2", [2 * D, 1], f32, kind="ExternalInput").ap()
    b3 = nc.dram_tensor("b3", [1, med], f32, kind="ExternalInput").ap()
    out = nc.dram_tensor("out", [bc, med], bf, kind="ExternalOutput").ap()

    ops_by_win = {}
    for op in st["ops"]:
        ops_by_win.setdefault(op["win"], []).append(op)

    with tile.TileContext(nc) as tc:
        with (
            tc.tile_pool(name="const", bufs=1) as cpool,
            tc.tile_pool(name="gi", bufs=3) as gi_pool,
            tc.tile_pool(name="mi", bufs=3) as mi_pool,
            tc.tile_pool(name="em", bufs=8) as em_pool,
            tc.tile_pool(name="gath", bufs=8) as gath_pool,
            tc.tile_pool(name="sT", bufs=8) as sT_pool,
            tc.tile_pool(name="acts", bufs=8) as act_pool,
            tc.tile_pool(name="osb", bufs=2) as out_pool,
            tc.tile_pool(name="spsum", bufs=4, space="PSUM") as s_psum,
            tc.tile_pool(name="mpsum", bufs=2, space="PSUM") as m_psum,
            tc.tile_pool(name="opsum", bufs=2, space="PSUM") as o_psum,
        ):
            iota_t = cpool.tile([P, P], bf, tag="iota")
            iota_loaded = [False]

            consts = {}

            def load_consts():
                # Emitted after window 0's gather ops so the first gathers
                # aren't queued behind ~1.3MB of weight preloads.
                ones = cpool.tile([1, P], f32, tag="ones")
                nc.gpsimd.memset(ones[:], 1.0)
                w1t_k = []
                for k in range(2):
                    t = cpool.tile([D, D], f32, tag=f"w1t{k}")
                    nc.sync.dma_start(t[:], w1t[k * D : (k + 1) * D, :])
                    w1t_k.append(t)
                w2t_km = {}
                for k in range(2):
                    for mm in range(2):
                        t = cpool.tile([D, D], f32, tag=f"w2t{k}{mm}")
                        nc.sync.dma_start(
                            t[:], w2t[k * D : (k + 1) * D, mm * D : (mm + 1) * D]
                        )
                        w2t_km[(k, mm)] = t
                w3t_k = []
                for k in range(2):
                    t = cpool.tile([D, med], f32, tag=f"w3t{k}")
                    nc.sync.dma_start(t[:], w3t[k * D : (k + 1) * D, :])
                    w3t_k.append(t)
                b1_t = cpool.tile([D, 1], f32, tag="b1")
                nc.sync.dma_start(b1_t[:], b1[:, :])
                b2_t = []
                for mm in range(2):
                    t = cpool.tile([D, 1], f32, tag=f"b2{mm}")
                    nc.sync.dma_start(t[:], b2[mm * D : (mm + 1) * D, :])
                    b2_t.append(t)
                b3_t = cpool.tile([1, med], f32, tag="b3")
                nc.sync.dma_start(b3_t[:], b3[:, :])
                consts.update(ones=ones, w1t_k=w1t_k, w2t_km=w2t_km,
                              w3t_k=w3t_k, b1_t=b1_t, b2_t=b2_t, b3_t=b3_t)

            for rep in range(cfg.get("reps", 1)):
              for w in range(NWIN):
                s_ps = [s_psum.tile([D, WIN], f32, tag="s", name=f"s{rep}_{w}_{i}") for i in range(4)]
                wops = ops_by_win[w]
                w_idx_off = wops[0]["idx_off"]
                w_blk_off = wops[0]["blk_off"]
                w_n = sum(op["n"] for op in wops)
                w_nb = sum(op["nb"] for op in wops)
                gi = gi_pool.tile([P, w_n // 16], i16, tag="gi")
                if not cfg.get("skip_gi"):
                    nc.sync.dma_start(
                        gi[:],
                        gidx[:, w_idx_off // 16 : (w_idx_off + w_n) // 16],
                    )
                mi = mi_pool.tile([P, w_nb], f32, tag="mi")
                nc.sync.dma_start(
                    mi[:], mids[:, w_blk_off : w_blk_off + w_nb]
                )
                if not iota_loaded[0]:
                    # emitted after window 0's index feeds so the first
                    # gather's gi DMA heads the SP queue
                    nc.sync.dma_start(iota_t[:], iota[:, :])
                    iota_loaded[0] = True
                for op in wops:
                    n, nb = op["n"], op["nb"]
                    o16 = (op["idx_off"] - w_idx_off) // 16
                    ob0 = op["blk_off"] - w_blk_off
                    gt = gath_pool.tile([P, nb * D], bf, tag="gath")
                    if not cfg.get("skip_gather"):
                        nc.gpsimd.dma_gather(
                            out_ap=gt[:].rearrange("p (c d) -> p c d", d=D),
                            in_ap=tbl[
                                op["ck"] * (CS + 1) : (op["ck"] + 1) * (CS + 1), :
                            ],
                            idxs_ap=gi[:, o16 : o16 + n // 16],
                            num_idxs=n,
                            num_idxs_reg=n,
                            elem_size=D,
                            single_packet=False,
                        )
                    gt3 = gt[:].rearrange("p (c d) -> p c d", d=D)
                    em = em_pool.tile([P, P * nb], bf, tag="em")
                    for b in range(nb):
                        nc.vector.tensor_scalar(
                            em[:, b * P : (b + 1) * P],
                            iota_t[:],
                            mi[:, ob0 + b : ob0 + b + 1],
                            None,
                            EQ,
                        )
                    if cfg.get("skip_smm"):
                        continue
                    for b, (bg, start, stop) in enumerate(op["blocks"]):
                        nc.tensor.matmul(
                            s_ps[bg][:],
                            lhsT=gt3[:, b, :],
                            rhs=em[:, b * P : (b + 1) * P],
                            start=start,
                            stop=stop,
                            skip_group_check=True,
                        )
                if not consts:
                    load_consts()
                ones = consts["ones"]
                w1t_k, w2t_km = consts["w1t_k"], consts["w2t_km"]
                w3t_k = consts["w3t_k"]
                b1_t, b2_t, b3_t = consts["b1_t"], consts["b2_t"], consts["b3_t"]
                if cfg.get("skip_mlp"):
                    continue
                # bag sums (feature-major) PSUM -> SBUF on ACT
                sT = []
                for bg in range(4):
                    t = sT_pool.tile([D, P], f32, tag="sT", name=f"sT{w}_{bg}")
                    if bg % 2 == 0:
                        nc.scalar.activation(t[:], s_ps[bg][:], AF.Copy)
                    else:
                        nc.vector.tensor_copy(t[:], s_ps[bg][:])
                    sT.append(t)

                l1 = []
                for ka, kb in ((0, 1), (2, 3)):
                    pc = m_psum.tile([P, P], f32, tag="mp")
                    nc.tensor.matmul(
                        pc[:], lhsT=w1t_k[0][:], rhs=sT[ka][:], start=True, stop=False
                    )
                    nc.tensor.matmul(
                        pc[:], lhsT=w1t_k[1][:], rhs=sT[kb][:], start=False, stop=True
                    )
                    xt = act_pool.tile([D, P], f32, tag="l1")
                    nc.scalar.activation(xt[:], pc[:], AF.Identity, bias=b1_t[:])
                    l1.append(xt)

                hT = []
                for mm in range(2):
                    ph = m_psum.tile([P, P], f32, tag="mp")
                    nc.tensor.matmul(
                        ph[:], lhsT=w2t_km[(0, mm)][:], rhs=l1[0][:],
                        start=True, stop=False,
                    )
                    nc.tensor.matmul(
                        ph[:], lhsT=w2t_km[(1, mm)][:], rhs=l1[1][:],
                        start=False, stop=True,
                    )
                    ht = act_pool.tile([D, P], f32, tag="l2")
                    nc.scalar.activation(ht[:], ph[:], AF.Relu, bias=b2_t[mm][:])
                    hT.append(ht)

                ob = out_pool.tile([P, med], bf, tag="osb")
                for h_i in range(2):
                    n0, n1 = h_i * n_half, (h_i + 1) * n_half
                    po = o_psum.tile([P, n_half], f32, tag="op")
                    nc.tensor.matmul(
                        po[:], lhsT=ones[:1, :], rhs=b3_t[:1, n0:n1],
                        start=True, stop=False,
                    )
                    nc.tensor.matmul(
                        po[:], lhsT=hT[0][:], rhs=w3t_k[0][:, n0:n1],
                        start=False, stop=False,
                    )
                    nc.tensor.matmul(
                        po[:], lhsT=hT[1][:], rhs=w3t_k[1][:, n0:n1],
                        start=False, stop=True,
                    )
                    nc.scalar.activation(ob[:, n0:n1], po[:], AF.Sigmoid)
                nc.scalar.dma_start(out[w * P : (w + 1) * P, :], ob[:])

    nc.compile()
    return nc


def kernel(**inputs) -> np.ndarray:
    from concourse.bass_utils import run_bass_kernel_spmd

    in_maps, cfg = host_prep(inputs)
    nc = build_nc(cfg)
    res = run_bass_kernel_spmd(nc, in_maps, core_ids=list(range(N_CORES)))
    return assemble(res.results, cfg)


# revision 35
# speedup vs baseline: 1.0005x; 1.0001x over previous
"""Trainium2 Bass kernel: 4x EmbeddingBag(sum over 32 codes) + 3-layer MLP.

Data-parallel over 8 NeuronCores (batch 16384 -> 8 x 2048).  Embedding tables
are concatenated (proc offset by +100000), cast to bf16 and split into 5
chunks of <=32000 rows (so per-chunk row indices fit int16 for dma_gather),
each chunk followed by one zero row used as gather padding.  Table rows are
assigned to chunks by a quota-balancing greedy (plus a repair pass) so that
every (core, window, bag) cell's per-chunk lookup counts stay under rotated
multiples-of-128 quotas - minimizing the ceil-128 padding below.

Per core the 262144 lookups (4 bags x 2048 examples x 32 codes) are sorted by
(window of 128 examples, chunk, bag).  Each (win,ck,bag) segment is padded to
a multiple of 128 rows ("blocks") with zero-row fetches.  One dma_gather per
(win, ck) pulls all its blocks' rows (bf16, 256B each) into SBUF in
partition-fastest order.  Per block, the selection matrix E [128 rows x 128
examples] is generated ON-CHIP by the vector engine (tensor_scalar is_equal
of a constant iota row-tile against the block's per-row example ids "mids",
a per-window fp32 DMA), then one PE matmul per block accumulates the rows
into a per-(bag, win) PSUM tile [D=128, 128 examples] in fp32 - start=True on
the first block of each (win,bag), stop on the last.  Pad rows fetch the
chunk's zero row and carry a sentinel mid (no E column), so they add zero.

The MLP then runs per window in fp32: bag sums are already feature-major in
PSUM, copy to SBUF (ACT/DVE), layer1+layer2 feature-major (ACT applies
bias/relu on the PSUM->SBUF copy), layer3 uses the activations as lhsT to
emit example-major [128, 1000] directly (bias via a K=1 ones-row matmul PSUM
init), ACT sigmoid, output DMA'd bf16 on the ACT HWDGE queue (host converts
to fp32).  Weight preloads are emitted after window 0's gathers; the last two
windows' gather ops are split so the drain tail overlaps remaining gathers.

The Bass program structure is shared by all 8 cores (SPMD); per-op sizes are
the max over cores, deficit cores pad with zero-row gathers and sentinel mids.
"""

import numpy as np

B, L, D = 16384, 32, 128
DIAG_LEN, PROC_LEN, MED_LEN = 100000, 50000, 1000
N_CORES = 8
P = 128
CS = 32000          # chunk size (int16-addressable)
NCK = 5             # chunks
WIN = 128           # examples per window
SENT = 200.0        # mids sentinel (never equals iota 0..127; exact in bf16)
QUOTA = (7, 7, 7, 6, 6)  # per-(win,bag) chunk quotas in 128-blocks, rotated
PLACE = False       # example->window placement (didn't help; identity)


def _balance_chunks(rows_all, cell_all, v_cat, nwin):
    """Assign table rows to chunks so that per-(core,win,bag) chunk counts
    stay under rotated 128-multiple quotas (minimizing ceil-128 padding).

    rows_all/cell_all: per-lookup row id and cell id (c*nwin*4 + w*4 + b).
    Returns (asg [v_cat] chunk id, loc [v_cat] position within chunk).
    """
    n_cells = cell_all.max() + 1
    o = np.argsort(rows_all, kind="stable")
    rs, cells_s = rows_all[o], cell_all[o]
    row_start = np.searchsorted(rs, np.arange(v_cat + 1))
    cnts = np.diff(row_start)

    # per-(row, cell) occurrence counts, row-major
    key = rs * n_cells + cells_s
    ukey, uocc = np.unique(key, return_counts=True)
    urow = ukey // n_cells
    ucell = ukey % n_cells
    ustart = np.searchsorted(urow, np.arange(v_cat + 1))

    # quotas per (ck, cell): rotate QUOTA by (w*4+b) % NCK
    j = np.arange(n_cells) % (nwin * 4)
    q = np.array(QUOTA, np.int64) * P
    Q = np.empty((NCK, n_cells), np.int64)
    for ck in range(NCK):
        Q[ck] = q[(ck + j) % NCK]

    wb = j  # cell -> (w*4+b) group id
    n_grp = nwin * 4
    row_order = np.argsort(-cnts, kind="stable")
    BS = 512

    def greedy_pack(Cap0):
        """One greedy packing run with initial per-(ck,cell) ceilings Cap0.
        Ceilings ratchet up by 128 when a (w,b,ck) group overflows (the
        extra block is paid once per group; later rows fill it free)."""
        L = np.zeros((NCK, n_cells), np.int64)
        cap = np.full(NCK, CS, np.int64)
        asg = np.full(v_cat, -1, np.int64)
        C = Cap0.copy()
        for i0 in range(0, v_cat, BS):
            br = row_order[i0 : i0 + BS]
            ent_s = ustart[br]
            ent_e = ustart[br + 1]
            ent_n = ent_e - ent_s
            flat = np.concatenate(
                [np.arange(s, e) for s, e in zip(ent_s, ent_e)]
            ) if ent_n.sum() else np.empty(0, np.int64)
            bounds = np.concatenate([[0], np.cumsum(ent_n)])
            bcell = ucell[flat]
            bocc = uocc[flat]
            nb = br.size
            slack = np.full((NCK, nb), 1 << 30, np.int64)
            has = ent_n > 0
            red_idx = bounds[:-1][has]
            for ck in range(NCK):
                cs_ = C[ck, bcell] - L[ck, bcell] - bocc
                if red_idx.size:
                    slack[ck, has] = np.minimum.reduceat(cs_, red_idx)
                slack[ck, ~has] = 1 << 30
                if cap[ck] <= 0:
                    slack[ck, :] = -(1 << 30)
            choice = np.argmax(slack, axis=0)
            asg[br] = choice
            for ck in range(NCK):
                sel = choice == ck
                cap[ck] -= int(sel.sum())
                csel = np.repeat(sel, ent_n)
                np.add.at(L[ck], bcell[csel], bocc[csel])
                gmax = np.zeros(n_grp, np.int64)
                np.maximum.at(gmax, wb, L[ck])
                gceil = -(-gmax // P) * P
                C[ck] = np.maximum(Cap0[ck], gceil[wb])
        return asg, L, cap, C

    def total_blocks(L):
        t = 0
        for ck in range(NCK):
            gmax = np.zeros(n_grp, np.int64)
            np.maximum.at(gmax, wb, L[ck])
            t += int((-(-gmax // P)).sum())
        return t

    # iterate: re-pack from scratch with ceilings tightened by 128 on groups
    # that overflowed their quota in the best run so far; keep the best
    asg, L, cap, C = greedy_pack(Q)
    best = (total_blocks(L), asg, L, cap, C)
    for _ in range(3):
        _, asg_b, L_b = best[0], best[1], best[2]
        Ct = Q.copy()
        for ck in range(NCK):
            gmax = np.zeros(n_grp, np.int64)
            np.maximum.at(gmax, wb, L_b[ck])
            gceil = -(-gmax // P) * P
            tgt = np.maximum(Q[ck, :], (gceil - P)[wb])
            Ct[ck] = np.minimum(np.maximum(Q[ck], gceil[wb]), tgt + P)
            Ct[ck] = np.maximum(Q[ck], tgt)
        asg, L, cap, C = greedy_pack(Ct)
        tb = total_blocks(L)
        if tb < best[0]:
            best = (tb, asg, L, cap, C)
        else:
            break
    _, asg, L, cap, C = best

    # repair pass: groups (w,b,ck) barely over a 128 boundary -> move rows
    # contributing to the over-boundary cores into chunks with slack
    cell_rows_order = np.argsort(ucell, kind="stable")
    cell_start = np.searchsorted(ucell[cell_rows_order], np.arange(n_cells + 1))
    for _ in range(2):
        gmaxs = np.zeros((NCK, n_grp), np.int64)
        for ck in range(NCK):
            np.maximum.at(gmaxs[ck], wb, L[ck])
        over = gmaxs % P
        order = np.argsort(np.where(over > 0, over, 1 << 30).reshape(-1))
        moved = 0
        for flatg in order:
            ck, g = divmod(int(flatg), n_grp)
            exc = int(over[ck, g])
            if exc == 0 or exc > 48:
                break
            floor_l = gmaxs[ck, g] - exc
            # offending cells of this group
            gcells = np.nonzero(wb == g)[0]
            bad = gcells[L[ck, gcells] > floor_l]
            for cell in bad:
                need = int(L[ck, cell] - floor_l)
                ent = cell_rows_order[cell_start[cell] : cell_start[cell + 1]]
                cand = ent[asg[urow[ent]] == ck]
                # smallest contributors first
                cand = cand[np.argsort(uocc[cand], kind="stable")]
                for e in cand:
                    if need <= 0:
                        break
                    r = int(urow[e])
                    es, ee = int(ustart[r]), int(ustart[r + 1])
                    rc, ro = ucell[es:ee], uocc[es:ee]
                    for ck2 in range(NCK):
                        if ck2 == ck or cap[ck2] <= 0:
                            continue
                        if np.all(C[ck2, rc] - L[ck2, rc] >= ro):
                            asg[r] = ck2
                            L[ck, rc] -= ro
                            L[ck2, rc] += ro
                            cap[ck] += 1
                            cap[ck2] -= 1
                            need -= int(ro[np.nonzero(rc == cell)[0][0]])
                            moved += 1
                            break
        if moved == 0:
            break

    # positions within chunks (original row order)
    loc = np.zeros(v_cat, np.int64)
    for ck in range(NCK):
        sel = np.nonzero(asg == ck)[0]
        loc[sel] = np.arange(sel.size)
    return asg, loc


def _structure(counts):
    """Static program structure from per-core segment counts.

    counts: [n_cores, NWIN, NCK, 4] lookup counts per (win, ck, bag) segment.
    """
    n_cores, NWIN, NCK, NB_ = counts.shape
    cmax = counts.max(axis=0)  # [NWIN, NCK, 4]
    nb = -(-cmax // P)  # ceil -> blocks per segment
    nb[:, 0, :][nb[:, 0, :] == 0] = 1  # ck0 segments host the start=True matmul
    ops = []
    idx_off = 0
    blk_off = 0
    for w in range(NWIN):
        win_blocks = {bg: [] for bg in range(4)}
        win_ops = []
        for ck in range(NCK):
            op_blocks = []
            for bg in range(4):
                for b in range(int(nb[w, ck, bg])):
                    blk = [bg, False, False]
                    op_blocks.append(blk)
                    win_blocks[bg].append(blk)
            # split the last windows' ops so tail compute overlaps remaining
            # gathers; the final window splits at the bag0+1/bag2+3 boundary
            # so half the MLP inputs complete one sub-op early
            if w == NWIN - 1:
                cut = int(nb[w, ck, 0] + nb[w, ck, 1])
                pieces = [op_blocks[:cut], op_blocks[cut:]]
            elif w == NWIN - 2:
                per = -(-len(op_blocks) // 2)
                pieces = [op_blocks[:per], op_blocks[per:]]
            elif w == 0 and ck == 0:
                # tiny head op so the first gather starts ASAP
                pieces = [op_blocks[:4], op_blocks[4:]]
            else:
                pieces = [op_blocks]
            for pb in pieces:
                if not pb:
                    continue
                n_op = len(pb) * P
                win_ops.append(
                    dict(win=w, ck=ck, idx_off=idx_off, blk_off=blk_off,
                         nb=len(pb), n=n_op, blocks=pb)
                )
                idx_off += n_op
                blk_off += len(pb)
        for bg in range(4):
            assert win_blocks[bg], "every bag needs blocks in every window"
            win_blocks[bg][0][1] = True   # start
            win_blocks[bg][-1][2] = True  # stop
        ops.extend(win_ops)
    return dict(ops=ops, tot_idx=idx_off, tot_blk=blk_off, nb_arr=nb,
                NWIN=NWIN, NCK=NCK)


def host_prep(inputs, n_cores=N_CORES):
    import ml_dtypes

    bf16 = ml_dtypes.bfloat16

    diag = np.asarray(inputs["diag_emb"], np.float32)
    proc = np.asarray(inputs["proc_emb"], np.float32)
    v_diag, d = diag.shape
    tcat = np.concatenate([diag, proc], axis=0)
    v_cat = tcat.shape[0]
    assert NCK * CS >= v_cat

    gl = {
        "cd": np.asarray(inputs["diag_codes"], np.int64),
        "cp": np.asarray(inputs["proc_codes"], np.int64) + v_diag,
        "pd": np.asarray(inputs["prev_diag_codes"], np.int64),
        "pp": np.asarray(inputs["prev_proc_codes"], np.int64) + v_diag,
    }
    b_total, l_codes = gl["cd"].shape
    assert b_total % n_cores == 0
    bc = b_total // n_cores
    assert bc % WIN == 0
    NWIN = bc // WIN

    # per-core flat (row, example, bag) streams
    core_g, core_e, core_bag = [], [], []
    for c in range(n_cores):
        gs, bags = [], []
        for bi, name in enumerate(("cd", "cp", "pd", "pp")):
            g = gl[name][c * bc : (c + 1) * bc].reshape(-1)
            gs.append(g)
            bags.append(np.full(g.size, bi, np.int64))
        core_g.append(np.concatenate(gs))
        core_bag.append(np.concatenate(bags))
        core_e.append(np.tile(np.repeat(np.arange(bc, dtype=np.int64), l_codes), 4))

    # balance rows across chunks to minimize ceil-128 padding
    rows_all = np.concatenate(core_g)
    cell_all = np.concatenate(
        [
            c * (NWIN * 4) + (core_e[c] // WIN) * 4 + core_bag[c]
            for c in range(n_cores)
        ]
    )
    asg, lmap = _balance_chunks(rows_all, cell_all, v_cat, NWIN)

    # place examples into windows (core-local permutation) to flatten each
    # window's per-(bag, chunk) lookup counts, then re-balance rows with the
    # easier cell structure.  wmaps[c][e] = permuted example slot.
    wmaps = [np.arange(bc, dtype=np.int64) for _ in range(n_cores)]
    if PLACE:
        for c in range(n_cores):
            prof = np.zeros((bc, 4, NCK), np.int64)
            np.add.at(
                prof.reshape(-1),
                (core_e[c] * 4 + core_bag[c]) * NCK + asg[core_g[c]],
                1,
            )
            prof = prof.reshape(bc, 4 * NCK).astype(np.float64)
            target = prof.sum(axis=0) / bc  # per-example mean profile
            Wsum = np.zeros((NWIN, 4 * NCK), np.float64)
            n_w = np.zeros(NWIN, np.int64)
            # most extreme examples first
            eorder = np.argsort(-np.abs(prof - target).sum(axis=1), kind="stable")
            slot = np.empty(bc, np.int64)
            for e in eorder:
                dev = Wsum + prof[e] - target * (n_w + 1)[:, None]
                score = np.square(dev).sum(axis=1)
                score[n_w >= WIN] = np.inf
                w = int(np.argmin(score))
                slot[e] = w * WIN + n_w[w]
                Wsum[w] += prof[e]
                n_w[w] += 1
            wmaps[c] = slot
        cell_all = np.concatenate(
            [
                c * (NWIN * 4) + (wmaps[c][core_e[c]] // WIN) * 4 + core_bag[c]
                for c in range(n_cores)
            ]
        )
        asg, lmap = _balance_chunks(rows_all, cell_all, v_cat, NWIN)

    tbl_dev = np.zeros(((CS + 1) * NCK, d), bf16)
    tbl_dev[asg * (CS + 1) + lmap] = tcat.astype(bf16)

    # flat per-core lookup streams, sorted by (win, ck, bag)
    per_core = []
    counts = np.zeros((n_cores, NWIN, NCK, 4), np.int64)
    for c in range(n_cores):
        g, bag = core_g[c], core_bag[c]
        e = wmaps[c][core_e[c]]
        ck = asg[g]
        loc = lmap[g]
        win = e // WIN
        m = e % WIN
        seg = (win * NCK + ck) * 4 + bag
        order = np.argsort(seg, kind="stable")
        per_core.append((seg[order], loc[order], m[order]))
        np.add.at(counts[c].reshape(-1), seg, 1)

    st = _structure(counts)
    TOT_IDX, TOT_B = st["tot_idx"], st["tot_blk"]

    # static per-segment offsets
    seg_sizes = st["nb_arr"].reshape(-1) * P
    seg_off = np.concatenate([[0], np.cumsum(seg_sizes)])[:-1]

    in_maps = []
    iota_np = np.broadcast_to(
        np.arange(P, dtype=np.float32), (P, P)
    ).astype(bf16).copy()
    for c in range(n_cores):
        seg_s, loc_s, m_s = per_core[c]
        pos_in_seg = np.arange(seg_s.size) - np.concatenate(
            [[0], np.cumsum(np.bincount(seg_s, minlength=seg_sizes.size))]
        )[:-1][seg_s]
        pos = seg_off[seg_s] + pos_in_seg
        idx_flat = np.full(TOT_IDX, CS, np.int16)  # pad -> zero row
        idx_flat[pos] = loc_s.astype(np.int16)
        m_flat = np.full(TOT_IDX, SENT, np.float32)
        m_flat[pos] = m_s
        # pack gidx: position i -> [16k + i%16, i//16]
        blk = idx_flat.reshape(TOT_IDX // 16, 16).T
        gidx = np.tile(blk, (8, 1)).copy()
        # mids: position i -> [i%128, i//128], bf16
        mids = np.ascontiguousarray(m_flat.reshape(TOT_B, P).T)
        in_maps.append(dict(tbl=tbl_dev, gidx=gidx, mids=mids, iota=iota_np))

    w1t = np.ascontiguousarray(np.asarray(inputs["W1"], np.float32).T)
    w2t = np.ascontiguousarray(np.asarray(inputs["W2"], np.float32).T)
    w3t = np.ascontiguousarray(np.asarray(inputs["W3"], np.float32).T)
    b1 = np.ascontiguousarray(np.asarray(inputs["b1"], np.float32).reshape(-1, 1))
    b2 = np.ascontiguousarray(np.asarray(inputs["b2"], np.float32).reshape(-1, 1))
    b3 = np.ascontiguousarray(np.asarray(inputs["b3"], np.float32).reshape(1, -1))
    for im in in_maps:
        im.update(w1t=w1t, w2t=w2t, w3t=w3t, b1=b1, b2=b2, b3=b3)

    med = w3t.shape[1]
    cfg = dict(b_core=bc, med=med, v_dev=tbl_dev.shape[0], st=st, wmaps=wmaps)
    return in_maps, cfg


def assemble(results, cfg):
    """Concatenate per-core outputs, undoing the example->window placement
    permutation, and convert to fp32."""
    outs = []
    for c, r in enumerate(results):
        o = r["out"].astype(np.float32)
        outs.append(o[cfg["wmaps"][c]])
    return np.concatenate(outs, axis=0)


def build_nc(cfg):
    import concourse.bass as bass
    import concourse.mybir as mybir
    import concourse.tile as tile
    from concourse import bacc

    f32 = mybir.dt.float32
    bf = mybir.dt.bfloat16
    i16 = mybir.dt.int16
    AF = mybir.ActivationFunctionType
    EQ = mybir.AluOpType.is_equal

    bc, med, v_dev = cfg["b_core"], cfg["med"], cfg["v_dev"]
    st = cfg["st"]
    NWIN, NCK = st["NWIN"], st["NCK"]
    TOT_IDX, TOT_B = st["tot_idx"], st["tot_blk"]
    n_half = med // 2
    assert n_half <= 512

    nc = bacc.Bacc("TRN2", target_bir_lowering=False, debug=False,
                   enable_asserts=False, num_devices=N_CORES)

    tbl = nc.dram_tensor("tbl", [v_dev, D], bf, kind="ExternalInput").ap()
    gidx = nc.dram_tensor("gidx", [P, TOT_IDX // 16], i16, kind="ExternalInput").ap()
    mids = nc.dram_tensor("mids", [P, TOT_B], f32, kind="ExternalInput").ap()
    iota = nc.dram_tensor("iota", [P, P], bf, kind="ExternalInput").ap()
    w1t = nc.dram_tensor("w1t", [2 * D, D], f32, kind="ExternalInput").ap()
    w2t = nc.dram_tensor("w2t", [2 * D, 2 * D], f32, kind="ExternalInput").ap()
    w3t = nc.dram_tensor("w3t", [2 * D, med], f32, kind="ExternalInput").ap()
    b1 = nc.dram_tensor("b1", [D, 1], f32, kind="ExternalInput").ap()
    b2 = nc.dram_tensor("b2", [2 * D, 1], f32, kind="ExternalInput").ap()
    b3 = nc.dram_tensor("b3", [1, med], f32, kind="ExternalInput").ap()
    out = nc.dram_tensor("out", [bc, med], bf, kind="ExternalOutput").ap()

    ops_by_win = {}
    for op in st["ops"]:
        ops_by_win.setdefault(op["win"], []).append(op)

    with tile.TileContext(nc) as tc:
        with (
            tc.tile_pool(name="const", bufs=1) as cpool,
            tc.tile_pool(name="gi", bufs=3) as gi_pool,
            tc.tile_pool(name="mi", bufs=3) as mi_pool,
            tc.tile_pool(name="em", bufs=8) as em_pool,
            tc.tile_pool(name="gath", bufs=8) as gath_pool,
            tc.tile_pool(name="sT", bufs=8) as sT_pool,
            tc.tile_pool(name="acts", bufs=8) as act_pool,
            tc.tile_pool(name="osb", bufs=2) as out_pool,
            tc.tile_pool(name="spsum", bufs=4, space="PSUM") as s_psum,
            tc.tile_pool(name="mpsum", bufs=2, space="PSUM") as m_psum,
            tc.tile_pool(name="opsum", bufs=2, space="PSUM") as o_psum,
        ):
            iota_t = cpool.tile([P, P], bf, tag="iota")
            iota_loaded = [False]

            consts = {}

            def load_consts():
                # Emitted after window 0's gather ops so the first gathers
                # aren't queued behind ~1.3MB of weight preloads.
                ones = cpool.tile([1, P], f32, tag="ones")
                nc.gpsimd.memset(ones[:], 1.0)
                w1t_k = []
                for k in range(2):
                    t = cpool.tile([D, D], f32, tag=f"w1t{k}")
                    nc.sync.dma_start(t[:], w1t[k * D : (k + 1) * D, :])
                    w1t_k.append(t)
                w2t_km = {}
                for k in range(2):
                    for mm in range(2):
                        t = cpool.tile([D, D], f32, tag=f"w2t{k}{mm}")
                        nc.sync.dma_start(
                            t[:], w2t[k * D : (k + 1) * D, mm * D : (mm + 1) * D]
                        )
                        w2t_km[(k, mm)] = t
                w3t_k = []
                for k in range(2):
                    t = cpool.tile([D, med], f32, tag=f"w3t{k}")
                    nc.sync.dma_start(t[:], w3t[k * D : (k + 1) * D, :])
                    w3t_k.append(t)
                b1_t = cpool.tile([D, 1], f32, tag="b1")
                nc.sync.dma_start(b1_t[:], b1[:, :])
                b2_t = []
                for mm in range(2):
                    t = cpool.tile([D, 1], f32, tag=f"b2{mm}")
                    nc.sync.dma_start(t[:], b2[mm * D : (mm + 1) * D, :])
                    b2_t.append(t)
                b3_t = cpool.tile([1, med], f32, tag="b3")
                nc.sync.dma_start(b3_t[:], b3[:, :])
                consts.update(ones=ones, w1t_k=w1t_k, w2t_km=w2t_km,
                              w3t_k=w3t_k, b1_t=b1_t, b2_t=b2_t, b3_t=b3_t)

            for rep in range(cfg.get("reps", 1)):
              for w in range(NWIN):
                s_ps = [s_psum.tile([D, WIN], f32, tag="s", name=f"s{rep}_{w}_{i}") for i in range(4)]
                wops = ops_by_win[w]
                w_idx_off = wops[0]["idx_off"]
                w_blk_off = wops[0]["blk_off"]
                w_n = sum(op["n"] for op in wops)
                w_nb = sum(op["nb"] for op in wops)
                gi = gi_pool.tile([P, w_n // 16], i16, tag="gi")
                if not cfg.get("skip_gi"):
                    nc.sync.dma_start(
                        gi[:],
                        gidx[:, w_idx_off // 16 : (w_idx_off + w_n) // 16],
                    )
                mi = mi_pool.tile([P, w_nb], f32, tag="mi")
                nc.sync.dma_start(
                    mi[:], mids[:, w_blk_off : w_blk_off + w_nb]
                )
                if not iota_loaded[0]:
                    # emitted after window 0's index feeds so the first
                    # gather's gi DMA heads the SP queue
                    nc.sync.dma_start(iota_t[:], iota[:, :])
                    iota_loaded[0] = True
                for op in wops:
                    n, nb = op["n"], op["nb"]
                    o16 = (op["idx_off"] - w_idx_off) // 16
                    ob0 = op["blk_off"] - w_blk_off
                    gt = gath_pool.tile([P, nb * D], bf, tag="gath")
                    if not cfg.get("skip_gather"):
                        nc.gpsimd.dma_gather(
                            out_ap=gt[:].rearrange("p (c d) -> p c d", d=D),
                            in_ap=tbl[
                                op["ck"] * (CS + 1) : (op["ck"] + 1) * (CS + 1), :
                            ],
                            idxs_ap=gi[:, o16 : o16 + n // 16],
                            num_idxs=n,
                            num_idxs_reg=n,
                            elem_size=D,
                            single_packet=False,
                        )
                    gt3 = gt[:].rearrange("p (c d) -> p c d", d=D)
                    em = em_pool.tile([P, P * nb], bf, tag="em")
                    for b in range(nb):
                        nc.vector.tensor_scalar(
                            em[:, b * P : (b + 1) * P],
                            iota_t[:],
                            mi[:, ob0 + b : ob0 + b + 1],
                            None,
                            EQ,
                        )
                    if cfg.get("skip_smm"):
                        continue
                    for b, (bg, start, stop) in enumerate(op["blocks"]):
                        nc.tensor.matmul(
                            s_ps[bg][:],
                            lhsT=gt3[:, b, :],
                            rhs=em[:, b * P : (b + 1) * P],
                            start=start,
                            stop=stop,
                            skip_group_check=True,
                        )
                if not consts:
                    load_consts()
                ones = consts["ones"]
                w1t_k, w2t_km = consts["w1t_k"], consts["w2t_km"]
                w3t_k = consts["w3t_k"]
                b1_t, b2_t, b3_t = consts["b1_t"], consts["b2_t"], consts["b3_t"]
                if cfg.get("skip_mlp"):
                    continue
                # bag sums (feature-major) PSUM -> SBUF on ACT
                sT = []
                for bg in range(4):
                    t = sT_pool.tile([D, P], f32, tag="sT", name=f"sT{w}_{bg}")
                    if bg % 2 == 0:
                        nc.scalar.activation(t[:], s_ps[bg][:], AF.Copy)
                    else:
                        nc.vector.tensor_copy(t[:], s_ps[bg][:])
                    sT.append(t)

                l1 = []
                for ka, kb in ((0, 1), (2, 3)):
                    pc = m_psum.tile([P, P], f32, tag="mp")
                    nc.tensor.matmul(
                        pc[:], lhsT=w1t_k[0][:], rhs=sT[ka][:], start=True, stop=False
                    )
                    nc.tensor.matmul(
                        pc[:], lhsT=w1t_k[1][:], rhs=sT[kb][:], start=False, stop=True
                    )
                    xt = act_pool.tile([D, P], f32, tag="l1")
                    nc.scalar.activation(xt[:], pc[:], AF.Identity, bias=b1_t[:])
                    l1.append(xt)

                hT = []
                for mm in range(2):
                    ph = m_psum.tile([P, P], f32, tag="mp")
                    nc.tensor.matmul(
                        ph[:], lhsT=w2t_km[(0, mm)][:], rhs=l1[0][:],
                        start=True, stop=False,
                    )
                    nc.tensor.matmul(
                        ph[:], lhsT=w2t_km[(1, mm)][:], rhs=l1[1][:],
                        start=False, stop=True,
                    )
                    ht = act_pool.tile([D, P], f32, tag="l2")
                    nc.scalar.activation(ht[:], ph[:], AF.Relu, bias=b2_t[mm][:])
                    hT.append(ht)

                ob = out_pool.tile([P, med], bf, tag="osb")
                for h_i in range(2):
                    n0, n1 = h_i * n_half, (h_i + 1) * n_half
                    po = o_psum.tile([P, n_half], f32, tag="op")
                    nc.tensor.matmul(
                        po[:], lhsT=ones[:1, :], rhs=b3_t[:1, n0:n1],
                        start=True, stop=False,
                    )
                    nc.tensor.matmul(
                        po[:], lhsT=hT[0][:], rhs=w3t_k[0][:, n0:n1],
                        start=False, stop=False,
                    )
                    nc.tensor.matmul(
                        po[:], lhsT=hT[1][:], rhs=w3t_k[1][:, n0:n1],
                        start=False, stop=True,
                    )
                    nc.scalar.activation(ob[:, n0:n1], po[:], AF.Sigmoid)
                nc.scalar.dma_start(out[w * P : (w + 1) * P, :], ob[:])

    nc.compile()
    return nc


def kernel(**inputs) -> np.ndarray:
    from concourse.bass_utils import run_bass_kernel_spmd

    in_maps, cfg = host_prep(inputs)
    nc = build_nc(cfg)
    res = run_bass_kernel_spmd(nc, in_maps, core_ids=list(range(N_CORES)))
    return assemble(res.results, cfg)


# revision 39
# speedup vs baseline: 1.0114x; 1.0109x over previous
"""Trainium2 Bass kernel: 4x EmbeddingBag(sum over 32 codes) + 3-layer MLP.

Data-parallel over 8 NeuronCores (batch 16384 -> 8 x 2048).  Embedding tables
are concatenated (proc offset by +100000), cast to bf16 and split into 5
chunks of <=32000 rows (so per-chunk row indices fit int16 for dma_gather),
each chunk followed by one zero row used as gather padding.  Table rows are
assigned to chunks by a quota-balancing greedy (plus a repair pass) so that
every (core, window, bag) cell's per-chunk lookup counts stay under rotated
multiples-of-128 quotas - minimizing the ceil-128 padding below.

Per core the 262144 lookups (4 bags x 2048 examples x 32 codes) are sorted by
(window of 128 examples, chunk, bag).  Each (win,ck,bag) segment is padded to
a multiple of 128 rows ("blocks") with zero-row fetches.  One dma_gather per
(win, ck) pulls all its blocks' rows (bf16, 256B each) into SBUF in
partition-fastest order.  Per block, the selection matrix E [128 rows x 128
examples] is generated ON-CHIP by the vector engine (tensor_scalar is_equal
of a constant iota row-tile against the block's per-row example ids "mids",
a per-window fp32 DMA), then one PE matmul per block accumulates the rows
into a per-(bag, win) PSUM tile [D=128, 128 examples] in fp32 - start=True on
the first block of each (win,bag), stop on the last.  Pad rows fetch the
chunk's zero row and carry a sentinel mid (no E column), so they add zero.

The MLP then runs per window in fp32: bag sums are already feature-major in
PSUM, copy to SBUF (ACT/DVE), layer1+layer2 feature-major (ACT applies
bias/relu on the PSUM->SBUF copy), layer3 uses the activations as lhsT to
emit example-major [128, 1000] directly (bias via a K=1 ones-row matmul PSUM
init), ACT sigmoid, output DMA'd bf16 on the ACT HWDGE queue (host converts
to fp32).  Weight preloads are emitted after window 0's gathers; the last two
windows' gather ops are split so the drain tail overlaps remaining gathers.

The Bass program structure is shared by all 8 cores (SPMD); per-op sizes are
the max over cores, deficit cores pad with zero-row gathers and sentinel mids.
"""

import numpy as np

B, L, D = 16384, 32, 128
DIAG_LEN, PROC_LEN, MED_LEN = 100000, 50000, 1000
N_CORES = 8
P = 128
CS = 32000          # chunk size (int16-addressable)
NCK = 5             # chunks
WIN = 128           # examples per window
SENT = 200.0        # mids sentinel (never equals iota 0..127; exact in bf16)
QUOTA = (7, 7, 7, 6, 6)  # per-(win,bag) chunk quotas in 128-blocks, rotated
PLACE = False       # example->window placement (didn't help; identity)


def _balance_chunks(rows_all, cell_all, v_cat, nwin):
    """Assign table rows to chunks so that per-(core,win,bag) chunk counts
    stay under rotated 128-multiple quotas (minimizing ceil-128 padding).

    rows_all/cell_all: per-lookup row id and cell id (c*nwin*4 + w*4 + b).
    Returns (asg [v_cat] chunk id, loc [v_cat] position within chunk).
    """
    n_cells = cell_all.max() + 1
    o = np.argsort(rows_all, kind="stable")
    rs, cells_s = rows_all[o], cell_all[o]
    row_start = np.searchsorted(rs, np.arange(v_cat + 1))
    cnts = np.diff(row_start)

    # per-(row, cell) occurrence counts, row-major
    key = rs * n_cells + cells_s
    ukey, uocc = np.unique(key, return_counts=True)
    urow = ukey // n_cells
    ucell = ukey % n_cells
    ustart = np.searchsorted(urow, np.arange(v_cat + 1))

    # quotas per (ck, cell): rotate QUOTA by (w*4+b) % NCK
    j = np.arange(n_cells) % (nwin * 4)
    q = np.array(QUOTA, np.int64) * P
    Q = np.empty((NCK, n_cells), np.int64)
    for ck in range(NCK):
        Q[ck] = q[(ck + j) % NCK]

    wb = j  # cell -> (w*4+b) group id
    n_grp = nwin * 4
    row_order = np.argsort(-cnts, kind="stable")
    BS = 512

    def greedy_pack(Cap0):
        """One greedy packing run with initial per-(ck,cell) ceilings Cap0.
        Ceilings ratchet up by 128 when a (w,b,ck) group overflows (the
        extra block is paid once per group; later rows fill it free)."""
        L = np.zeros((NCK, n_cells), np.int64)
        cap = np.full(NCK, CS, np.int64)
        asg = np.full(v_cat, -1, np.int64)
        C = Cap0.copy()
        for i0 in range(0, v_cat, BS):
            br = row_order[i0 : i0 + BS]
            ent_s = ustart[br]
            ent_e = ustart[br + 1]
            ent_n = ent_e - ent_s
            flat = np.concatenate(
                [np.arange(s, e) for s, e in zip(ent_s, ent_e)]
            ) if ent_n.sum() else np.empty(0, np.int64)
            bounds = np.concatenate([[0], np.cumsum(ent_n)])
            bcell = ucell[flat]
            bocc = uocc[flat]
            nb = br.size
            slack = np.full((NCK, nb), 1 << 30, np.int64)
            has = ent_n > 0
            red_idx = bounds[:-1][has]
            for ck in range(NCK):
                cs_ = C[ck, bcell] - L[ck, bcell] - bocc
                if red_idx.size:
                    slack[ck, has] = np.minimum.reduceat(cs_, red_idx)
                slack[ck, ~has] = 1 << 30
                if cap[ck] <= 0:
                    slack[ck, :] = -(1 << 30)
            choice = np.argmax(slack, axis=0)
            asg[br] = choice
            for ck in range(NCK):
                sel = choice == ck
                cap[ck] -= int(sel.sum())
                csel = np.repeat(sel, ent_n)
                np.add.at(L[ck], bcell[csel], bocc[csel])
                gmax = np.zeros(n_grp, np.int64)
                np.maximum.at(gmax, wb, L[ck])
                gceil = -(-gmax // P) * P
                C[ck] = np.maximum(Cap0[ck], gceil[wb])
        return asg, L, cap, C

    def total_blocks(L):
        t = 0
        for ck in range(NCK):
            gmax = np.zeros(n_grp, np.int64)
            np.maximum.at(gmax, wb, L[ck])
            t += int((-(-gmax // P)).sum())
        return t

    # iterate: re-pack from scratch with ceilings tightened by 128 on groups
    # that overflowed their quota in the best run so far; keep the best
    asg, L, cap, C = greedy_pack(Q)
    best = (total_blocks(L), asg, L, cap, C)
    for _ in range(3):
        _, asg_b, L_b = best[0], best[1], best[2]
        Ct = Q.copy()
        for ck in range(NCK):
            gmax = np.zeros(n_grp, np.int64)
            np.maximum.at(gmax, wb, L_b[ck])
            gceil = -(-gmax // P) * P
            tgt = np.maximum(Q[ck, :], (gceil - P)[wb])
            Ct[ck] = np.minimum(np.maximum(Q[ck], gceil[wb]), tgt + P)
            Ct[ck] = np.maximum(Q[ck], tgt)
        asg, L, cap, C = greedy_pack(Ct)
        tb = total_blocks(L)
        if tb < best[0]:
            best = (tb, asg, L, cap, C)
        else:
            break
    _, asg, L, cap, C = best

    # repair pass: groups (w,b,ck) barely over a 128 boundary -> move rows
    # contributing to the over-boundary cores into chunks with slack
    cell_rows_order = np.argsort(ucell, kind="stable")
    cell_start = np.searchsorted(ucell[cell_rows_order], np.arange(n_cells + 1))
    for _ in range(2):
        gmaxs = np.zeros((NCK, n_grp), np.int64)
        for ck in range(NCK):
            np.maximum.at(gmaxs[ck], wb, L[ck])
        over = gmaxs % P
        order = np.argsort(np.where(over > 0, over, 1 << 30).reshape(-1))
        moved = 0
        for flatg in order:
            ck, g = divmod(int(flatg), n_grp)
            exc = int(over[ck, g])
            if exc == 0 or exc > 48:
                break
            floor_l = gmaxs[ck, g] - exc
            # offending cells of this group
            gcells = np.nonzero(wb == g)[0]
            bad = gcells[L[ck, gcells] > floor_l]
            for cell in bad:
                need = int(L[ck, cell] - floor_l)
                ent = cell_rows_order[cell_start[cell] : cell_start[cell + 1]]
                cand = ent[asg[urow[ent]] == ck]
                # smallest contributors first
                cand = cand[np.argsort(uocc[cand], kind="stable")]
                for e in cand:
                    if need <= 0:
                        break
                    r = int(urow[e])
                    es, ee = int(ustart[r]), int(ustart[r + 1])
                    rc, ro = ucell[es:ee], uocc[es:ee]
                    for ck2 in range(NCK):
                        if ck2 == ck or cap[ck2] <= 0:
                            continue
                        if np.all(C[ck2, rc] - L[ck2, rc] >= ro):
                            asg[r] = ck2
                            L[ck, rc] -= ro
                            L[ck2, rc] += ro
                            cap[ck] += 1
                            cap[ck2] -= 1
                            need -= int(ro[np.nonzero(rc == cell)[0][0]])
                            moved += 1
                            break
        if moved == 0:
            break

    # positions within chunks (original row order)
    loc = np.zeros(v_cat, np.int64)
    for ck in range(NCK):
        sel = np.nonzero(asg == ck)[0]
        loc[sel] = np.arange(sel.size)
    return asg, loc


def _structure(counts):
    """Static program structure from per-core segment counts.

    counts: [n_cores, NWIN, NCK, 4] lookup counts per (win, ck, bag) segment.
    """
    n_cores, NWIN, NCK, NB_ = counts.shape
    cmax = counts.max(axis=0)  # [NWIN, NCK, 4]
    nb = -(-cmax // P)  # ceil -> blocks per segment
    nb[:, 0, :][nb[:, 0, :] == 0] = 1  # ck0 segments host the start=True matmul
    ops = []
    idx_off = 0
    blk_off = 0
    for w in range(NWIN):
        win_blocks = {bg: [] for bg in range(4)}
        win_ops = []
        for ck in range(NCK):
            op_blocks = []
            for bg in range(4):
                for b in range(int(nb[w, ck, bg])):
                    blk = [bg, False, False]
                    op_blocks.append(blk)
                    win_blocks[bg].append(blk)
            # split the last windows' ops so tail compute overlaps remaining
            # gathers; the final window splits at the bag0+1/bag2+3 boundary
            # so half the MLP inputs complete one sub-op early
            if w == NWIN - 1:
                cut = int(nb[w, ck, 0] + nb[w, ck, 1])
                pieces = [op_blocks[:cut], op_blocks[cut:]]
            elif w == NWIN - 2:
                per = -(-len(op_blocks) // 2)
                pieces = [op_blocks[:per], op_blocks[per:]]
            elif w == 0 and ck == 0:
                # tiny head op so the first gather starts ASAP
                pieces = [op_blocks[:4], op_blocks[4:]]
            else:
                pieces = [op_blocks]
            for pb in pieces:
                if not pb:
                    continue
                n_op = len(pb) * P
                win_ops.append(
                    dict(win=w, ck=ck, idx_off=idx_off, blk_off=blk_off,
                         nb=len(pb), n=n_op, blocks=pb)
                )
                idx_off += n_op
                blk_off += len(pb)
        for bg in range(4):
            assert win_blocks[bg], "every bag needs blocks in every window"
            win_blocks[bg][0][1] = True   # start
            win_blocks[bg][-1][2] = True  # stop
        ops.extend(win_ops)
    return dict(ops=ops, tot_idx=idx_off, tot_blk=blk_off, nb_arr=nb,
                NWIN=NWIN, NCK=NCK)


def host_prep(inputs, n_cores=N_CORES):
    import ml_dtypes

    bf16 = ml_dtypes.bfloat16

    diag = np.asarray(inputs["diag_emb"], np.float32)
    proc = np.asarray(inputs["proc_emb"], np.float32)
    v_diag, d = diag.shape
    tcat = np.concatenate([diag, proc], axis=0)
    v_cat = tcat.shape[0]
    assert NCK * CS >= v_cat

    gl = {
        "cd": np.asarray(inputs["diag_codes"], np.int64),
        "cp": np.asarray(inputs["proc_codes"], np.int64) + v_diag,
        "pd": np.asarray(inputs["prev_diag_codes"], np.int64),
        "pp": np.asarray(inputs["prev_proc_codes"], np.int64) + v_diag,
    }
    b_total, l_codes = gl["cd"].shape
    assert b_total % n_cores == 0
    bc = b_total // n_cores
    assert bc % WIN == 0
    NWIN = bc // WIN

    # per-core flat (row, example, bag) streams
    core_g, core_e, core_bag = [], [], []
    for c in range(n_cores):
        gs, bags = [], []
        for bi, name in enumerate(("cd", "cp", "pd", "pp")):
            g = gl[name][c * bc : (c + 1) * bc].reshape(-1)
            gs.append(g)
            bags.append(np.full(g.size, bi, np.int64))
        core_g.append(np.concatenate(gs))
        core_bag.append(np.concatenate(bags))
        core_e.append(np.tile(np.repeat(np.arange(bc, dtype=np.int64), l_codes), 4))

    # balance rows across chunks to minimize ceil-128 padding
    rows_all = np.concatenate(core_g)
    cell_all = np.concatenate(
        [
            c * (NWIN * 4) + (core_e[c] // WIN) * 4 + core_bag[c]
            for c in range(n_cores)
        ]
    )
    asg, lmap = _balance_chunks(rows_all, cell_all, v_cat, NWIN)

    # place examples into windows (core-local permutation) to flatten each
    # window's per-(bag, chunk) lookup counts, then re-balance rows with the
    # easier cell structure.  wmaps[c][e] = permuted example slot.
    wmaps = [np.arange(bc, dtype=np.int64) for _ in range(n_cores)]
    if PLACE:
        for c in range(n_cores):
            prof = np.zeros((bc, 4, NCK), np.int64)
            np.add.at(
                prof.reshape(-1),
                (core_e[c] * 4 + core_bag[c]) * NCK + asg[core_g[c]],
                1,
            )
            prof = prof.reshape(bc, 4 * NCK).astype(np.float64)
            target = prof.sum(axis=0) / bc  # per-example mean profile
            Wsum = np.zeros((NWIN, 4 * NCK), np.float64)
            n_w = np.zeros(NWIN, np.int64)
            # most extreme examples first
            eorder = np.argsort(-np.abs(prof - target).sum(axis=1), kind="stable")
            slot = np.empty(bc, np.int64)
            for e in eorder:
                dev = Wsum + prof[e] - target * (n_w + 1)[:, None]
                score = np.square(dev).sum(axis=1)
                score[n_w >= WIN] = np.inf
                w = int(np.argmin(score))
                slot[e] = w * WIN + n_w[w]
                Wsum[w] += prof[e]
                n_w[w] += 1
            wmaps[c] = slot
        cell_all = np.concatenate(
            [
                c * (NWIN * 4) + (wmaps[c][core_e[c]] // WIN) * 4 + core_bag[c]
                for c in range(n_cores)
            ]
        )
        asg, lmap = _balance_chunks(rows_all, cell_all, v_cat, NWIN)

    tbl_dev = np.zeros(((CS + 1) * NCK, d), bf16)
    tbl_dev[asg * (CS + 1) + lmap] = tcat.astype(bf16)

    # flat per-core lookup streams, sorted by (win, ck, bag)
    per_core = []
    counts = np.zeros((n_cores, NWIN, NCK, 4), np.int64)
    for c in range(n_cores):
        g, bag = core_g[c], core_bag[c]
        e = wmaps[c][core_e[c]]
        ck = asg[g]
        loc = lmap[g]
        win = e // WIN
        m = e % WIN
        seg = (win * NCK + ck) * 4 + bag
        order = np.argsort(seg, kind="stable")
        per_core.append((seg[order], loc[order], m[order]))
        np.add.at(counts[c].reshape(-1), seg, 1)

    st = _structure(counts)
    TOT_IDX, TOT_B = st["tot_idx"], st["tot_blk"]

    # static per-segment offsets
    seg_sizes = st["nb_arr"].reshape(-1) * P
    seg_off = np.concatenate([[0], np.cumsum(seg_sizes)])[:-1]

    in_maps = []
    iota_np = np.broadcast_to(
        np.arange(P, dtype=np.float32), (P, P)
    ).astype(bf16).copy()
    e16_np = (np.arange(P)[None, :] % 16 == np.arange(16)[:, None]).astype(
        np.float32
    )
    for c in range(n_cores):
        seg_s, loc_s, m_s = per_core[c]
        pos_in_seg = np.arange(seg_s.size) - np.concatenate(
            [[0], np.cumsum(np.bincount(seg_s, minlength=seg_sizes.size))]
        )[:-1][seg_s]
        pos = seg_off[seg_s] + pos_in_seg
        idx_flat = np.full(TOT_IDX, CS, np.int16)  # pad -> zero row
        idx_flat[pos] = loc_s.astype(np.int16)
        m_flat = np.full(TOT_IDX, SENT, np.float32)
        m_flat[pos] = m_s
        # pack gidx: position i -> [i%16, i//16], fp32 (replicated to 128
        # partitions on-chip via a PE selection matmul)
        gidx = np.ascontiguousarray(
            idx_flat.reshape(TOT_IDX // 16, 16).T
        ).astype(np.float32)
        # mids: position i -> [i%128, i//128], bf16
        mids = np.ascontiguousarray(m_flat.reshape(TOT_B, P).T)
        in_maps.append(dict(tbl=tbl_dev, gidx=gidx, mids=mids, iota=iota_np,
                            e16=e16_np))

    w1t = np.ascontiguousarray(np.asarray(inputs["W1"], np.float32).T)
    w2t = np.ascontiguousarray(np.asarray(inputs["W2"], np.float32).T)
    w3t = np.ascontiguousarray(np.asarray(inputs["W3"], np.float32).T)
    b1 = np.ascontiguousarray(np.asarray(inputs["b1"], np.float32).reshape(-1, 1))
    b2 = np.ascontiguousarray(np.asarray(inputs["b2"], np.float32).reshape(-1, 1))
    b3 = np.ascontiguousarray(np.asarray(inputs["b3"], np.float32).reshape(1, -1))
    for im in in_maps:
        im.update(w1t=w1t, w2t=w2t, w3t=w3t, b1=b1, b2=b2, b3=b3)

    med = w3t.shape[1]
    cfg = dict(b_core=bc, med=med, v_dev=tbl_dev.shape[0], st=st, wmaps=wmaps)
    return in_maps, cfg


def assemble(results, cfg):
    """Concatenate per-core outputs, undoing the example->window placement
    permutation, and convert to fp32."""
    outs = []
    for c, r in enumerate(results):
        o = r["out"].astype(np.float32)
        outs.append(o[cfg["wmaps"][c]])
    return np.concatenate(outs, axis=0)


def build_nc(cfg):
    import concourse.bass as bass
    import concourse.mybir as mybir
    import concourse.tile as tile
    from concourse import bacc

    f32 = mybir.dt.float32
    bf = mybir.dt.bfloat16
    i16 = mybir.dt.int16
    AF = mybir.ActivationFunctionType
    EQ = mybir.AluOpType.is_equal

    bc, med, v_dev = cfg["b_core"], cfg["med"], cfg["v_dev"]
    st = cfg["st"]
    NWIN, NCK = st["NWIN"], st["NCK"]
    TOT_IDX, TOT_B = st["tot_idx"], st["tot_blk"]
    n_half = med // 2
    assert n_half <= 512

    nc = bacc.Bacc("TRN2", target_bir_lowering=False, debug=False,
                   enable_asserts=False, num_devices=N_CORES)

    tbl = nc.dram_tensor("tbl", [v_dev, D], bf, kind="ExternalInput").ap()
    gidx = nc.dram_tensor("gidx", [16, TOT_IDX // 16], f32, kind="ExternalInput").ap()
    e16 = nc.dram_tensor("e16", [16, P], f32, kind="ExternalInput").ap()
    mids = nc.dram_tensor("mids", [P, TOT_B], f32, kind="ExternalInput").ap()
    iota = nc.dram_tensor("iota", [P, P], bf, kind="ExternalInput").ap()
    w1t = nc.dram_tensor("w1t", [2 * D, D], f32, kind="ExternalInput").ap()
    w2t = nc.dram_tensor("w2t", [2 * D, 2 * D], f32, kind="ExternalInput").ap()
    w3t = nc.dram_tensor("w3t", [2 * D, med], f32, kind="ExternalInput").ap()
    b1 = nc.dram_tensor("b1", [D, 1], f32, kind="ExternalInput").ap()
    b2 = nc.dram_tensor("b2", [2 * D, 1], f32, kind="ExternalInput").ap()
    b3 = nc.dram_tensor("b3", [1, med], f32, kind="ExternalInput").ap()
    out = nc.dram_tensor("out", [bc, med], bf, kind="ExternalOutput").ap()

    ops_by_win = {}
    for op in st["ops"]:
        ops_by_win.setdefault(op["win"], []).append(op)

    with tile.TileContext(nc) as tc:
        with (
            tc.tile_pool(name="const", bufs=1) as cpool,
            tc.tile_pool(name="gi", bufs=4) as gi_pool,
            tc.tile_pool(name="gst", bufs=3) as gst_pool,
            tc.tile_pool(name="mi", bufs=4) as mi_pool,
            tc.tile_pool(name="em", bufs=8) as em_pool,
            tc.tile_pool(name="gath", bufs=8) as gath_pool,
            tc.tile_pool(name="sT", bufs=8) as sT_pool,
            tc.tile_pool(name="acts", bufs=8) as act_pool,
            tc.tile_pool(name="osb", bufs=2) as out_pool,
            tc.tile_pool(name="spsum", bufs=4, space="PSUM") as s_psum,
            tc.tile_pool(name="mpsum", bufs=2, space="PSUM") as m_psum,
            tc.tile_pool(name="opsum", bufs=2, space="PSUM") as o_psum,
        ):
            iota_t = cpool.tile([P, P], bf, tag="iota")
            iota_loaded = [False]
            e16_t = cpool.tile([16, P], f32, tag="e16")
            nc.sync.dma_start(e16_t[:], e16[:, :])

            consts = {}

            def load_consts():
                # Emitted after window 0's gather ops so the first gathers
                # aren't queued behind ~1.3MB of weight preloads.
                ones = cpool.tile([1, P], f32, tag="ones")
                nc.gpsimd.memset(ones[:], 1.0)
                w1t_k = []
                for k in range(2):
                    t = cpool.tile([D, D], f32, tag=f"w1t{k}")
                    nc.sync.dma_start(t[:], w1t[k * D : (k + 1) * D, :])
                    w1t_k.append(t)
                w2t_km = {}
                for k in range(2):
                    for mm in range(2):
                        t = cpool.tile([D, D], f32, tag=f"w2t{k}{mm}")
                        nc.sync.dma_start(
                            t[:], w2t[k * D : (k + 1) * D, mm * D : (mm + 1) * D]
                        )
                        w2t_km[(k, mm)] = t
                w3t_k = []
                for k in range(2):
                    t = cpool.tile([D, med], f32, tag=f"w3t{k}")
                    nc.sync.dma_start(t[:], w3t[k * D : (k + 1) * D, :])
                    w3t_k.append(t)
                b1_t = cpool.tile([D, 1], f32, tag="b1")
                nc.sync.dma_start(b1_t[:], b1[:, :])
                b2_t = []
                for mm in range(2):
                    t = cpool.tile([D, 1], f32, tag=f"b2{mm}")
                    nc.sync.dma_start(t[:], b2[mm * D : (mm + 1) * D, :])
                    b2_t.append(t)
                b3_t = cpool.tile([1, med], f32, tag="b3")
                nc.sync.dma_start(b3_t[:], b3[:, :])
                consts.update(ones=ones, w1t_k=w1t_k, w2t_km=w2t_km,
                              w3t_k=w3t_k, b1_t=b1_t, b2_t=b2_t, b3_t=b3_t)

            for rep in range(cfg.get("reps", 1)):
              win_feed = {}

              def produce_feed(w):
                # stage the window's fp32 gidx [16, cols], replicate to 128
                # partitions on the PE (out[m,n] = gst[m%16,n]) and cast
                # fp32->int16 on DVE.  Called 2 windows ahead of use so the
                # in-order PE stream keeps the gather feed ahead of the
                # E-matmuls.
                wops = ops_by_win[w]
                w_idx_off = wops[0]["idx_off"]
                w_blk_off = wops[0]["blk_off"]
                w_n = sum(op["n"] for op in wops)
                w_nb = sum(op["nb"] for op in wops)
                cols = w_n // 16
                gst = gst_pool.tile([16, cols], f32, tag="gst")
                if not cfg.get("skip_gi"):
                    nc.sync.dma_start(
                        gst[:],
                        gidx[:, w_idx_off // 16 : (w_idx_off + w_n) // 16],
                    )
                gi = gi_pool.tile([P, cols], i16, tag="gi")
                for s in range(0, cols, 512):
                    nn = min(512, cols - s)
                    rp = o_psum.tile([P, 512], f32, tag="op", name="rp")
                    nc.tensor.matmul(
                        rp[:, :nn], lhsT=e16_t[:], rhs=gst[:, s : s + nn],
                        start=True, stop=True,
                    )
                    nc.vector.tensor_copy(gi[:, s : s + nn], rp[:, :nn])
                mi = mi_pool.tile([P, w_nb], f32, tag="mi")
                nc.sync.dma_start(
                    mi[:], mids[:, w_blk_off : w_blk_off + w_nb]
                )
                if not iota_loaded[0]:
                    nc.sync.dma_start(iota_t[:], iota[:, :])
                    iota_loaded[0] = True
                win_feed[w] = (gi, mi, w_idx_off, w_blk_off)

              produce_feed(0)
              produce_feed(1)
              for w in range(NWIN):
                s_ps = [s_psum.tile([D, WIN], f32, tag="s", name=f"s{rep}_{w}_{i}") for i in range(4)]
                if w + 2 < NWIN:
                    produce_feed(w + 2)
                gi, mi, w_idx_off, w_blk_off = win_feed.pop(w)
                wops = ops_by_win[w]
                for op in wops:
                    n, nb = op["n"], op["nb"]
                    o16 = (op["idx_off"] - w_idx_off) // 16
                    ob0 = op["blk_off"] - w_blk_off
                    gt = gath_pool.tile([P, nb * D], bf, tag="gath")
                    if not cfg.get("skip_gather"):
                        nc.gpsimd.dma_gather(
                            out_ap=gt[:].rearrange("p (c d) -> p c d", d=D),
                            in_ap=tbl[
                                op["ck"] * (CS + 1) : (op["ck"] + 1) * (CS + 1), :
                            ],
                            idxs_ap=gi[:, o16 : o16 + n // 16],
                            num_idxs=n,
                            num_idxs_reg=n,
                            elem_size=D,
                            single_packet=False,
                        )
                    gt3 = gt[:].rearrange("p (c d) -> p c d", d=D)
                    em = em_pool.tile([P, P * nb], bf, tag="em")
                    for b in range(nb):
                        nc.vector.tensor_scalar(
                            em[:, b * P : (b + 1) * P],
                            iota_t[:],
                            mi[:, ob0 + b : ob0 + b + 1],
                            None,
                            EQ,
                        )
                    if cfg.get("skip_smm"):
                        continue
                    for b, (bg, start, stop) in enumerate(op["blocks"]):
                        nc.tensor.matmul(
                            s_ps[bg][:],
                            lhsT=gt3[:, b, :],
                            rhs=em[:, b * P : (b + 1) * P],
                            start=start,
                            stop=stop,
                            skip_group_check=True,
                        )
                if not consts:
                    load_consts()
                ones = consts["ones"]
                w1t_k, w2t_km = consts["w1t_k"], consts["w2t_km"]
                w3t_k = consts["w3t_k"]
                b1_t, b2_t, b3_t = consts["b1_t"], consts["b2_t"], consts["b3_t"]
                if cfg.get("skip_mlp"):
                    continue
                # bag sums (feature-major) PSUM -> SBUF on ACT
                sT = []
                for bg in range(4):
                    t = sT_pool.tile([D, P], f32, tag="sT", name=f"sT{w}_{bg}")
                    if bg % 2 == 0:
                        nc.scalar.activation(t[:], s_ps[bg][:], AF.Copy)
                    else:
                        nc.vector.tensor_copy(t[:], s_ps[bg][:])
                    sT.append(t)

                l1 = []
                for ka, kb in ((0, 1), (2, 3)):
                    pc = m_psum.tile([P, P], f32, tag="mp")
                    nc.tensor.matmul(
                        pc[:], lhsT=w1t_k[0][:], rhs=sT[ka][:], start=True, stop=False
                    )
                    nc.tensor.matmul(
                        pc[:], lhsT=w1t_k[1][:], rhs=sT[kb][:], start=False, stop=True
                    )
                    xt = act_pool.tile([D, P], f32, tag="l1")
                    nc.scalar.activation(xt[:], pc[:], AF.Identity, bias=b1_t[:])
                    l1.append(xt)

                hT = []
                for mm in range(2):
                    ph = m_psum.tile([P, P], f32, tag="mp")
                    nc.tensor.matmul(
                        ph[:], lhsT=w2t_km[(0, mm)][:], rhs=l1[0][:],
                        start=True, stop=False,
                    )
                    nc.tensor.matmul(
                        ph[:], lhsT=w2t_km[(1, mm)][:], rhs=l1[1][:],
                        start=False, stop=True,
                    )
                    ht = act_pool.tile([D, P], f32, tag="l2")
                    nc.scalar.activation(ht[:], ph[:], AF.Relu, bias=b2_t[mm][:])
                    hT.append(ht)

                ob = out_pool.tile([P, med], bf, tag="osb")
                for h_i in range(2):
                    n0, n1 = h_i * n_half, (h_i + 1) * n_half
                    po = o_psum.tile([P, n_half], f32, tag="op")
                    nc.tensor.matmul(
                        po[:], lhsT=ones[:1, :], rhs=b3_t[:1, n0:n1],
                        start=True, stop=False,
                    )
                    nc.tensor.matmul(
                        po[:], lhsT=hT[0][:], rhs=w3t_k[0][:, n0:n1],
                        start=False, stop=False,
                    )
                    nc.tensor.matmul(
                        po[:], lhsT=hT[1][:], rhs=w3t_k[1][:, n0:n1],
                        start=False, stop=True,
                    )
                    nc.scalar.activation(ob[:, n0:n1], po[:], AF.Sigmoid)
                nc.scalar.dma_start(out[w * P : (w + 1) * P, :], ob[:])

    nc.compile()
    return nc


def kernel(**inputs) -> np.ndarray:
    from concourse.bass_utils import run_bass_kernel_spmd

    in_maps, cfg = host_prep(inputs)
    nc = build_nc(cfg)
    res = run_bass_kernel_spmd(nc, in_maps, core_ids=list(range(N_CORES)))
    return assemble(res.results, cfg)


# revision 42
# speedup vs baseline: 1.0228x; 1.0113x over previous
"""Trainium2 Bass kernel: 4x EmbeddingBag(sum over 32 codes) + 3-layer MLP.

Data-parallel over 8 NeuronCores (batch 16384 -> 8 x 2048).  Embedding tables
are concatenated (proc offset by +100000), cast to bf16 and split into 5
chunks of <=32000 rows (so per-chunk row indices fit int16 for dma_gather),
each chunk followed by one zero row used as gather padding.  Table rows are
assigned to chunks by a quota-balancing greedy (plus a repair pass) so that
every (core, window, bag) cell's per-chunk lookup counts stay under rotated
multiples-of-128 quotas - minimizing the ceil-128 padding below.

Per core the 262144 lookups (4 bags x 2048 examples x 32 codes) are sorted by
(window of 128 examples, chunk, bag).  Each (win,ck,bag) segment is padded to
a multiple of 128 rows ("blocks") with zero-row fetches.  One dma_gather per
(win, ck) pulls all its blocks' rows (bf16, 256B each) into SBUF in
partition-fastest order.  Per block, the selection matrix E [128 rows x 128
examples] is generated ON-CHIP by the vector engine (tensor_scalar is_equal
of a constant iota row-tile against the block's per-row example ids "mids",
a per-window fp32 DMA), then one PE matmul per block accumulates the rows
into a per-(bag, win) PSUM tile [D=128, 128 examples] in fp32 - start=True on
the first block of each (win,bag), stop on the last.  Pad rows fetch the
chunk's zero row and carry a sentinel mid (no E column), so they add zero.

The MLP then runs per window in fp32: bag sums are already feature-major in
PSUM, copy to SBUF (ACT/DVE), layer1+layer2 feature-major (ACT applies
bias/relu on the PSUM->SBUF copy), layer3 uses the activations as lhsT to
emit example-major [128, 1000] directly (bias via a K=1 ones-row matmul PSUM
init), ACT sigmoid, output DMA'd bf16 on the ACT HWDGE queue (host converts
to fp32).  Weight preloads are emitted after window 0's gathers; the last two
windows' gather ops are split so the drain tail overlaps remaining gathers.

The Bass program structure is shared by all 8 cores (SPMD); per-op sizes are
the max over cores, deficit cores pad with zero-row gathers and sentinel mids.
"""

import numpy as np

B, L, D = 16384, 32, 128
DIAG_LEN, PROC_LEN, MED_LEN = 100000, 50000, 1000
N_CORES = 8
P = 128
CS = 32000          # chunk size (int16-addressable)
NCK = 5             # chunks
WIN = 128           # examples per window
SENT = 200.0        # mids sentinel (never equals iota 0..127; exact in bf16)
QUOTA = (7, 7, 7, 6, 6)  # per-(win,bag) chunk quotas in 128-blocks, rotated
PLACE = False       # example->window placement (didn't help; identity)


def _balance_chunks(rows_all, cell_all, v_cat, nwin):
    """Assign table rows to chunks so that per-(core,win,bag) chunk counts
    stay under rotated 128-multiple quotas (minimizing ceil-128 padding).

    rows_all/cell_all: per-lookup row id and cell id (c*nwin*4 + w*4 + b).
    Returns (asg [v_cat] chunk id, loc [v_cat] position within chunk).
    """
    n_cells = cell_all.max() + 1
    o = np.argsort(rows_all, kind="stable")
    rs, cells_s = rows_all[o], cell_all[o]
    row_start = np.searchsorted(rs, np.arange(v_cat + 1))
    cnts = np.diff(row_start)

    # per-(row, cell) occurrence counts, row-major
    key = rs * n_cells + cells_s
    ukey, uocc = np.unique(key, return_counts=True)
    urow = ukey // n_cells
    ucell = ukey % n_cells
    ustart = np.searchsorted(urow, np.arange(v_cat + 1))

    # quotas per (ck, cell): rotate QUOTA by (w*4+b) % NCK
    j = np.arange(n_cells) % (nwin * 4)
    q = np.array(QUOTA, np.int64) * P
    Q = np.empty((NCK, n_cells), np.int64)
    for ck in range(NCK):
        Q[ck] = q[(ck + j) % NCK]

    wb = j  # cell -> (w*4+b) group id
    n_grp = nwin * 4
    row_order = np.argsort(-cnts, kind="stable")
    BS = 512

    def greedy_pack(Cap0):
        """One greedy packing run with initial per-(ck,cell) ceilings Cap0.
        Ceilings ratchet up by 128 when a (w,b,ck) group overflows (the
        extra block is paid once per group; later rows fill it free)."""
        L = np.zeros((NCK, n_cells), np.int64)
        cap = np.full(NCK, CS, np.int64)
        asg = np.full(v_cat, -1, np.int64)
        C = Cap0.copy()
        for i0 in range(0, v_cat, BS):
            br = row_order[i0 : i0 + BS]
            ent_s = ustart[br]
            ent_e = ustart[br + 1]
            ent_n = ent_e - ent_s
            flat = np.concatenate(
                [np.arange(s, e) for s, e in zip(ent_s, ent_e)]
            ) if ent_n.sum() else np.empty(0, np.int64)
            bounds = np.concatenate([[0], np.cumsum(ent_n)])
            bcell = ucell[flat]
            bocc = uocc[flat]
            nb = br.size
            slack = np.full((NCK, nb), 1 << 30, np.int64)
            has = ent_n > 0
            red_idx = bounds[:-1][has]
            for ck in range(NCK):
                cs_ = C[ck, bcell] - L[ck, bcell] - bocc
                if red_idx.size:
                    slack[ck, has] = np.minimum.reduceat(cs_, red_idx)
                slack[ck, ~has] = 1 << 30
                if cap[ck] <= 0:
                    slack[ck, :] = -(1 << 30)
            choice = np.argmax(slack, axis=0)
            asg[br] = choice
            for ck in range(NCK):
                sel = choice == ck
                cap[ck] -= int(sel.sum())
                csel = np.repeat(sel, ent_n)
                np.add.at(L[ck], bcell[csel], bocc[csel])
                gmax = np.zeros(n_grp, np.int64)
                np.maximum.at(gmax, wb, L[ck])
                gceil = -(-gmax // P) * P
                C[ck] = np.maximum(Cap0[ck], gceil[wb])
        return asg, L, cap, C

    def total_blocks(L):
        t = 0
        for ck in range(NCK):
            gmax = np.zeros(n_grp, np.int64)
            np.maximum.at(gmax, wb, L[ck])
            t += int((-(-gmax // P)).sum())
        return t

    # iterate: re-pack from scratch with ceilings tightened by 128 on groups
    # that overflowed their quota in the best run so far; keep the best
    asg, L, cap, C = greedy_pack(Q)
    best = (total_blocks(L), asg, L, cap, C)
    for _ in range(3):
        _, asg_b, L_b = best[0], best[1], best[2]
        Ct = Q.copy()
        for ck in range(NCK):
            gmax = np.zeros(n_grp, np.int64)
            np.maximum.at(gmax, wb, L_b[ck])
            gceil = -(-gmax // P) * P
            tgt = np.maximum(Q[ck, :], (gceil - P)[wb])
            Ct[ck] = np.minimum(np.maximum(Q[ck], gceil[wb]), tgt + P)
            Ct[ck] = np.maximum(Q[ck], tgt)
        asg, L, cap, C = greedy_pack(Ct)
        tb = total_blocks(L)
        if tb < best[0]:
            best = (tb, asg, L, cap, C)
        else:
            break
    _, asg, L, cap, C = best

    # repair pass: groups (w,b,ck) barely over a 128 boundary -> move rows
    # contributing to the over-boundary cores into chunks with slack
    cell_rows_order = np.argsort(ucell, kind="stable")
    cell_start = np.searchsorted(ucell[cell_rows_order], np.arange(n_cells + 1))
    for _ in range(2):
        gmaxs = np.zeros((NCK, n_grp), np.int64)
        for ck in range(NCK):
            np.maximum.at(gmaxs[ck], wb, L[ck])
        over = gmaxs % P
        order = np.argsort(np.where(over > 0, over, 1 << 30).reshape(-1))
        moved = 0
        for flatg in order:
            ck, g = divmod(int(flatg), n_grp)
            exc = int(over[ck, g])
            if exc == 0 or exc > 48:
                break
            floor_l = gmaxs[ck, g] - exc
            # offending cells of this group
            gcells = np.nonzero(wb == g)[0]
            bad = gcells[L[ck, gcells] > floor_l]
            for cell in bad:
                need = int(L[ck, cell] - floor_l)
                ent = cell_rows_order[cell_start[cell] : cell_start[cell + 1]]
                cand = ent[asg[urow[ent]] == ck]
                # smallest contributors first
                cand = cand[np.argsort(uocc[cand], kind="stable")]
                for e in cand:
                    if need <= 0:
                        break
                    r = int(urow[e])
                    es, ee = int(ustart[r]), int(ustart[r + 1])
                    rc, ro = ucell[es:ee], uocc[es:ee]
                    for ck2 in range(NCK):
                        if ck2 == ck or cap[ck2] <= 0:
                            continue
                        if np.all(C[ck2, rc] - L[ck2, rc] >= ro):
                            asg[r] = ck2
                            L[ck, rc] -= ro
                            L[ck2, rc] += ro
                            cap[ck] += 1
                            cap[ck2] -= 1
                            need -= int(ro[np.nonzero(rc == cell)[0][0]])
                            moved += 1
                            break
        if moved == 0:
            break

    # positions within chunks (original row order)
    loc = np.zeros(v_cat, np.int64)
    for ck in range(NCK):
        sel = np.nonzero(asg == ck)[0]
        loc[sel] = np.arange(sel.size)
    return asg, loc


def _structure(counts):
    """Static program structure from per-core segment counts.

    counts: [n_cores, NWIN, NCK, 4] lookup counts per (win, ck, bag) segment.
    """
    n_cores, NWIN, NCK, NB_ = counts.shape
    cmax = counts.max(axis=0)  # [NWIN, NCK, 4]
    nb = -(-cmax // P)  # ceil -> blocks per segment
    nb[:, 0, :][nb[:, 0, :] == 0] = 1  # ck0 segments host the start=True matmul
    ops = []
    idx_off = 0
    blk_off = 0
    for w in range(NWIN):
        win_blocks = {bg: [] for bg in range(4)}
        win_ops = []
        for ck in range(NCK):
            op_blocks = []
            for bg in range(4):
                for b in range(int(nb[w, ck, bg])):
                    blk = [bg, False, False]
                    op_blocks.append(blk)
                    win_blocks[bg].append(blk)
            # split the last windows' ops so tail compute overlaps remaining
            # gathers; the final window splits at the bag0+1/bag2+3 boundary
            # so half the MLP inputs complete one sub-op early
            if w == NWIN - 1:
                cut = int(nb[w, ck, 0] + nb[w, ck, 1])
                pieces = [op_blocks[:cut], op_blocks[cut:]]
            elif w == NWIN - 2:
                per = -(-len(op_blocks) // 2)
                pieces = [op_blocks[:per], op_blocks[per:]]
            elif w == 0 and ck == 0:
                # tiny head op so the first gather starts ASAP
                pieces = [op_blocks[:4], op_blocks[4:]]
            else:
                pieces = [op_blocks]
            for pb in pieces:
                if not pb:
                    continue
                n_op = len(pb) * P
                win_ops.append(
                    dict(win=w, ck=ck, idx_off=idx_off, blk_off=blk_off,
                         nb=len(pb), n=n_op, blocks=pb)
                )
                idx_off += n_op
                blk_off += len(pb)
        for bg in range(4):
            assert win_blocks[bg], "every bag needs blocks in every window"
            win_blocks[bg][0][1] = True   # start
            win_blocks[bg][-1][2] = True  # stop
        ops.extend(win_ops)
    return dict(ops=ops, tot_idx=idx_off, tot_blk=blk_off, nb_arr=nb,
                NWIN=NWIN, NCK=NCK)


def host_prep(inputs, n_cores=N_CORES):
    import ml_dtypes

    bf16 = ml_dtypes.bfloat16

    diag = np.asarray(inputs["diag_emb"], np.float32)
    proc = np.asarray(inputs["proc_emb"], np.float32)
    v_diag, d = diag.shape
    tcat = np.concatenate([diag, proc], axis=0)
    v_cat = tcat.shape[0]
    assert NCK * CS >= v_cat

    gl = {
        "cd": np.asarray(inputs["diag_codes"], np.int64),
        "cp": np.asarray(inputs["proc_codes"], np.int64) + v_diag,
        "pd": np.asarray(inputs["prev_diag_codes"], np.int64),
        "pp": np.asarray(inputs["prev_proc_codes"], np.int64) + v_diag,
    }
    b_total, l_codes = gl["cd"].shape
    assert b_total % n_cores == 0
    bc = b_total // n_cores
    assert bc % WIN == 0
    NWIN = bc // WIN

    # per-core flat (row, example, bag) streams
    core_g, core_e, core_bag = [], [], []
    for c in range(n_cores):
        gs, bags = [], []
        for bi, name in enumerate(("cd", "cp", "pd", "pp")):
            g = gl[name][c * bc : (c + 1) * bc].reshape(-1)
            gs.append(g)
            bags.append(np.full(g.size, bi, np.int64))
        core_g.append(np.concatenate(gs))
        core_bag.append(np.concatenate(bags))
        core_e.append(np.tile(np.repeat(np.arange(bc, dtype=np.int64), l_codes), 4))

    # balance rows across chunks to minimize ceil-128 padding
    rows_all = np.concatenate(core_g)
    cell_all = np.concatenate(
        [
            c * (NWIN * 4) + (core_e[c] // WIN) * 4 + core_bag[c]
            for c in range(n_cores)
        ]
    )
    asg, lmap = _balance_chunks(rows_all, cell_all, v_cat, NWIN)

    # place examples into windows (core-local permutation) to flatten each
    # window's per-(bag, chunk) lookup counts, then re-balance rows with the
    # easier cell structure.  wmaps[c][e] = permuted example slot.
    wmaps = [np.arange(bc, dtype=np.int64) for _ in range(n_cores)]
    if PLACE:
        for c in range(n_cores):
            prof = np.zeros((bc, 4, NCK), np.int64)
            np.add.at(
                prof.reshape(-1),
                (core_e[c] * 4 + core_bag[c]) * NCK + asg[core_g[c]],
                1,
            )
            prof = prof.reshape(bc, 4 * NCK).astype(np.float64)
            target = prof.sum(axis=0) / bc  # per-example mean profile
            Wsum = np.zeros((NWIN, 4 * NCK), np.float64)
            n_w = np.zeros(NWIN, np.int64)
            # most extreme examples first
            eorder = np.argsort(-np.abs(prof - target).sum(axis=1), kind="stable")
            slot = np.empty(bc, np.int64)
            for e in eorder:
                dev = Wsum + prof[e] - target * (n_w + 1)[:, None]
                score = np.square(dev).sum(axis=1)
                score[n_w >= WIN] = np.inf
                w = int(np.argmin(score))
                slot[e] = w * WIN + n_w[w]
                Wsum[w] += prof[e]
                n_w[w] += 1
            wmaps[c] = slot
        cell_all = np.concatenate(
            [
                c * (NWIN * 4) + (wmaps[c][core_e[c]] // WIN) * 4 + core_bag[c]
                for c in range(n_cores)
            ]
        )
        asg, lmap = _balance_chunks(rows_all, cell_all, v_cat, NWIN)

    tbl_dev = np.zeros(((CS + 1) * NCK, d), bf16)
    tbl_dev[asg * (CS + 1) + lmap] = tcat.astype(bf16)

    # flat per-core lookup streams, sorted by (win, ck, bag)
    per_core = []
    counts = np.zeros((n_cores, NWIN, NCK, 4), np.int64)
    for c in range(n_cores):
        g, bag = core_g[c], core_bag[c]
        e = wmaps[c][core_e[c]]
        ck = asg[g]
        loc = lmap[g]
        win = e // WIN
        m = e % WIN
        seg = (win * NCK + ck) * 4 + bag
        order = np.argsort(seg, kind="stable")
        per_core.append((seg[order], loc[order], m[order]))
        np.add.at(counts[c].reshape(-1), seg, 1)

    st = _structure(counts)
    TOT_IDX, TOT_B = st["tot_idx"], st["tot_blk"]

    # static per-segment offsets
    seg_sizes = st["nb_arr"].reshape(-1) * P
    seg_off = np.concatenate([[0], np.cumsum(seg_sizes)])[:-1]

    in_maps = []
    iota_np = np.broadcast_to(
        np.arange(P, dtype=np.float32), (P, P)
    ).astype(bf16).copy()
    e16_np = (np.arange(P)[None, :] % 16 == np.arange(16)[:, None]).astype(
        np.float32
    )
    for c in range(n_cores):
        seg_s, loc_s, m_s = per_core[c]
        pos_in_seg = np.arange(seg_s.size) - np.concatenate(
            [[0], np.cumsum(np.bincount(seg_s, minlength=seg_sizes.size))]
        )[:-1][seg_s]
        pos = seg_off[seg_s] + pos_in_seg
        idx_flat = np.full(TOT_IDX, CS, np.int16)  # pad -> zero row
        idx_flat[pos] = loc_s.astype(np.int16)
        m_flat = np.full(TOT_IDX, SENT, np.float32)
        m_flat[pos] = m_s
        # pack gidx: position i -> [i%16, i//16], fp32 (replicated to 128
        # partitions on-chip via a PE selection matmul)
        gidx = np.ascontiguousarray(
            idx_flat.reshape(TOT_IDX // 16, 16).T
        ).astype(np.float32)
        # mids: position i -> [i%128, i//128], bf16
        mids = np.ascontiguousarray(m_flat.reshape(TOT_B, P).T)
        in_maps.append(dict(tbl=tbl_dev, gidx=gidx, mids=mids, iota=iota_np,
                            e16=e16_np))

    w1t = np.ascontiguousarray(np.asarray(inputs["W1"], np.float32).T).astype(bf16)
    w2t = np.ascontiguousarray(np.asarray(inputs["W2"], np.float32).T).astype(bf16)
    w3t = np.ascontiguousarray(np.asarray(inputs["W3"], np.float32).T).astype(bf16)
    b1 = np.ascontiguousarray(np.asarray(inputs["b1"], np.float32).reshape(-1, 1))
    b2 = np.ascontiguousarray(np.asarray(inputs["b2"], np.float32).reshape(-1, 1))
    b3 = np.ascontiguousarray(np.asarray(inputs["b3"], np.float32).reshape(1, -1).astype(bf16))
    for im in in_maps:
        im.update(w1t=w1t, w2t=w2t, w3t=w3t, b1=b1, b2=b2, b3=b3)

    med = w3t.shape[1]
    cfg = dict(b_core=bc, med=med, v_dev=tbl_dev.shape[0], st=st, wmaps=wmaps)
    return in_maps, cfg


def assemble(results, cfg):
    """Concatenate per-core outputs, undoing the example->window placement
    permutation, and convert to fp32."""
    outs = []
    for c, r in enumerate(results):
        o = r["out"].astype(np.float32)
        outs.append(o[cfg["wmaps"][c]])
    return np.concatenate(outs, axis=0)


def build_nc(cfg):
    import concourse.bass as bass
    import concourse.mybir as mybir
    import concourse.tile as tile
    from concourse import bacc

    f32 = mybir.dt.float32
    bf = mybir.dt.bfloat16
    i16 = mybir.dt.int16
    AF = mybir.ActivationFunctionType
    EQ = mybir.AluOpType.is_equal

    bc, med, v_dev = cfg["b_core"], cfg["med"], cfg["v_dev"]
    st = cfg["st"]
    NWIN, NCK = st["NWIN"], st["NCK"]
    TOT_IDX, TOT_B = st["tot_idx"], st["tot_blk"]
    n_half = med // 2
    assert n_half <= 512

    nc = bacc.Bacc("TRN2", target_bir_lowering=False, debug=False,
                   enable_asserts=False, num_devices=N_CORES)

    tbl = nc.dram_tensor("tbl", [v_dev, D], bf, kind="ExternalInput").ap()
    gidx = nc.dram_tensor("gidx", [16, TOT_IDX // 16], f32, kind="ExternalInput").ap()
    e16 = nc.dram_tensor("e16", [16, P], f32, kind="ExternalInput").ap()
    mids = nc.dram_tensor("mids", [P, TOT_B], f32, kind="ExternalInput").ap()
    iota = nc.dram_tensor("iota", [P, P], bf, kind="ExternalInput").ap()
    w1t = nc.dram_tensor("w1t", [2 * D, D], bf, kind="ExternalInput").ap()
    w2t = nc.dram_tensor("w2t", [2 * D, 2 * D], bf, kind="ExternalInput").ap()
    w3t = nc.dram_tensor("w3t", [2 * D, med], bf, kind="ExternalInput").ap()
    b1 = nc.dram_tensor("b1", [D, 1], f32, kind="ExternalInput").ap()
    b2 = nc.dram_tensor("b2", [2 * D, 1], f32, kind="ExternalInput").ap()
    b3 = nc.dram_tensor("b3", [1, med], bf, kind="ExternalInput").ap()
    out = nc.dram_tensor("out", [bc, med], bf, kind="ExternalOutput").ap()

    ops_by_win = {}
    for op in st["ops"]:
        ops_by_win.setdefault(op["win"], []).append(op)

    with tile.TileContext(nc) as tc:
        with (
            tc.tile_pool(name="const", bufs=1) as cpool,
            tc.tile_pool(name="gi", bufs=4) as gi_pool,
            tc.tile_pool(name="gst", bufs=3) as gst_pool,
            tc.tile_pool(name="mi", bufs=4) as mi_pool,
            tc.tile_pool(name="em", bufs=8) as em_pool,
            tc.tile_pool(name="gath", bufs=8) as gath_pool,
            tc.tile_pool(name="sT", bufs=8) as sT_pool,
            tc.tile_pool(name="acts", bufs=8) as act_pool,
            tc.tile_pool(name="osb", bufs=2) as out_pool,
            tc.tile_pool(name="spsum", bufs=4, space="PSUM") as s_psum,
            tc.tile_pool(name="mpsum", bufs=2, space="PSUM") as m_psum,
            tc.tile_pool(name="opsum", bufs=2, space="PSUM") as o_psum,
        ):
            iota_t = cpool.tile([P, P], bf, tag="iota")
            iota_loaded = [False]
            e16_t = cpool.tile([16, P], f32, tag="e16")
            nc.sync.dma_start(e16_t[:], e16[:, :])

            consts = {}

            def load_consts():
                # Emitted after window 0's gather ops so the first gathers
                # aren't queued behind ~1.3MB of weight preloads.
                ones = cpool.tile([1, P], bf, tag="ones")
                nc.gpsimd.memset(ones[:], 1.0)
                w1t_k = []
                for k in range(2):
                    t = cpool.tile([D, D], bf, tag=f"w1t{k}")
                    nc.sync.dma_start(t[:], w1t[k * D : (k + 1) * D, :])
                    w1t_k.append(t)
                w2t_km = {}
                for k in range(2):
                    for mm in range(2):
                        t = cpool.tile([D, D], bf, tag=f"w2t{k}{mm}")
                        nc.sync.dma_start(
                            t[:], w2t[k * D : (k + 1) * D, mm * D : (mm + 1) * D]
                        )
                        w2t_km[(k, mm)] = t
                w3t_k = []
                for k in range(2):
                    t = cpool.tile([D, med], bf, tag=f"w3t{k}")
                    nc.sync.dma_start(t[:], w3t[k * D : (k + 1) * D, :])
                    w3t_k.append(t)
                b1_t = cpool.tile([D, 1], f32, tag="b1")
                nc.sync.dma_start(b1_t[:], b1[:, :])
                b2_t = []
                for mm in range(2):
                    t = cpool.tile([D, 1], f32, tag=f"b2{mm}")
                    nc.sync.dma_start(t[:], b2[mm * D : (mm + 1) * D, :])
                    b2_t.append(t)
                b3_t = cpool.tile([1, med], bf, tag="b3")
                nc.sync.dma_start(b3_t[:], b3[:, :])
                consts.update(ones=ones, w1t_k=w1t_k, w2t_km=w2t_km,
                              w3t_k=w3t_k, b1_t=b1_t, b2_t=b2_t, b3_t=b3_t)

            for rep in range(cfg.get("reps", 1)):
              win_feed = {}

              def produce_feed(w):
                # stage the window's fp32 gidx [16, cols], replicate to 128
                # partitions on the PE (out[m,n] = gst[m%16,n]) and cast
                # fp32->int16 on DVE.  Called 2 windows ahead of use so the
                # in-order PE stream keeps the gather feed ahead of the
                # E-matmuls.
                wops = ops_by_win[w]
                w_idx_off = wops[0]["idx_off"]
                w_blk_off = wops[0]["blk_off"]
                w_n = sum(op["n"] for op in wops)
                w_nb = sum(op["nb"] for op in wops)
                cols = w_n // 16
                gst = gst_pool.tile([16, cols], f32, tag="gst")
                if not cfg.get("skip_gi"):
                    nc.sync.dma_start(
                        gst[:],
                        gidx[:, w_idx_off // 16 : (w_idx_off + w_n) // 16],
                    )
                gi = gi_pool.tile([P, cols], i16, tag="gi")
                for s in range(0, cols, 512):
                    nn = min(512, cols - s)
                    rp = o_psum.tile([P, 512], f32, tag="op", name="rp")
                    nc.tensor.matmul(
                        rp[:, :nn], lhsT=e16_t[:], rhs=gst[:, s : s + nn],
                        start=True, stop=True,
                    )
                    nc.vector.tensor_copy(gi[:, s : s + nn], rp[:, :nn])
                mi = mi_pool.tile([P, w_nb], f32, tag="mi")
                nc.sync.dma_start(
                    mi[:], mids[:, w_blk_off : w_blk_off + w_nb]
                )
                if not iota_loaded[0]:
                    nc.sync.dma_start(iota_t[:], iota[:, :])
                    iota_loaded[0] = True
                win_feed[w] = (gi, mi, w_idx_off, w_blk_off)

              produce_feed(0)
              produce_feed(1)
              for w in range(NWIN):
                s_ps = [s_psum.tile([D, WIN], f32, tag="s", name=f"s{rep}_{w}_{i}") for i in range(4)]
                if w + 2 < NWIN:
                    produce_feed(w + 2)
                gi, mi, w_idx_off, w_blk_off = win_feed.pop(w)
                wops = ops_by_win[w]
                for op in wops:
                    n, nb = op["n"], op["nb"]
                    o16 = (op["idx_off"] - w_idx_off) // 16
                    ob0 = op["blk_off"] - w_blk_off
                    gt = gath_pool.tile([P, nb * D], bf, tag="gath")
                    if not cfg.get("skip_gather"):
                        nc.gpsimd.dma_gather(
                            out_ap=gt[:].rearrange("p (c d) -> p c d", d=D),
                            in_ap=tbl[
                                op["ck"] * (CS + 1) : (op["ck"] + 1) * (CS + 1), :
                            ],
                            idxs_ap=gi[:, o16 : o16 + n // 16],
                            num_idxs=n,
                            num_idxs_reg=n,
                            elem_size=D,
                            single_packet=False,
                        )
                    gt3 = gt[:].rearrange("p (c d) -> p c d", d=D)
                    em = em_pool.tile([P, P * nb], bf, tag="em")
                    for b in range(nb):
                        nc.vector.tensor_scalar(
                            em[:, b * P : (b + 1) * P],
                            iota_t[:],
                            mi[:, ob0 + b : ob0 + b + 1],
                            None,
                            EQ,
                        )
                    if cfg.get("skip_smm"):
                        continue
                    for b, (bg, start, stop) in enumerate(op["blocks"]):
                        nc.tensor.matmul(
                            s_ps[bg][:],
                            lhsT=gt3[:, b, :],
                            rhs=em[:, b * P : (b + 1) * P],
                            start=start,
                            stop=stop,
                            skip_group_check=True,
                        )
                if not consts:
                    load_consts()
                ones = consts["ones"]
                w1t_k, w2t_km = consts["w1t_k"], consts["w2t_km"]
                w3t_k = consts["w3t_k"]
                b1_t, b2_t, b3_t = consts["b1_t"], consts["b2_t"], consts["b3_t"]
                if cfg.get("skip_mlp"):
                    continue
                # bag sums (feature-major) PSUM -> SBUF on ACT
                sT = []
                for bg in range(4):
                    t = sT_pool.tile([D, P], bf, tag="sT", name=f"sT{w}_{bg}")
                    if bg % 2 == 0:
                        nc.scalar.activation(t[:], s_ps[bg][:], AF.Copy)
                    else:
                        nc.vector.tensor_copy(t[:], s_ps[bg][:])
                    sT.append(t)

                l1 = []
                for ka, kb in ((0, 1), (2, 3)):
                    pc = m_psum.tile([P, P], f32, tag="mp")
                    nc.tensor.matmul(
                        pc[:], lhsT=w1t_k[0][:], rhs=sT[ka][:], start=True, stop=False
                    )
                    nc.tensor.matmul(
                        pc[:], lhsT=w1t_k[1][:], rhs=sT[kb][:], start=False, stop=True
                    )
                    xt = act_pool.tile([D, P], bf, tag="l1")
                    nc.scalar.activation(xt[:], pc[:], AF.Identity, bias=b1_t[:])
                    l1.append(xt)

                hT = []
                for mm in range(2):
                    ph = m_psum.tile([P, P], f32, tag="mp")
                    nc.tensor.matmul(
                        ph[:], lhsT=w2t_km[(0, mm)][:], rhs=l1[0][:],
                        start=True, stop=False,
                    )
                    nc.tensor.matmul(
                        ph[:], lhsT=w2t_km[(1, mm)][:], rhs=l1[1][:],
                        start=False, stop=True,
                    )
                    ht = act_pool.tile([D, P], bf, tag="l2")
                    nc.scalar.activation(ht[:], ph[:], AF.Relu, bias=b2_t[mm][:])
                    hT.append(ht)

                ob = out_pool.tile([P, med], bf, tag="osb")
                for h_i in range(2):
                    n0, n1 = h_i * n_half, (h_i + 1) * n_half
                    po = o_psum.tile([P, n_half], f32, tag="op", name="po")
                    nc.tensor.matmul(
                        po[:], lhsT=ones[:1, :], rhs=b3_t[:1, n0:n1],
                        start=True, stop=False,
                    )
                    nc.tensor.matmul(
                        po[:], lhsT=hT[0][:], rhs=w3t_k[0][:, n0:n1],
                        start=False, stop=False,
                    )
                    nc.tensor.matmul(
                        po[:], lhsT=hT[1][:], rhs=w3t_k[1][:, n0:n1],
                        start=False, stop=True,
                    )
                    nc.scalar.activation(ob[:, n0:n1], po[:], AF.Sigmoid)
                nc.scalar.dma_start(out[w * P : (w + 1) * P, :], ob[:])

    nc.compile()
    return nc


def kernel(**inputs) -> np.ndarray:
    from concourse.bass_utils import run_bass_kernel_spmd

    in_maps, cfg = host_prep(inputs)
    nc = build_nc(cfg)
    res = run_bass_kernel_spmd(nc, in_maps, core_ids=list(range(N_CORES)))
    return assemble(res.results, cfg)


# revision 43
# speedup vs baseline: 1.0256x; 1.0027x over previous
"""Trainium2 Bass kernel: 4x EmbeddingBag(sum over 32 codes) + 3-layer MLP.

Data-parallel over 8 NeuronCores (batch 16384 -> 8 x 2048).  Embedding tables
are concatenated (proc offset by +100000), cast to bf16 and split into 5
chunks of <=32000 rows (so per-chunk row indices fit int16 for dma_gather),
each chunk followed by one zero row used as gather padding.  Table rows are
assigned to chunks by a quota-balancing greedy (plus a repair pass) so that
every (core, window, bag) cell's per-chunk lookup counts stay under rotated
multiples-of-128 quotas - minimizing the ceil-128 padding below.

Per core the 262144 lookups (4 bags x 2048 examples x 32 codes) are sorted by
(window of 128 examples, chunk, bag).  Each (win,ck,bag) segment is padded to
a multiple of 128 rows ("blocks") with zero-row fetches.  One dma_gather per
(win, ck) pulls all its blocks' rows (bf16, 256B each) into SBUF in
partition-fastest order.  Per block, the selection matrix E [128 rows x 128
examples] is generated ON-CHIP by the vector engine (tensor_scalar is_equal
of a constant iota row-tile against the block's per-row example ids "mids",
a per-window fp32 DMA), then one PE matmul per block accumulates the rows
into a per-(bag, win) PSUM tile [D=128, 128 examples] in fp32 - start=True on
the first block of each (win,bag), stop on the last.  Pad rows fetch the
chunk's zero row and carry a sentinel mid (no E column), so they add zero.

The MLP then runs per window in fp32: bag sums are already feature-major in
PSUM, copy to SBUF (ACT/DVE), layer1+layer2 feature-major (ACT applies
bias/relu on the PSUM->SBUF copy), layer3 uses the activations as lhsT to
emit example-major [128, 1000] directly (bias via a K=1 ones-row matmul PSUM
init), ACT sigmoid, output DMA'd bf16 on the ACT HWDGE queue (host converts
to fp32).  Weight preloads are emitted after window 0's gathers; the last two
windows' gather ops are split so the drain tail overlaps remaining gathers.

The Bass program structure is shared by all 8 cores (SPMD); per-op sizes are
the max over cores, deficit cores pad with zero-row gathers and sentinel mids.
"""

import numpy as np

B, L, D = 16384, 32, 128
DIAG_LEN, PROC_LEN, MED_LEN = 100000, 50000, 1000
N_CORES = 8
P = 128
CS = 32000          # chunk size (int16-addressable)
NCK = 5             # chunks
WIN = 128           # examples per window
SENT = 200.0        # mids sentinel (never equals iota 0..127; exact in bf16)
QUOTA = (7, 7, 7, 6, 6)  # per-(win,bag) chunk quotas in 128-blocks, rotated
PLACE = False       # example->window placement (didn't help; identity)


def _balance_chunks(rows_all, cell_all, v_cat, nwin):
    """Assign table rows to chunks so that per-(core,win,bag) chunk counts
    stay under rotated 128-multiple quotas (minimizing ceil-128 padding).

    rows_all/cell_all: per-lookup row id and cell id (c*nwin*4 + w*4 + b).
    Returns (asg [v_cat] chunk id, loc [v_cat] position within chunk).
    """
    n_cells = cell_all.max() + 1
    o = np.argsort(rows_all, kind="stable")
    rs, cells_s = rows_all[o], cell_all[o]
    row_start = np.searchsorted(rs, np.arange(v_cat + 1))
    cnts = np.diff(row_start)

    # per-(row, cell) occurrence counts, row-major
    key = rs * n_cells + cells_s
    ukey, uocc = np.unique(key, return_counts=True)
    urow = ukey // n_cells
    ucell = ukey % n_cells
    ustart = np.searchsorted(urow, np.arange(v_cat + 1))

    # quotas per (ck, cell): rotate QUOTA by (w*4+b) % NCK
    j = np.arange(n_cells) % (nwin * 4)
    q = np.array(QUOTA, np.int64) * P
    Q = np.empty((NCK, n_cells), np.int64)
    for ck in range(NCK):
        Q[ck] = q[(ck + j) % NCK]

    wb = j  # cell -> (w*4+b) group id
    n_grp = nwin * 4
    row_order = np.argsort(-cnts, kind="stable")
    BS = 512

    def greedy_pack(Cap0):
        """One greedy packing run with initial per-(ck,cell) ceilings Cap0.
        Ceilings ratchet up by 128 when a (w,b,ck) group overflows (the
        extra block is paid once per group; later rows fill it free)."""
        L = np.zeros((NCK, n_cells), np.int64)
        cap = np.full(NCK, CS, np.int64)
        asg = np.full(v_cat, -1, np.int64)
        C = Cap0.copy()
        for i0 in range(0, v_cat, BS):
            br = row_order[i0 : i0 + BS]
            ent_s = ustart[br]
            ent_e = ustart[br + 1]
            ent_n = ent_e - ent_s
            flat = np.concatenate(
                [np.arange(s, e) for s, e in zip(ent_s, ent_e)]
            ) if ent_n.sum() else np.empty(0, np.int64)
            bounds = np.concatenate([[0], np.cumsum(ent_n)])
            bcell = ucell[flat]
            bocc = uocc[flat]
            nb = br.size
            slack = np.full((NCK, nb), 1 << 30, np.int64)
            has = ent_n > 0
            red_idx = bounds[:-1][has]
            for ck in range(NCK):
                cs_ = C[ck, bcell] - L[ck, bcell] - bocc
                if red_idx.size:
                    slack[ck, has] = np.minimum.reduceat(cs_, red_idx)
                slack[ck, ~has] = 1 << 30
                if cap[ck] <= 0:
                    slack[ck, :] = -(1 << 30)
            choice = np.argmax(slack, axis=0)
            asg[br] = choice
            for ck in range(NCK):
                sel = choice == ck
                cap[ck] -= int(sel.sum())
                csel = np.repeat(sel, ent_n)
                np.add.at(L[ck], bcell[csel], bocc[csel])
                gmax = np.zeros(n_grp, np.int64)
                np.maximum.at(gmax, wb, L[ck])
                gceil = -(-gmax // P) * P
                C[ck] = np.maximum(Cap0[ck], gceil[wb])
        return asg, L, cap, C

    def total_blocks(L):
        t = 0
        for ck in range(NCK):
            gmax = np.zeros(n_grp, np.int64)
            np.maximum.at(gmax, wb, L[ck])
            t += int((-(-gmax // P)).sum())
        return t

    # iterate: re-pack from scratch with ceilings tightened by 128 on groups
    # that overflowed their quota in the best run so far; keep the best
    asg, L, cap, C = greedy_pack(Q)
    best = (total_blocks(L), asg, L, cap, C)
    for _ in range(3):
        _, asg_b, L_b = best[0], best[1], best[2]
        Ct = Q.copy()
        for ck in range(NCK):
            gmax = np.zeros(n_grp, np.int64)
            np.maximum.at(gmax, wb, L_b[ck])
            gceil = -(-gmax // P) * P
            tgt = np.maximum(Q[ck, :], (gceil - P)[wb])
            Ct[ck] = np.minimum(np.maximum(Q[ck], gceil[wb]), tgt + P)
            Ct[ck] = np.maximum(Q[ck], tgt)
        asg, L, cap, C = greedy_pack(Ct)
        tb = total_blocks(L)
        if tb < best[0]:
            best = (tb, asg, L, cap, C)
        else:
            break
    _, asg, L, cap, C = best

    # repair pass: groups (w,b,ck) barely over a 128 boundary -> move rows
    # contributing to the over-boundary cores into chunks with slack
    cell_rows_order = np.argsort(ucell, kind="stable")
    cell_start = np.searchsorted(ucell[cell_rows_order], np.arange(n_cells + 1))
    for _ in range(2):
        gmaxs = np.zeros((NCK, n_grp), np.int64)
        for ck in range(NCK):
            np.maximum.at(gmaxs[ck], wb, L[ck])
        over = gmaxs % P
        order = np.argsort(np.where(over > 0, over, 1 << 30).reshape(-1))
        moved = 0
        for flatg in order:
            ck, g = divmod(int(flatg), n_grp)
            exc = int(over[ck, g])
            if exc == 0 or exc > 48:
                break
            floor_l = gmaxs[ck, g] - exc
            # offending cells of this group
            gcells = np.nonzero(wb == g)[0]
            bad = gcells[L[ck, gcells] > floor_l]
            for cell in bad:
                need = int(L[ck, cell] - floor_l)
                ent = cell_rows_order[cell_start[cell] : cell_start[cell + 1]]
                cand = ent[asg[urow[ent]] == ck]
                # smallest contributors first
                cand = cand[np.argsort(uocc[cand], kind="stable")]
                for e in cand:
                    if need <= 0:
                        break
                    r = int(urow[e])
                    es, ee = int(ustart[r]), int(ustart[r + 1])
                    rc, ro = ucell[es:ee], uocc[es:ee]
                    for ck2 in range(NCK):
                        if ck2 == ck or cap[ck2] <= 0:
                            continue
                        if np.all(C[ck2, rc] - L[ck2, rc] >= ro):
                            asg[r] = ck2
                            L[ck, rc] -= ro
                            L[ck2, rc] += ro
                            cap[ck] += 1
                            cap[ck2] -= 1
                            need -= int(ro[np.nonzero(rc == cell)[0][0]])
                            moved += 1
                            break
        if moved == 0:
            break

    # positions within chunks (original row order)
    loc = np.zeros(v_cat, np.int64)
    for ck in range(NCK):
        sel = np.nonzero(asg == ck)[0]
        loc[sel] = np.arange(sel.size)
    return asg, loc


def _structure(counts):
    """Static program structure from per-core segment counts.

    counts: [n_cores, NWIN, NCK, 4] lookup counts per (win, ck, bag) segment.
    """
    n_cores, NWIN, NCK, NB_ = counts.shape
    cmax = counts.max(axis=0)  # [NWIN, NCK, 4]
    nb = -(-cmax // P)  # ceil -> blocks per segment
    nb[:, 0, :][nb[:, 0, :] == 0] = 1  # ck0 segments host the start=True matmul
    ops = []
    idx_off = 0
    blk_off = 0
    for w in range(NWIN):
        win_blocks = {bg: [] for bg in range(4)}
        win_ops = []
        for ck in range(NCK):
            op_blocks = []
            for bg in range(4):
                for b in range(int(nb[w, ck, bg])):
                    blk = [bg, False, False]
                    op_blocks.append(blk)
                    win_blocks[bg].append(blk)
            # split the last windows' ops so tail compute overlaps remaining
            # gathers; the final window splits at the bag0+1/bag2+3 boundary
            # so half the MLP inputs complete one sub-op early
            if w == NWIN - 1:
                c01 = int(nb[w, ck, 0] + nb[w, ck, 1])
                c2 = c01 + int(nb[w, ck, 2])
                pieces = [op_blocks[:c01], op_blocks[c01:c2], op_blocks[c2:]]
            elif w == NWIN - 2:
                per = -(-len(op_blocks) // 2)
                pieces = [op_blocks[:per], op_blocks[per:]]
            elif w == 0 and ck == 0:
                # tiny head op so the first gather starts ASAP
                pieces = [op_blocks[:4], op_blocks[4:]]
            else:
                pieces = [op_blocks]
            for pb in pieces:
                if not pb:
                    continue
                n_op = len(pb) * P
                win_ops.append(
                    dict(win=w, ck=ck, idx_off=idx_off, blk_off=blk_off,
                         nb=len(pb), n=n_op, blocks=pb)
                )
                idx_off += n_op
                blk_off += len(pb)
        for bg in range(4):
            assert win_blocks[bg], "every bag needs blocks in every window"
            win_blocks[bg][0][1] = True   # start
            win_blocks[bg][-1][2] = True  # stop
        ops.extend(win_ops)
    return dict(ops=ops, tot_idx=idx_off, tot_blk=blk_off, nb_arr=nb,
                NWIN=NWIN, NCK=NCK)


def host_prep(inputs, n_cores=N_CORES):
    import ml_dtypes

    bf16 = ml_dtypes.bfloat16

    diag = np.asarray(inputs["diag_emb"], np.float32)
    proc = np.asarray(inputs["proc_emb"], np.float32)
    v_diag, d = diag.shape
    tcat = np.concatenate([diag, proc], axis=0)
    v_cat = tcat.shape[0]
    assert NCK * CS >= v_cat

    gl = {
        "cd": np.asarray(inputs["diag_codes"], np.int64),
        "cp": np.asarray(inputs["proc_codes"], np.int64) + v_diag,
        "pd": np.asarray(inputs["prev_diag_codes"], np.int64),
        "pp": np.asarray(inputs["prev_proc_codes"], np.int64) + v_diag,
    }
    b_total, l_codes = gl["cd"].shape
    assert b_total % n_cores == 0
    bc = b_total // n_cores
    assert bc % WIN == 0
    NWIN = bc // WIN

    # per-core flat (row, example, bag) streams
    core_g, core_e, core_bag = [], [], []
    for c in range(n_cores):
        gs, bags = [], []
        for bi, name in enumerate(("cd", "cp", "pd", "pp")):
            g = gl[name][c * bc : (c + 1) * bc].reshape(-1)
            gs.append(g)
            bags.append(np.full(g.size, bi, np.int64))
        core_g.append(np.concatenate(gs))
        core_bag.append(np.concatenate(bags))
        core_e.append(np.tile(np.repeat(np.arange(bc, dtype=np.int64), l_codes), 4))

    # balance rows across chunks to minimize ceil-128 padding
    rows_all = np.concatenate(core_g)
    cell_all = np.concatenate(
        [
            c * (NWIN * 4) + (core_e[c] // WIN) * 4 + core_bag[c]
            for c in range(n_cores)
        ]
    )
    asg, lmap = _balance_chunks(rows_all, cell_all, v_cat, NWIN)

    # place examples into windows (core-local permutation) to flatten each
    # window's per-(bag, chunk) lookup counts, then re-balance rows with the
    # easier cell structure.  wmaps[c][e] = permuted example slot.
    wmaps = [np.arange(bc, dtype=np.int64) for _ in range(n_cores)]
    if PLACE:
        for c in range(n_cores):
            prof = np.zeros((bc, 4, NCK), np.int64)
            np.add.at(
                prof.reshape(-1),
                (core_e[c] * 4 + core_bag[c]) * NCK + asg[core_g[c]],
                1,
            )
            prof = prof.reshape(bc, 4 * NCK).astype(np.float64)
            target = prof.sum(axis=0) / bc  # per-example mean profile
            Wsum = np.zeros((NWIN, 4 * NCK), np.float64)
            n_w = np.zeros(NWIN, np.int64)
            # most extreme examples first
            eorder = np.argsort(-np.abs(prof - target).sum(axis=1), kind="stable")
            slot = np.empty(bc, np.int64)
            for e in eorder:
                dev = Wsum + prof[e] - target * (n_w + 1)[:, None]
                score = np.square(dev).sum(axis=1)
                score[n_w >= WIN] = np.inf
                w = int(np.argmin(score))
                slot[e] = w * WIN + n_w[w]
                Wsum[w] += prof[e]
                n_w[w] += 1
            wmaps[c] = slot
        cell_all = np.concatenate(
            [
                c * (NWIN * 4) + (wmaps[c][core_e[c]] // WIN) * 4 + core_bag[c]
                for c in range(n_cores)
            ]
        )
        asg, lmap = _balance_chunks(rows_all, cell_all, v_cat, NWIN)

    tbl_dev = np.zeros(((CS + 1) * NCK, d), bf16)
    tbl_dev[asg * (CS + 1) + lmap] = tcat.astype(bf16)

    # flat per-core lookup streams, sorted by (win, ck, bag)
    per_core = []
    counts = np.zeros((n_cores, NWIN, NCK, 4), np.int64)
    for c in range(n_cores):
        g, bag = core_g[c], core_bag[c]
        e = wmaps[c][core_e[c]]
        ck = asg[g]
        loc = lmap[g]
        win = e // WIN
        m = e % WIN
        seg = (win * NCK + ck) * 4 + bag
        order = np.argsort(seg, kind="stable")
        per_core.append((seg[order], loc[order], m[order]))
        np.add.at(counts[c].reshape(-1), seg, 1)

    st = _structure(counts)
    TOT_IDX, TOT_B = st["tot_idx"], st["tot_blk"]

    # static per-segment offsets
    seg_sizes = st["nb_arr"].reshape(-1) * P
    seg_off = np.concatenate([[0], np.cumsum(seg_sizes)])[:-1]

    in_maps = []
    iota_np = np.broadcast_to(
        np.arange(P, dtype=np.float32), (P, P)
    ).astype(bf16).copy()
    e16_np = (np.arange(P)[None, :] % 16 == np.arange(16)[:, None]).astype(
        np.float32
    )
    for c in range(n_cores):
        seg_s, loc_s, m_s = per_core[c]
        pos_in_seg = np.arange(seg_s.size) - np.concatenate(
            [[0], np.cumsum(np.bincount(seg_s, minlength=seg_sizes.size))]
        )[:-1][seg_s]
        pos = seg_off[seg_s] + pos_in_seg
        idx_flat = np.full(TOT_IDX, CS, np.int16)  # pad -> zero row
        idx_flat[pos] = loc_s.astype(np.int16)
        m_flat = np.full(TOT_IDX, SENT, np.float32)
        m_flat[pos] = m_s
        # pack gidx: position i -> [i%16, i//16], fp32 (replicated to 128
        # partitions on-chip via a PE selection matmul)
        gidx = np.ascontiguousarray(
            idx_flat.reshape(TOT_IDX // 16, 16).T
        ).astype(np.float32)
        # mids: position i -> [i%128, i//128], bf16
        mids = np.ascontiguousarray(m_flat.reshape(TOT_B, P).T)
        in_maps.append(dict(tbl=tbl_dev, gidx=gidx, mids=mids, iota=iota_np,
                            e16=e16_np))

    w1t = np.ascontiguousarray(np.asarray(inputs["W1"], np.float32).T).astype(bf16)
    w2t = np.ascontiguousarray(np.asarray(inputs["W2"], np.float32).T).astype(bf16)
    w3t = np.ascontiguousarray(np.asarray(inputs["W3"], np.float32).T).astype(bf16)
    b1 = np.ascontiguousarray(np.asarray(inputs["b1"], np.float32).reshape(-1, 1))
    b2 = np.ascontiguousarray(np.asarray(inputs["b2"], np.float32).reshape(-1, 1))
    b3 = np.ascontiguousarray(np.asarray(inputs["b3"], np.float32).reshape(1, -1).astype(bf16))
    for im in in_maps:
        im.update(w1t=w1t, w2t=w2t, w3t=w3t, b1=b1, b2=b2, b3=b3)

    med = w3t.shape[1]
    cfg = dict(b_core=bc, med=med, v_dev=tbl_dev.shape[0], st=st, wmaps=wmaps)
    return in_maps, cfg


def assemble(results, cfg):
    """Concatenate per-core outputs, undoing the example->window placement
    permutation, and convert to fp32."""
    outs = []
    for c, r in enumerate(results):
        o = r["out"].astype(np.float32)
        outs.append(o[cfg["wmaps"][c]])
    return np.concatenate(outs, axis=0)


def build_nc(cfg):
    import concourse.bass as bass
    import concourse.mybir as mybir
    import concourse.tile as tile
    from concourse import bacc

    f32 = mybir.dt.float32
    bf = mybir.dt.bfloat16
    i16 = mybir.dt.int16
    AF = mybir.ActivationFunctionType
    EQ = mybir.AluOpType.is_equal

    bc, med, v_dev = cfg["b_core"], cfg["med"], cfg["v_dev"]
    st = cfg["st"]
    NWIN, NCK = st["NWIN"], st["NCK"]
    TOT_IDX, TOT_B = st["tot_idx"], st["tot_blk"]
    n_half = med // 2
    assert n_half <= 512

    nc = bacc.Bacc("TRN2", target_bir_lowering=False, debug=False,
                   enable_asserts=False, num_devices=N_CORES)

    tbl = nc.dram_tensor("tbl", [v_dev, D], bf, kind="ExternalInput").ap()
    gidx = nc.dram_tensor("gidx", [16, TOT_IDX // 16], f32, kind="ExternalInput").ap()
    e16 = nc.dram_tensor("e16", [16, P], f32, kind="ExternalInput").ap()
    mids = nc.dram_tensor("mids", [P, TOT_B], f32, kind="ExternalInput").ap()
    iota = nc.dram_tensor("iota", [P, P], bf, kind="ExternalInput").ap()
    w1t = nc.dram_tensor("w1t", [2 * D, D], bf, kind="ExternalInput").ap()
    w2t = nc.dram_tensor("w2t", [2 * D, 2 * D], bf, kind="ExternalInput").ap()
    w3t = nc.dram_tensor("w3t", [2 * D, med], bf, kind="ExternalInput").ap()
    b1 = nc.dram_tensor("b1", [D, 1], f32, kind="ExternalInput").ap()
    b2 = nc.dram_tensor("b2", [2 * D, 1], f32, kind="ExternalInput").ap()
    b3 = nc.dram_tensor("b3", [1, med], bf, kind="ExternalInput").ap()
    out = nc.dram_tensor("out", [bc, med], bf, kind="ExternalOutput").ap()

    ops_by_win = {}
    for op in st["ops"]:
        ops_by_win.setdefault(op["win"], []).append(op)

    with tile.TileContext(nc) as tc:
        with (
            tc.tile_pool(name="const", bufs=1) as cpool,
            tc.tile_pool(name="gi", bufs=4) as gi_pool,
            tc.tile_pool(name="gst", bufs=3) as gst_pool,
            tc.tile_pool(name="mi", bufs=4) as mi_pool,
            tc.tile_pool(name="em", bufs=8) as em_pool,
            tc.tile_pool(name="gath", bufs=8) as gath_pool,
            tc.tile_pool(name="sT", bufs=8) as sT_pool,
            tc.tile_pool(name="acts", bufs=8) as act_pool,
            tc.tile_pool(name="osb", bufs=2) as out_pool,
            tc.tile_pool(name="spsum", bufs=4, space="PSUM") as s_psum,
            tc.tile_pool(name="mpsum", bufs=2, space="PSUM") as m_psum,
            tc.tile_pool(name="opsum", bufs=2, space="PSUM") as o_psum,
        ):
            iota_t = cpool.tile([P, P], bf, tag="iota")
            iota_loaded = [False]
            e16_t = cpool.tile([16, P], f32, tag="e16")
            nc.sync.dma_start(e16_t[:], e16[:, :])

            consts = {}

            def load_consts():
                # Emitted after window 0's gather ops so the first gathers
                # aren't queued behind ~1.3MB of weight preloads.
                ones = cpool.tile([1, P], bf, tag="ones")
                nc.gpsimd.memset(ones[:], 1.0)
                w1t_k = []
                for k in range(2):
                    t = cpool.tile([D, D], bf, tag=f"w1t{k}")
                    nc.sync.dma_start(t[:], w1t[k * D : (k + 1) * D, :])
                    w1t_k.append(t)
                w2t_km = {}
                for k in range(2):
                    for mm in range(2):
                        t = cpool.tile([D, D], bf, tag=f"w2t{k}{mm}")
                        nc.sync.dma_start(
                            t[:], w2t[k * D : (k + 1) * D, mm * D : (mm + 1) * D]
                        )
                        w2t_km[(k, mm)] = t
                w3t_k = []
                for k in range(2):
                    t = cpool.tile([D, med], bf, tag=f"w3t{k}")
                    nc.sync.dma_start(t[:], w3t[k * D : (k + 1) * D, :])
                    w3t_k.append(t)
                b1_t = cpool.tile([D, 1], f32, tag="b1")
                nc.sync.dma_start(b1_t[:], b1[:, :])
                b2_t = []
                for mm in range(2):
                    t = cpool.tile([D, 1], f32, tag=f"b2{mm}")
                    nc.sync.dma_start(t[:], b2[mm * D : (mm + 1) * D, :])
                    b2_t.append(t)
                b3_t = cpool.tile([1, med], bf, tag="b3")
                nc.sync.dma_start(b3_t[:], b3[:, :])
                consts.update(ones=ones, w1t_k=w1t_k, w2t_km=w2t_km,
                              w3t_k=w3t_k, b1_t=b1_t, b2_t=b2_t, b3_t=b3_t)

            for rep in range(cfg.get("reps", 1)):
              win_feed = {}

              def produce_feed(w):
                # stage the window's fp32 gidx [16, cols], replicate to 128
                # partitions on the PE (out[m,n] = gst[m%16,n]) and cast
                # fp32->int16 on DVE.  Called 2 windows ahead of use so the
                # in-order PE stream keeps the gather feed ahead of the
                # E-matmuls.
                wops = ops_by_win[w]
                w_idx_off = wops[0]["idx_off"]
                w_blk_off = wops[0]["blk_off"]
                w_n = sum(op["n"] for op in wops)
                w_nb = sum(op["nb"] for op in wops)
                cols = w_n // 16
                gst = gst_pool.tile([16, cols], f32, tag="gst")
                if not cfg.get("skip_gi"):
                    nc.sync.dma_start(
                        gst[:],
                        gidx[:, w_idx_off // 16 : (w_idx_off + w_n) // 16],
                    )
                gi = gi_pool.tile([P, cols], i16, tag="gi")
                # small first piece on window 0 so the head gather starts ASAP
                cuts = [0, 64] if w == 0 else [0]
                while cuts[-1] < cols:
                    cuts.append(min(cuts[-1] + 512, cols))
                for s, e in zip(cuts, cuts[1:]):
                    nn = e - s
                    rp = o_psum.tile([P, 512], f32, tag="op", name="rp")
                    nc.tensor.matmul(
                        rp[:, :nn], lhsT=e16_t[:], rhs=gst[:, s : s + nn],
                        start=True, stop=True,
                    )
                    nc.vector.tensor_copy(gi[:, s : s + nn], rp[:, :nn])
                mi = mi_pool.tile([P, w_nb], f32, tag="mi")
                nc.sync.dma_start(
                    mi[:], mids[:, w_blk_off : w_blk_off + w_nb]
                )
                if not iota_loaded[0]:
                    nc.sync.dma_start(iota_t[:], iota[:, :])
                    iota_loaded[0] = True
                win_feed[w] = (gi, mi, w_idx_off, w_blk_off)

              produce_feed(0)
              produce_feed(1)
              for w in range(NWIN):
                s_ps = [s_psum.tile([D, WIN], f32, tag="s", name=f"s{rep}_{w}_{i}") for i in range(4)]
                if w + 2 < NWIN:
                    produce_feed(w + 2)
                gi, mi, w_idx_off, w_blk_off = win_feed.pop(w)
                wops = ops_by_win[w]
                for op in wops:
                    n, nb = op["n"], op["nb"]
                    o16 = (op["idx_off"] - w_idx_off) // 16
                    ob0 = op["blk_off"] - w_blk_off
                    gt = gath_pool.tile([P, nb * D], bf, tag="gath")
                    if not cfg.get("skip_gather"):
                        nc.gpsimd.dma_gather(
                            out_ap=gt[:].rearrange("p (c d) -> p c d", d=D),
                            in_ap=tbl[
                                op["ck"] * (CS + 1) : (op["ck"] + 1) * (CS + 1), :
                            ],
                            idxs_ap=gi[:, o16 : o16 + n // 16],
                            num_idxs=n,
                            num_idxs_reg=n,
                            elem_size=D,
                            single_packet=False,
                        )
                    gt3 = gt[:].rearrange("p (c d) -> p c d", d=D)
                    em = em_pool.tile([P, P * nb], bf, tag="em")
                    for b in range(nb):
                        nc.vector.tensor_scalar(
                            em[:, b * P : (b + 1) * P],
                            iota_t[:],
                            mi[:, ob0 + b : ob0 + b + 1],
                            None,
                            EQ,
                        )
                    if cfg.get("skip_smm"):
                        continue
                    for b, (bg, start, stop) in enumerate(op["blocks"]):
                        nc.tensor.matmul(
                            s_ps[bg][:],
                            lhsT=gt3[:, b, :],
                            rhs=em[:, b * P : (b + 1) * P],
                            start=start,
                            stop=stop,
                            skip_group_check=True,
                        )
                if not consts:
                    load_consts()
                ones = consts["ones"]
                w1t_k, w2t_km = consts["w1t_k"], consts["w2t_km"]
                w3t_k = consts["w3t_k"]
                b1_t, b2_t, b3_t = consts["b1_t"], consts["b2_t"], consts["b3_t"]
                if cfg.get("skip_mlp"):
                    continue
                # bag sums (feature-major) PSUM -> SBUF on ACT
                sT = []
                for bg in range(4):
                    t = sT_pool.tile([D, P], bf, tag="sT", name=f"sT{w}_{bg}")
                    if bg % 2 == 0:
                        nc.scalar.activation(t[:], s_ps[bg][:], AF.Copy)
                    else:
                        nc.vector.tensor_copy(t[:], s_ps[bg][:])
                    sT.append(t)

                l1 = []
                for ka, kb in ((0, 1), (2, 3)):
                    pc = m_psum.tile([P, P], f32, tag="mp")
                    nc.tensor.matmul(
                        pc[:], lhsT=w1t_k[0][:], rhs=sT[ka][:], start=True, stop=False
                    )
                    nc.tensor.matmul(
                        pc[:], lhsT=w1t_k[1][:], rhs=sT[kb][:], start=False, stop=True
                    )
                    xt = act_pool.tile([D, P], bf, tag="l1")
                    nc.scalar.activation(xt[:], pc[:], AF.Identity, bias=b1_t[:])
                    l1.append(xt)

                hT = []
                for mm in range(2):
                    ph = m_psum.tile([P, P], f32, tag="mp")
                    nc.tensor.matmul(
                        ph[:], lhsT=w2t_km[(0, mm)][:], rhs=l1[0][:],
                        start=True, stop=False,
                    )
                    nc.tensor.matmul(
                        ph[:], lhsT=w2t_km[(1, mm)][:], rhs=l1[1][:],
                        start=False, stop=True,
                    )
                    ht = act_pool.tile([D, P], bf, tag="l2")
                    nc.scalar.activation(ht[:], ph[:], AF.Relu, bias=b2_t[mm][:])
                    hT.append(ht)

                ob = out_pool.tile([P, med], bf, tag="osb")
                for h_i in range(2):
                    n0, n1 = h_i * n_half, (h_i + 1) * n_half
                    po = o_psum.tile([P, n_half], f32, tag="op", name="po")
                    nc.tensor.matmul(
                        po[:], lhsT=ones[:1, :], rhs=b3_t[:1, n0:n1],
                        start=True, stop=False,
                    )
                    nc.tensor.matmul(
                        po[:], lhsT=hT[0][:], rhs=w3t_k[0][:, n0:n1],
                        start=False, stop=False,
                    )
                    nc.tensor.matmul(
                        po[:], lhsT=hT[1][:], rhs=w3t_k[1][:, n0:n1],
                        start=False, stop=True,
                    )
                    nc.scalar.activation(ob[:, n0:n1], po[:], AF.Sigmoid)
                nc.scalar.dma_start(out[w * P : (w + 1) * P, :], ob[:])

    nc.compile()
    return nc


def kernel(**inputs) -> np.ndarray:
    from concourse.bass_utils import run_bass_kernel_spmd

    in_maps, cfg = host_prep(inputs)
    nc = build_nc(cfg)
    res = run_bass_kernel_spmd(nc, in_maps, core_ids=list(range(N_CORES)))
    return assemble(res.results, cfg)


# revision 50
# speedup vs baseline: 1.1488x; 1.1202x over previous
"""Trainium2 Bass kernel: 4x EmbeddingBag(sum over 32 codes) + 3-layer MLP.

Data-parallel over 8 NeuronCores (batch 16384 -> 8 x 2048).  Embedding tables
are concatenated (proc offset by +100000), cast to bf16 and split into 5
chunks of <=32000 rows (so per-chunk row indices fit int16 for dma_gather),
each chunk followed by one zero row used as gather padding.  Table rows are
assigned to chunks by a quota-balancing greedy (plus a repair pass) so that
every (core, window, bag) cell's per-chunk lookup counts stay under rotated
multiples-of-128 quotas - minimizing the ceil-128 padding below.

Per core the 262144 lookups (4 bags x 2048 examples x 32 codes) are sorted by
(window of 128 examples, chunk, bag).  Each (win,ck,bag) segment is padded to
a multiple of 128 rows ("blocks") with zero-row fetches.  One dma_gather per
(win, ck) pulls all its blocks' rows (bf16, 256B each) into SBUF in
partition-fastest order.  Per block, the selection matrix E [128 rows x 128
examples] is generated ON-CHIP by the vector engine (tensor_scalar is_equal
of a constant iota row-tile against the block's per-row example ids "mids",
a per-window fp32 DMA), then one PE matmul per block accumulates the rows
into a per-(bag, win) PSUM tile [D=128, 128 examples] in fp32 - start=True on
the first block of each (win,bag), stop on the last.  Pad rows fetch the
chunk's zero row and carry a sentinel mid (no E column), so they add zero.

The gather's int16 index stream must be laid out [128, n/16] with the same
16-partition block replicated 8x for the Q7 cores; instead of DMAing the
replicated 4.4MB, it is shipped once as fp32 [16, n/16] and replicated
on-chip: a PE matmul with a constant 0/1 selection matrix (out[m,n] =
gidx[m%16,n]) into PSUM, cast fp32->int16 by DVE.  Each window's feed is
produced two windows ahead of use so the in-order PE stream keeps the gather
feed ahead of the E-matmuls.

The MLP then runs per window with bf16 weights/activations (fp32 PSUM
accumulation): bag sums are feature-major in PSUM, copy to SBUF bf16
(ACT/DVE), layer1+layer2 feature-major (ACT applies bias/relu on the
PSUM->SBUF copy), layer3 uses the activations as lhsT to emit example-major
[128, 1000] directly (bias via a K=1 ones-row matmul PSUM init), ACT sigmoid,
output DMA'd bf16 on the ACT HWDGE queue (host converts to fp32).  Weight
preloads are emitted after window 0's gathers; the last two windows' gather
ops are split (the final window per bag-pair) so the drain tail overlaps
remaining gathers.

The Bass program structure is shared by all 8 cores (SPMD); per-op sizes are
the max over cores, deficit cores pad with zero-row gathers and sentinel mids.
"""

import numpy as np

B, L, D = 16384, 32, 128
DIAG_LEN, PROC_LEN, MED_LEN = 100000, 50000, 1000
N_CORES = 8
P = 128
SLOTS = 30000       # 512B pair-slots per chunk (int16-addressable)
NCK = 3             # chunks
WIN = 128           # examples per window
SENT = 200.0        # mids sentinel (never equals iota 0..127; exact in bf16)
QUOTA = (7, 7, 7, 6, 6)  # per-(win,bag) chunk quotas in 128-blocks, rotated
PLACE = False       # example->window placement (didn't help; identity)


def _balance_chunks(rows_all, cell_all, v_cat, nwin):
    """Assign table rows to chunks so that per-(core,win,bag) chunk counts
    stay under rotated 128-multiple quotas (minimizing ceil-128 padding).

    rows_all/cell_all: per-lookup row id and cell id (c*nwin*4 + w*4 + b).
    Returns (asg [v_cat] chunk id, loc [v_cat] position within chunk).
    """
    n_cells = cell_all.max() + 1
    o = np.argsort(rows_all, kind="stable")
    rs, cells_s = rows_all[o], cell_all[o]
    row_start = np.searchsorted(rs, np.arange(v_cat + 1))
    cnts = np.diff(row_start)

    # per-(row, cell) occurrence counts, row-major
    key = rs * n_cells + cells_s
    ukey, uocc = np.unique(key, return_counts=True)
    urow = ukey // n_cells
    ucell = ukey % n_cells
    ustart = np.searchsorted(urow, np.arange(v_cat + 1))

    # quotas per (ck, cell): rotate QUOTA by (w*4+b) % NCK
    j = np.arange(n_cells) % (nwin * 4)
    q = np.array(QUOTA, np.int64) * P
    Q = np.empty((NCK, n_cells), np.int64)
    for ck in range(NCK):
        Q[ck] = q[(ck + j) % NCK]

    wb = j  # cell -> (w*4+b) group id
    n_grp = nwin * 4
    row_order = np.argsort(-cnts, kind="stable")
    BS = 512

    def greedy_pack(Cap0):
        """One greedy packing run with initial per-(ck,cell) ceilings Cap0.
        Ceilings ratchet up by 128 when a (w,b,ck) group overflows (the
        extra block is paid once per group; later rows fill it free)."""
        L = np.zeros((NCK, n_cells), np.int64)
        cap = np.full(NCK, CS, np.int64)
        asg = np.full(v_cat, -1, np.int64)
        C = Cap0.copy()
        for i0 in range(0, v_cat, BS):
            br = row_order[i0 : i0 + BS]
            ent_s = ustart[br]
            ent_e = ustart[br + 1]
            ent_n = ent_e - ent_s
            flat = np.concatenate(
                [np.arange(s, e) for s, e in zip(ent_s, ent_e)]
            ) if ent_n.sum() else np.empty(0, np.int64)
            bounds = np.concatenate([[0], np.cumsum(ent_n)])
            bcell = ucell[flat]
            bocc = uocc[flat]
            nb = br.size
            slack = np.full((NCK, nb), 1 << 30, np.int64)
            has = ent_n > 0
            red_idx = bounds[:-1][has]
            for ck in range(NCK):
                cs_ = C[ck, bcell] - L[ck, bcell] - bocc
                if red_idx.size:
                    slack[ck, has] = np.minimum.reduceat(cs_, red_idx)
                slack[ck, ~has] = 1 << 30
                if cap[ck] <= 0:
                    slack[ck, :] = -(1 << 30)
            choice = np.argmax(slack, axis=0)
            asg[br] = choice
            for ck in range(NCK):
                sel = choice == ck
                cap[ck] -= int(sel.sum())
                csel = np.repeat(sel, ent_n)
                np.add.at(L[ck], bcell[csel], bocc[csel])
                gmax = np.zeros(n_grp, np.int64)
                np.maximum.at(gmax, wb, L[ck])
                gceil = -(-gmax // P) * P
                C[ck] = np.maximum(Cap0[ck], gceil[wb])
        return asg, L, cap, C

    def total_blocks(L):
        t = 0
        for ck in range(NCK):
            gmax = np.zeros(n_grp, np.int64)
            np.maximum.at(gmax, wb, L[ck])
            t += int((-(-gmax // P)).sum())
        return t

    # iterate: re-pack from scratch with ceilings tightened by 128 on groups
    # that overflowed their quota in the best run so far; keep the best
    asg, L, cap, C = greedy_pack(Q)
    best = (total_blocks(L), asg, L, cap, C)
    for _ in range(3):
        _, asg_b, L_b = best[0], best[1], best[2]
        Ct = Q.copy()
        for ck in range(NCK):
            gmax = np.zeros(n_grp, np.int64)
            np.maximum.at(gmax, wb, L_b[ck])
            gceil = -(-gmax // P) * P
            tgt = np.maximum(Q[ck, :], (gceil - P)[wb])
            Ct[ck] = np.minimum(np.maximum(Q[ck], gceil[wb]), tgt + P)
            Ct[ck] = np.maximum(Q[ck], tgt)
        asg, L, cap, C = greedy_pack(Ct)
        tb = total_blocks(L)
        if tb < best[0]:
            best = (tb, asg, L, cap, C)
        else:
            break
    _, asg, L, cap, C = best

    # repair pass: groups (w,b,ck) barely over a 128 boundary -> move rows
    # contributing to the over-boundary cores into chunks with slack
    cell_rows_order = np.argsort(ucell, kind="stable")
    cell_start = np.searchsorted(ucell[cell_rows_order], np.arange(n_cells + 1))
    for _ in range(2):
        gmaxs = np.zeros((NCK, n_grp), np.int64)
        for ck in range(NCK):
            np.maximum.at(gmaxs[ck], wb, L[ck])
        over = gmaxs % P
        order = np.argsort(np.where(over > 0, over, 1 << 30).reshape(-1))
        moved = 0
        for flatg in order:
            ck, g = divmod(int(flatg), n_grp)
            exc = int(over[ck, g])
            if exc == 0 or exc > 48:
                break
            floor_l = gmaxs[ck, g] - exc
            # offending cells of this group
            gcells = np.nonzero(wb == g)[0]
            bad = gcells[L[ck, gcells] > floor_l]
            for cell in bad:
                need = int(L[ck, cell] - floor_l)
                ent = cell_rows_order[cell_start[cell] : cell_start[cell + 1]]
                cand = ent[asg[urow[ent]] == ck]
                # smallest contributors first
                cand = cand[np.argsort(uocc[cand], kind="stable")]
                for e in cand:
                    if need <= 0:
                        break
                    r = int(urow[e])
                    es, ee = int(ustart[r]), int(ustart[r + 1])
                    rc, ro = ucell[es:ee], uocc[es:ee]
                    for ck2 in range(NCK):
                        if ck2 == ck or cap[ck2] <= 0:
                            continue
                        if np.all(C[ck2, rc] - L[ck2, rc] >= ro):
                            asg[r] = ck2
                            L[ck, rc] -= ro
                            L[ck2, rc] += ro
                            cap[ck] += 1
                            cap[ck2] -= 1
                            need -= int(ro[np.nonzero(rc == cell)[0][0]])
                            moved += 1
                            break
        if moved == 0:
            break

    # positions within chunks (original row order)
    loc = np.zeros(v_cat, np.int64)
    for ck in range(NCK):
        sel = np.nonzero(asg == ck)[0]
        loc[sel] = np.arange(sel.size)
    return asg, loc


def _structure(counts):
    """Static program structure from per-core segment counts.

    counts: [n_cores, NWIN, NCK, 4] lookup counts per (win, ck, bag) segment.
    """
    n_cores, NWIN, NCK, NB_ = counts.shape
    cmax = counts.max(axis=0)  # [NWIN, NCK, 4]
    nb = -(-cmax // P)  # ceil -> blocks per segment
    nb[:, 0, :][nb[:, 0, :] == 0] = 1  # ck0 segments host the start=True matmul
    ops = []
    idx_off = 0
    blk_off = 0
    for w in range(NWIN):
        win_blocks = {bg: [] for bg in range(4)}
        win_ops = []
        for ck in range(NCK):
            op_blocks = []
            for bg in range(4):
                for b in range(int(nb[w, ck, bg])):
                    blk = [bg, False, False]
                    op_blocks.append(blk)
                    win_blocks[bg].append(blk)
            # split the last windows' ops so tail compute overlaps remaining
            # gathers; the final window splits at the bag0+1/bag2+3 boundary
            # so half the MLP inputs complete one sub-op early
            CAP = 20  # blocks per gather op (SBUF: 512B/part per block x2)
            if w == 0 and ck == 0:
                cuts = [0, 4]
            else:
                cuts = [0]
            while cuts[-1] < len(op_blocks):
                cuts.append(min(cuts[-1] + CAP, len(op_blocks)))
            pieces = [op_blocks[a:b] for a, b in zip(cuts, cuts[1:])]
            for pb in pieces:
                if not pb:
                    continue
                n_op = len(pb) * P
                win_ops.append(
                    dict(win=w, ck=ck, idx_off=idx_off, blk_off=blk_off,
                         nb=len(pb), n=n_op, blocks=pb)
                )
                idx_off += n_op
                blk_off += len(pb)
        for bg in range(4):
            assert win_blocks[bg], "every bag needs blocks in every window"
            win_blocks[bg][0][1] = True   # start
            win_blocks[bg][-1][2] = True  # stop
        ops.extend(win_ops)
    return dict(ops=ops, tot_idx=idx_off, tot_blk=blk_off, nb_arr=nb,
                NWIN=NWIN, NCK=NCK)


def host_prep(inputs, n_cores=N_CORES):
    import ml_dtypes

    bf16 = ml_dtypes.bfloat16

    diag = np.asarray(inputs["diag_emb"], np.float32)
    proc = np.asarray(inputs["proc_emb"], np.float32)
    v_diag, d = diag.shape
    tcat = np.concatenate([diag, proc], axis=0).astype(bf16)
    v_cat = tcat.shape[0]

    gl = {
        "cd": np.asarray(inputs["diag_codes"], np.int64),
        "cp": np.asarray(inputs["proc_codes"], np.int64) + v_diag,
        "pd": np.asarray(inputs["prev_diag_codes"], np.int64),
        "pp": np.asarray(inputs["prev_proc_codes"], np.int64) + v_diag,
    }
    b_total, l_codes = gl["cd"].shape
    bc = b_total // n_cores
    NWIN = bc // WIN

    # per-core streams and walk-placement into 512B pair-slots
    core_desc = []   # per core: dict (w,ck,bag) -> list of (slot, mA, mB)
    core_cells = []  # per core: cell assignment per row (-1 unplaced)
    counts = np.zeros((n_cores, NWIN, NCK, 4), np.int64)
    for c in range(n_cores):
        cells = np.full(v_cat, -1, np.int64)
        nf = 0  # next free cell (even-aligned pair slots)
        descs = {}
        for w in range(NWIN):
            e0 = w * WIN
            for bag, name in enumerate(("cd", "cp", "pd", "pp")):
                g = gl[name][c * bc + e0 : c * bc + e0 + WIN].reshape(-1)
                m = np.repeat(np.arange(WIN, dtype=np.int64), l_codes)
                placed = cells[g] >= 0
                # new rows: first occurrence of a row in this segment wins;
                # later duplicates in the same segment count as placed-late
                newg, newm = [], []
                seen = set()
                pl_g, pl_m = [], []
                for gi_, mi_ in zip(g.tolist(), m.tolist()):
                    if cells[gi_] >= 0 or gi_ in seen:
                        pl_g.append(gi_); pl_m.append(mi_)
                    else:
                        seen.add(gi_); newg.append(gi_); newm.append(mi_)
                seg = {}
                # pair new rows two per slot
                for i in range(0, len(newg) - 1, 2):
                    r1, r2 = newg[i], newg[i + 1]
                    cells[r1] = nf; cells[r2] = nf + 1
                    seg[nf // 2] = [newm[i], newm[i + 1]]
                    nf += 2
                if len(newg) % 2:
                    r1 = newg[-1]
                    cells[r1] = nf
                    seg[nf // 2] = [newm[-1], SENT]
                    nf += 2
                # placed rows: group by slot, fill parity sides
                extra = []
                for gi_, mi_ in zip(pl_g, pl_m):
                    cell = cells[gi_]
                    slot, par = cell >> 1, cell & 1
                    ent = seg.get(slot)
                    if ent is not None and ent[par] == SENT:
                        ent[par] = mi_
                    else:
                        e2 = [SENT, SENT]
                        e2[par] = mi_
                        extra.append((slot, e2))
                items = [(s, v[0], v[1]) for s, v in seg.items()]
                items += [(s, v[0], v[1]) for s, v in extra]
                for slot, mA, mB in items:
                    ck = slot // SLOTS
                    descs.setdefault((w, ck, bag), []).append(
                        (slot - ck * SLOTS, mA, mB)
                    )
                    counts[c, w, ck, bag] += 1
        core_desc.append(descs)
        core_cells.append(cells)
        assert nf <= 2 * SLOTS * NCK

    st = _structure(counts)
    TOT_IDX, TOT_B = st["tot_idx"], st["tot_blk"]
    seg_sizes = st["nb_arr"].reshape(-1) * P
    seg_off = np.concatenate([[0], np.cumsum(seg_sizes)])[:-1]

    in_maps = []
    iota_np = np.broadcast_to(
        np.arange(P, dtype=np.float32), (P, P)
    ).astype(bf16).copy()
    e16_np = (np.arange(P)[None, :] % 16 == np.arange(16)[:, None]).astype(
        np.float32
    )
    for c in range(n_cores):
        # per-core table: rows at their cells, zero slot at SLOTS per chunk
        tbl_dev = np.zeros(((SLOTS + 1) * NCK, 2 * d), bf16)
        cells = core_cells[c]
        pl = np.nonzero(cells >= 0)[0]
        cell = cells[pl]
        slot = cell >> 1
        ck_of = slot // SLOTS
        dslot = ck_of * (SLOTS + 1) + (slot - ck_of * SLOTS)
        par = cell & 1
        flat = tbl_dev.reshape(-1, d)
        flat[2 * dslot + par] = tcat[pl]

        idx_flat = np.full(TOT_IDX, SLOTS, np.int16)
        mA_flat = np.full(TOT_IDX, SENT, np.float32)
        mB_flat = np.full(TOT_IDX, SENT, np.float32)
        for (w, ck, bag), lst in core_desc[c].items():
            off = seg_off[(w * NCK + ck) * 4 + bag]
            arr = np.asarray(lst, np.float64)
            n = arr.shape[0]
            idx_flat[off : off + n] = arr[:, 0].astype(np.int16)
            mA_flat[off : off + n] = arr[:, 1]
            mB_flat[off : off + n] = arr[:, 2]
        gidx = np.ascontiguousarray(
            idx_flat.reshape(TOT_IDX // 16, 16).T
        ).astype(np.float32)
        mids = np.empty((P, 2 * TOT_B), np.float32)
        mids[:, 0::2] = mA_flat.reshape(TOT_B, P).T
        mids[:, 1::2] = mB_flat.reshape(TOT_B, P).T
        in_maps.append(dict(tbl=tbl_dev, gidx=gidx, mids=mids, iota=iota_np,
                            e16=e16_np))

    w1t = np.asarray(inputs["W1"], np.float32).T
    w2t = np.asarray(inputs["W2"], np.float32).T
    w12 = np.ascontiguousarray(np.concatenate(
        [w1t[0:d, :], w1t[d : 2 * d, :],
         w2t[0:d, 0:d], w2t[0:d, d : 2 * d],
         w2t[d : 2 * d, 0:d], w2t[d : 2 * d, d : 2 * d]],
        axis=1,
    )).astype(bf16)
    w3t = np.ascontiguousarray(np.asarray(inputs["W3"], np.float32).T).astype(bf16)
    b1 = np.asarray(inputs["b1"], np.float32).reshape(-1, 1)
    b2 = np.asarray(inputs["b2"], np.float32).reshape(-1, 1)
    b12 = np.ascontiguousarray(
        np.concatenate([b1, b2[0:d], b2[d : 2 * d]], axis=1)
    )
    b3 = np.ascontiguousarray(np.asarray(inputs["b3"], np.float32).reshape(1, -1).astype(bf16))
    for im in in_maps:
        im.update(w12=w12, w3t=w3t, b12=b12, b3=b3)

    med = w3t.shape[1]
    cfg = dict(b_core=bc, med=med, v_dev=(SLOTS + 1) * NCK, st=st,
               wmaps=[np.arange(bc) for _ in range(n_cores)])
    return in_maps, cfg


def assemble(results, cfg):
    """Concatenate per-core outputs (identity permutation) as fp32."""
    outs = []
    for c, r in enumerate(results):
        o = r["out"].astype(np.float32)
        outs.append(o[cfg["wmaps"][c]])
    return np.concatenate(outs, axis=0)


def build_nc(cfg):
    import concourse.bass as bass
    import concourse.mybir as mybir
    import concourse.tile as tile
    from concourse import bacc

    f32 = mybir.dt.float32
    bf = mybir.dt.bfloat16
    i16 = mybir.dt.int16
    AF = mybir.ActivationFunctionType
    EQ = mybir.AluOpType.is_equal

    bc, med, v_dev = cfg["b_core"], cfg["med"], cfg["v_dev"]
    st = cfg["st"]
    NWIN, NCK = st["NWIN"], st["NCK"]
    TOT_IDX, TOT_B = st["tot_idx"], st["tot_blk"]
    n_half = med // 2
    assert n_half <= 512

    nc = bacc.Bacc("TRN2", target_bir_lowering=False, debug=False,
                   enable_asserts=False, num_devices=N_CORES)

    tbl = nc.dram_tensor("tbl", [v_dev, 2 * D], bf, kind="ExternalInput").ap()
    gidx = nc.dram_tensor("gidx", [16, TOT_IDX // 16], f32, kind="ExternalInput").ap()
    e16 = nc.dram_tensor("e16", [16, P], f32, kind="ExternalInput").ap()
    mids = nc.dram_tensor("mids", [P, 2 * TOT_B], f32, kind="ExternalInput").ap()
    iota = nc.dram_tensor("iota", [P, P], bf, kind="ExternalInput").ap()
    w12 = nc.dram_tensor("w12", [D, 6 * D], bf, kind="ExternalInput").ap()
    w3t = nc.dram_tensor("w3t", [2 * D, med], bf, kind="ExternalInput").ap()
    b12 = nc.dram_tensor("b12", [D, 3], f32, kind="ExternalInput").ap()
    b3 = nc.dram_tensor("b3", [1, med], bf, kind="ExternalInput").ap()
    out = nc.dram_tensor("out", [bc, med], bf, kind="ExternalOutput").ap()

    ops_by_win = {}
    for op in st["ops"]:
        ops_by_win.setdefault(op["win"], []).append(op)

    with tile.TileContext(nc) as tc:
        with (
            tc.tile_pool(name="const", bufs=1) as cpool,
            tc.tile_pool(name="gi", bufs=4) as gi_pool,
            tc.tile_pool(name="gst", bufs=3) as gst_pool,
            tc.tile_pool(name="mi", bufs=4) as mi_pool,
            tc.tile_pool(name="em", bufs=8) as em_pool,
            tc.tile_pool(name="gath", bufs=8) as gath_pool,
            tc.tile_pool(name="sT", bufs=8) as sT_pool,
            tc.tile_pool(name="acts", bufs=8) as act_pool,
            tc.tile_pool(name="osb", bufs=2) as out_pool,
            tc.tile_pool(name="spsum", bufs=4, space="PSUM") as s_psum,
            tc.tile_pool(name="mpsum", bufs=2, space="PSUM") as m_psum,
            tc.tile_pool(name="opsum", bufs=2, space="PSUM") as o_psum,
        ):
            iota_t = cpool.tile([P, P], bf, tag="iota")
            iota_loaded = [False]
            e16_t = cpool.tile([16, P], f32, tag="e16")
            nc.sync.dma_start(e16_t[:], e16[:, :])

            consts = {}

            def load_consts():
                # Emitted after window 0's gather ops so the first gathers
                # aren't queued behind ~1.3MB of weight preloads.
                ones = cpool.tile([1, P], bf, tag="ones")
                nc.gpsimd.memset(ones[:], 1.0)
                w12_t = cpool.tile([D, 6 * D], bf, tag="w12")
                nc.sync.dma_start(w12_t[:], w12[:, :])
                w1t_k = [w12_t[:, k * D : (k + 1) * D] for k in range(2)]
                w2t_km = {
                    (k, mm): w12_t[:, (2 + k * 2 + mm) * D : (3 + k * 2 + mm) * D]
                    for k in range(2) for mm in range(2)
                }
                w3t_k = []
                for k in range(2):
                    t = cpool.tile([D, med], bf, tag=f"w3t{k}")
                    nc.sync.dma_start(t[:], w3t[k * D : (k + 1) * D, :])
                    w3t_k.append(t)
                b12_t = cpool.tile([D, 3], f32, tag="b12")
                nc.sync.dma_start(b12_t[:], b12[:, :])
                b1_t = b12_t[:, 0:1]
                b2_t = [b12_t[:, 1:2], b12_t[:, 2:3]]
                b3_t = cpool.tile([1, med], bf, tag="b3")
                nc.sync.dma_start(b3_t[:], b3[:, :])
                consts.update(ones=ones, w1t_k=w1t_k, w2t_km=w2t_km,
                              w3t_k=w3t_k, b1_t=b1_t, b2_t=b2_t, b3_t=b3_t)

            for rep in range(cfg.get("reps", 1)):
              win_feed = {}

              def produce_feed(w):
                # stage the window's fp32 gidx [16, cols], replicate to 128
                # partitions on the PE (out[m,n] = gst[m%16,n]) and cast
                # fp32->int16 on DVE.  Called 2 windows ahead of use so the
                # in-order PE stream keeps the gather feed ahead of the
                # E-matmuls.
                wops = ops_by_win[w]
                w_idx_off = wops[0]["idx_off"]
                w_blk_off = wops[0]["blk_off"]
                w_n = sum(op["n"] for op in wops)
                w_nb = sum(op["nb"] for op in wops)
                cols = w_n // 16
                gst = gst_pool.tile([16, cols], f32, tag="gst")
                if not cfg.get("skip_gi"):
                    nc.sync.dma_start(
                        gst[:],
                        gidx[:, w_idx_off // 16 : (w_idx_off + w_n) // 16],
                    )
                gi = gi_pool.tile([P, cols], i16, tag="gi")
                # small first piece on window 0 so the head gather starts ASAP
                cuts = [0, 64] if w == 0 else [0]
                while cuts[-1] < cols:
                    cuts.append(min(cuts[-1] + 512, cols))
                for s, e in zip(cuts, cuts[1:]):
                    nn = e - s
                    rp = o_psum.tile([P, 512], f32, tag="op", name="rp")
                    nc.tensor.matmul(
                        rp[:, :nn], lhsT=e16_t[:], rhs=gst[:, s : s + nn],
                        start=True, stop=True,
                    )
                    nc.vector.tensor_copy(gi[:, s : s + nn], rp[:, :nn])
                mi = mi_pool.tile([P, 2 * w_nb], f32, tag="mi")
                nc.sync.dma_start(
                    mi[:], mids[:, 2 * w_blk_off : 2 * (w_blk_off + w_nb)]
                )
                if not iota_loaded[0]:
                    nc.sync.dma_start(iota_t[:], iota[:, :])
                    iota_loaded[0] = True
                win_feed[w] = (gi, mi, w_idx_off, w_blk_off)

              produce_feed(0)
              produce_feed(1)
              for w in range(NWIN):
                s_ps = [s_psum.tile([D, WIN], f32, tag="s", name=f"s{rep}_{w}_{i}") for i in range(4)]
                if w + 2 < NWIN:
                    produce_feed(w + 2)
                gi, mi, w_idx_off, w_blk_off = win_feed.pop(w)
                wops = ops_by_win[w]
                for op in wops:
                    n, nb = op["n"], op["nb"]
                    o16 = (op["idx_off"] - w_idx_off) // 16
                    ob0 = op["blk_off"] - w_blk_off
                    gt = gath_pool.tile([P, nb * 2 * D], bf, tag="gath")
                    if not cfg.get("skip_gather"):
                        nc.gpsimd.dma_gather(
                            out_ap=gt[:].rearrange("p (c d) -> p c d", d=2 * D),
                            in_ap=tbl[
                                op["ck"] * (SLOTS + 1) : (op["ck"] + 1) * (SLOTS + 1), :
                            ],
                            idxs_ap=gi[:, o16 : o16 + n // 16],
                            num_idxs=n,
                            num_idxs_reg=n,
                            elem_size=2 * D,
                            single_packet=False,
                        )
                    gt3 = gt[:].rearrange("p (c d) -> p c d", d=2 * D)
                    em = em_pool.tile([P, 2 * P * nb], bf, tag="em")
                    for b2 in range(2 * nb):
                        nc.vector.tensor_scalar(
                            em[:, b2 * P : (b2 + 1) * P],
                            iota_t[:],
                            mi[:, 2 * ob0 + b2 : 2 * ob0 + b2 + 1],
                            None,
                            EQ,
                        )
                    if cfg.get("skip_smm"):
                        continue
                    for b, (bg, start, stop) in enumerate(op["blocks"]):
                        nc.tensor.matmul(
                            s_ps[bg][:],
                            lhsT=gt3[:, b, 0:D],
                            rhs=em[:, 2 * b * P : (2 * b + 1) * P],
                            start=start,
                            stop=False,
                            skip_group_check=True,
                        )
                        nc.tensor.matmul(
                            s_ps[bg][:],
                            lhsT=gt3[:, b, D : 2 * D],
                            rhs=em[:, (2 * b + 1) * P : (2 * b + 2) * P],
                            start=False,
                            stop=stop,
                            skip_group_check=True,
                        )
                if not consts:
                    load_consts()
                ones = consts["ones"]
                w1t_k, w2t_km = consts["w1t_k"], consts["w2t_km"]
                w3t_k = consts["w3t_k"]
                b1_t, b2_t, b3_t = consts["b1_t"], consts["b2_t"], consts["b3_t"]
                if cfg.get("skip_mlp"):
                    continue
                # bag sums (feature-major) PSUM -> SBUF on ACT
                sT = []
                for bg in range(4):
                    t = sT_pool.tile([D, P], bf, tag="sT", name=f"sT{w}_{bg}")
                    if bg % 2 == 0:
                        nc.scalar.activation(t[:], s_ps[bg][:], AF.Copy)
                    else:
                        nc.vector.tensor_copy(t[:], s_ps[bg][:])
                    sT.append(t)

                l1 = []
                for ka, kb in ((0, 1), (2, 3)):
                    pc = m_psum.tile([P, P], f32, tag="mp")
                    nc.tensor.matmul(
                        pc[:], lhsT=w1t_k[0], rhs=sT[ka][:], start=True, stop=False
                    )
                    nc.tensor.matmul(
                        pc[:], lhsT=w1t_k[1], rhs=sT[kb][:], start=False, stop=True
                    )
                    xt = act_pool.tile([D, P], bf, tag="l1")
                    nc.scalar.activation(xt[:], pc[:], AF.Identity, bias=b1_t)
                    l1.append(xt)

                hT = []
                for mm in range(2):
                    ph = m_psum.tile([P, P], f32, tag="mp")
                    nc.tensor.matmul(
                        ph[:], lhsT=w2t_km[(0, mm)], rhs=l1[0][:],
                        start=True, stop=False,
                    )
                    nc.tensor.matmul(
                        ph[:], lhsT=w2t_km[(1, mm)], rhs=l1[1][:],
                        start=False, stop=True,
                    )
                    ht = act_pool.tile([D, P], bf, tag="l2")
                    nc.scalar.activation(ht[:], ph[:], AF.Relu, bias=b2_t[mm])
                    hT.append(ht)

                ob = out_pool.tile([P, med], bf, tag="osb")
                for h_i in range(2):
                    n0, n1 = h_i * n_half, (h_i + 1) * n_half
                    po = o_psum.tile([P, n_half], f32, tag="op", name="po")
                    nc.tensor.matmul(
                        po[:], lhsT=ones[:1, :], rhs=b3_t[:1, n0:n1],
                        start=True, stop=False,
                    )
                    nc.tensor.matmul(
                        po[:], lhsT=hT[0][:], rhs=w3t_k[0][:, n0:n1],
                        start=False, stop=False,
                    )
                    nc.tensor.matmul(
                        po[:], lhsT=hT[1][:], rhs=w3t_k[1][:, n0:n1],
                        start=False, stop=True,
                    )
                    nc.scalar.activation(ob[:, n0:n1], po[:], AF.Sigmoid)
                nc.scalar.dma_start(out[w * P : (w + 1) * P, :], ob[:])

    nc.compile()
    return nc


def kernel(**inputs) -> np.ndarray:
    from concourse.bass_utils import run_bass_kernel_spmd

    in_maps, cfg = host_prep(inputs)
    nc = build_nc(cfg)
    res = run_bass_kernel_spmd(nc, in_maps, core_ids=list(range(N_CORES)))
    return assemble(res.results, cfg)
